# revision 5
# baseline (speedup 1.0000x reference)
"""GCN (3x GraphConv + mean-pool + MLP head) on 8 Trainium2 NeuronCores.

Strategy (SPMD, one program on all 8 cores):
  - Nodes dst-sharded: core k owns padded node range [k*6272, (k+1)*6272).
  - Weight matrices replicated; degree norms folded into per-edge weights
    w_e = src_norm[src] * dst_norm[dst] carried by the scatter one-hots.
  - Per layer: project own shard (fp16 matmuls, PSUM f32) -> PE-transpose to
    node-major -> AllGather into a DRAM table [50176, 128] fp16 -> gather
    each edge's source row via indirect DMA (128 rows/instruction, edges
    sorted by dst and batch-padded per 128-dst window so the schedule is
    identical on every core) -> scatter-add via matmul with an on-chip
    weighted one-hot (iota==dst_local)*w_e accumulating in PSUM per window.
  - Layer 3's scatter is fused with dgl.mean_nodes: its one-hot is the
    8-wide graph-membership matrix scaled by w_e/cnt_g, so the whole last
    aggregation lands directly in a [128,8] pooled accumulator; partials
    are AllReduced and the tiny MLP head runs replicated on every core.
"""

import sys

sys.path.insert(0, "/opt/trn_rl_repo")

import numpy as np

import concourse.bass as bass
import concourse.mybir as mybir
import concourse.tile as tile
import bass_rust
from concourse.bass_utils import run_bass_kernel_spmd

F32 = mybir.dt.float32
F16 = mybir.dt.float16
I32 = mybir.dt.int32

NC = 8          # cores
D = 128         # feature dim (== partition width)
G = 8           # graphs
OUT = 10
NEG = 0.01      # LeakyReLU slope
WIN = 128       # dst window per PSUM tile

_split_ctr = [0]


def split_multiwaits(nc):
    """This walrus encodes at most ONE sync-wait per instruction; hoist
    extra waits into preceding EventSemaphore ops on the same engine."""
    for f in nc.m.functions:
        for blk in f.blocks:
            insts = list(blk.instructions)
            new, changed = [], False
            for inst in insts:
                si = inst.sync_info
                if si is not None and len(si.on_wait) > 1:
                    waits = list(si.on_wait)
                    for w in waits[:-1]:
                        _split_ctr[0] += 1
                        es = mybir.InstEventSemaphore(
                            name=f"mwsplit_{_split_ctr[0]}", ins=[], outs=[])
                        es.engine = inst.engine
                        es.sync_info = bass_rust.SyncInfo(on_wait=[w], on_update=[])
                        new.append(es)
                    si.on_wait = waits[-1:]
                    changed = True
                new.append(inst)
            if changed:
                blk.instructions = new


# ---------------------------------------------------------------- host prep

def _prep(x, src, dst, gid, n_nodes, shard):
    """Build per-core gather indices + one-hot scalar schedules."""
    npad = NC * shard
    nwin = shard // WIN
    out_deg = np.bincount(src, minlength=n_nodes)
    in_deg = np.bincount(dst, minlength=n_nodes)
    snorm = np.clip(out_deg, 1, None).astype(np.float32) ** -0.5
    dnorm = np.clip(in_deg, 1, None).astype(np.float32) ** -0.5
    we = snorm[src] * dnorm[dst]
    cnt = np.bincount(gid, minlength=G).astype(np.float32)
    cinv = (1.0 / np.clip(cnt, 1, None)).astype(np.float32)

    cores = []
    for k in range(NC):
        sel = (dst >= k * shard) & (dst < (k + 1) * shard)
        e_src = src[sel].astype(np.int64)
        e_slot = (dst[sel] - k * shard).astype(np.int64)
        e_w = we[sel].astype(np.float32)
        e_g = gid[np.minimum(dst[sel], n_nodes - 1)].astype(np.int64)
        o = np.argsort(e_slot, kind="stable")
        cores.append((e_src[o], e_slot[o], e_w[o], e_g[o]))

    # per-window batch counts, padded to the max over cores (SPMD-uniform)
    wcnt = np.zeros((NC, nwin), np.int64)
    for k in range(NC):
        wcnt[k] = np.bincount(cores[k][1] // WIN, minlength=nwin)
    bw = np.maximum(1, -(-wcnt.max(axis=0) // 128)).astype(np.int64)  # ceil
    offs = np.concatenate([[0], np.cumsum(bw)])
    nbw = int(offs[-1])

    nb3 = max(1, max(-(-len(c[0]) // 128) for c in cores))

    per_core = []
    for k in range(NC):
        e_src, e_slot, e_w, e_g = cores[k]
        gidx = np.zeros((128, nbw), np.int32)
        dstloc = np.full((128, nbw), -1.0, np.float32)
        wcol = np.zeros((128, nbw), np.float32)
        win = e_slot // WIN
        pos = np.zeros(len(e_src), np.int64)
        for w in range(nwin):
            m = win == w
            pos[m] = np.arange(m.sum())
        col = offs[win] + pos // 128
        row = pos % 128
        gidx[row, col] = e_src
        dstloc[row, col] = (e_slot - win * WIN).astype(np.float32)
        wcol[row, col] = e_w

        gidx3 = np.zeros((128, nb3), np.int32)
        oh3 = np.zeros((128, nb3 * G), np.float32)
        p = np.arange(len(e_src))
        b3, r3 = p // 128, p % 128
        gidx3[r3, b3] = e_src
        oh3[r3, b3 * G + e_g] = e_w * cinv[e_g]

        per_core.append(dict(gidx=gidx, dstloc=dstloc, wcol=wcol,
                             gidx3=gidx3, oh3=oh3))
    return per_core, bw, offs, nbw, nb3


# ---------------------------------------------------------------- program

def build_program(n_nodes, shard, nbw, nb3, bw, offs, n_edges_pad):
    npad = NC * shard
    nwin = shard // WIN
    nchunk = shard // 128  # transpose/eviction chunks

    nc = bass.Bass("TRN2", target_bir_lowering=False, debug=False,
                   num_devices=NC)

    # ---- external inputs
    xT = nc.dram_tensor("xT", [128, shard], F32, kind="ExternalInput")
    wg = [nc.dram_tensor(f"w{l}", [128, 128], F32, kind="ExternalInput")
          for l in (1, 2, 3)]
    bg = [nc.dram_tensor(f"b{l}", [128, 1], F32, kind="ExternalInput")
          for l in (1, 2, 3)]
    wc1 = nc.dram_tensor("wc1", [128, 512], F32, kind="ExternalInput")
    bc1 = nc.dram_tensor("bc1", [128, 4], F32, kind="ExternalInput")
    wc2 = nc.dram_tensor("wc2", [128, 4 * 256], F32, kind="ExternalInput")
    bc2 = nc.dram_tensor("bc2", [128, 2], F32, kind="ExternalInput")
    wc3 = nc.dram_tensor("wc3", [128, 2 * OUT], F32, kind="ExternalInput")
    bc3 = nc.dram_tensor("bc3", [128, 1], F32, kind="ExternalInput")
    gidx_in = nc.dram_tensor("gidx", [128, nbw], I32, kind="ExternalInput")
    dstloc_in = nc.dram_tensor("dstloc", [128, nbw], F32, kind="ExternalInput")
    wcol_in = nc.dram_tensor("wcol", [128, nbw], F32, kind="ExternalInput")
    gidx3_in = nc.dram_tensor("gidx3", [128, nb3], I32, kind="ExternalInput")
    oh3_in = nc.dram_tensor("oh3", [128, nb3 * G], F32, kind="ExternalInput")
    iota_in = nc.dram_tensor("iota", [128, 128], F32, kind="ExternalInput")
    ident_in = nc.dram_tensor("ident", [128, 128], F32, kind="ExternalInput")
    out_d = nc.dram_tensor("out", [OUT, G], F32, kind="ExternalOutput")

    # ---- internal DRAM
    bounce = [nc.dram_tensor(f"bnc{l}", [shard, 128], F16) for l in range(3)]
    table = [nc.dram_tensor(f"tbl{l}", [npad, 128], F16) for l in range(3)]
    pool_in = nc.dram_tensor("pool_in", [128, G], F32)
    pool_out = nc.dram_tensor("pool_out", [128, G], F32)

    NGBUF = 48

    with tile.TileContext(nc) as tc:
        with tc.tile_pool(name="sb", bufs=1) as sb, \
             tc.tile_pool(name="ps", bufs=4, space="PSUM") as ps, \
             tc.tile_pool(name="ps2", bufs=2, space="PSUM") as ps2:

            # ---------- load constants / schedules
            iota_t = sb.tile([128, 128], F32, name="iota_t")
            nc.sync.dma_start(out=iota_t[:], in_=iota_in[:])
            ident_t = sb.tile([128, 128], F32, name="ident_t")
            nc.sync.dma_start(out=ident_t[:], in_=ident_in[:])
            gidx_t = sb.tile([128, nbw], I32, name="gidx_t")
            nc.sync.dma_start(out=gidx_t[:], in_=gidx_in[:])
            dstloc_t = sb.tile([128, nbw], F32, name="dstloc_t")
            nc.sync.dma_start(out=dstloc_t[:], in_=dstloc_in[:])
            wcol_t = sb.tile([128, nbw], F32, name="wcol_t")
            nc.sync.dma_start(out=wcol_t[:], in_=wcol_in[:])
            gidx3_t = sb.tile([128, nb3], I32, name="gidx3_t")
            nc.sync.dma_start(out=gidx3_t[:], in_=gidx3_in[:])
            oh3_f32 = sb.tile([128, nb3 * G], F32, name="oh3_f32")
            nc.sync.dma_start(out=oh3_f32[:], in_=oh3_in[:])
            oh3_t = sb.tile([128, nb3 * G], F16, name="oh3_t")
            nc.vector.tensor_copy(out=oh3_t[:], in_=oh3_f32[:])

            w_t, b_t = [], []
            for l in range(3):
                wf = sb.tile([128, 128], F32, name=f"wf{l}")
                nc.sync.dma_start(out=wf[:], in_=wg[l][:])
                wh = sb.tile([128, 128], F16, name=f"wh{l}")
                nc.vector.tensor_copy(out=wh[:], in_=wf[:])
                w_t.append(wh)
                bt = sb.tile([128, 1], F32, name=f"bt{l}")
                nc.sync.dma_start(out=bt[:], in_=bg[l][:])
                b_t.append(bt)

            # ---------- layer-0 state: fp16 copy of xT
            xT_t = sb.tile([128, shard], F32, name="xT_t")
            nc.sync.dma_start(out=xT_t[:], in_=xT[:])
            state = sb.tile([128, shard], F16, name="state0")
            nc.vector.tensor_copy(out=state[:], in_=xT_t[:])

            def project_and_allgather(l, state_t):
                """x_proj^T = W_l.T @ state^T; transpose to node-major;
                AllGather into table[l]."""
                xp = sb.tile([128, shard], F32, name=f"xp{l}", tag="xpf")
                c = 0
                while c < shard:
                    w = min(512, shard - c)
                    pt = ps.tile([128, w], F32, space="PSUM",
                                 name=f"pj{l}_{c}", tag="a")
                    nc.tensor.matmul(out=pt[:], lhsT=w_t[l][:],
                                     rhs=state_t[:, c:c + w],
                                     start=True, stop=True)
                    nc.vector.tensor_copy(out=xp[:, c:c + w], in_=pt[:])
                    c += w
                xpnm = sb.tile([128, nchunk, 128], F16, name=f"xpnm{l}")
                for t in range(nchunk):
                    tp = ps.tile([128, 128], F32, space="PSUM",
                                 name=f"tp{l}_{t}", tag="a")
                    nc.tensor.transpose(out=tp[:],
                                        in_=xp[:, 128 * t:128 * (t + 1)],
                                        identity=ident_t[:])
                    nc.vector.tensor_copy(out=xpnm[:, t, :], in_=tp[:])
                nc.sync.dma_start(
                    out=bounce[l].ap().rearrange("(c p) f -> p c f", p=128),
                    in_=xpnm[:])
                nc.gpsimd.collective_compute(
                    "AllGather", mybir.AluOpType.bypass,
                    replica_groups=[list(range(NC))],
                    ins=[bounce[l].ap()], outs=[table[l].ap()])

            def gather_batch(l, col, src_idx_t):
                m = sb.tile([128, 128], F16, name=f"m{l}_{col}",
                            tag=f"g{col % NGBUF}")
                nc.gpsimd.indirect_dma_start(
                    out=m[:], out_offset=None, in_=table[l][:],
                    in_offset=bass.IndirectOffsetOnAxis(
                        ap=src_idx_t[:, col:col + 1], axis=0))
                return m

            # ---------- layers 1-2: windowed scatter
            for l in range(2):
                project_and_allgather(l, state)
                nstate = sb.tile([128, shard], F16, name=f"state{l + 1}")
                for w in range(nwin):
                    pw = ps.tile([128, WIN], F32, space="PSUM",
                                 name=f"pw{l}_{w}", tag="a")
                    nb = int(bw[w])
                    for j in range(nb):
                        col = int(offs[w]) + j
                        m = gather_batch(l, col, gidx_t)
                        oh = sb.tile([128, WIN], F16, name=f"oh{l}_{col}",
                                     tag=f"oh{col % 8}")
                        nc.vector.tensor_scalar(
                            out=oh[:], in0=iota_t[:],
                            scalar1=dstloc_t[:, col:col + 1],
                            scalar2=wcol_t[:, col:col + 1],
                            op0=mybir.AluOpType.is_equal,
                            op1=mybir.AluOpType.mult)
                        nc.tensor.matmul(out=pw[:], lhsT=m[:], rhs=oh[:],
                                         start=(j == 0), stop=(j == nb - 1))
                    nc.vector.tensor_scalar(
                        out=nstate[:, WIN * w:WIN * (w + 1)], in0=pw[:],
                        scalar1=b_t[l][:, 0:1], scalar2=None,
                        op0=mybir.AluOpType.add)
                state = nstate

            # ---------- layer 3 fused with mean-pool
            project_and_allgather(2, state)
            pp = ps2.tile([128, G], F32, space="PSUM", name="pp", tag="b")
            for b3 in range(nb3):
                m = gather_batch(2, b3, gidx3_t)
                nc.tensor.matmul(out=pp[:], lhsT=m[:],
                                 rhs=oh3_t[:, G * b3:G * (b3 + 1)],
                                 start=(b3 == 0), stop=(b3 == nb3 - 1))
            ppool = sb.tile([128, G], F32, name="ppool")
            nc.vector.tensor_copy(out=ppool[:], in_=pp[:])
            nc.sync.dma_start(out=pool_in[:], in_=ppool[:])
            nc.gpsimd.collective_compute(
                "AllReduce", mybir.AluOpType.add,
                replica_groups=[list(range(NC))],
                ins=[pool_in.ap()], outs=[pool_out.ap()])
            pooled_f = sb.tile([128, G], F32, name="pooled_f")
            nc.sync.dma_start(out=pooled_f[:], in_=pool_out[:])
            # + b3 (bias excluded from per-core partials), then to fp16
            nc.vector.tensor_scalar(out=pooled_f[:], in0=pooled_f[:],
                                    scalar1=b_t[2][:, 0:1], scalar2=None,
                                    op0=mybir.AluOpType.add)
            pooled = sb.tile([128, G], F16, name="pooled")
            nc.vector.tensor_copy(out=pooled[:], in_=pooled_f[:])

            # ---------- MLP head (replicated)
            _hctr = [0]

            def lrelu_evict(psrc, bias_tile, bias_c, dst16, nrows=128):
                _hctr[0] += 1
                t1 = sb.tile([128, G], F32, name=f"t1_{_hctr[0]}", tag="h1")
                nc.vector.tensor_scalar(out=t1[:nrows], in0=psrc[:nrows],
                                        scalar1=bias_tile[:nrows, bias_c:bias_c + 1],
                                        scalar2=None,
                                        op0=mybir.AluOpType.add)
                t2 = sb.tile([128, G], F32, name=f"t2_{_hctr[0]}", tag="h2")
                nc.vector.tensor_scalar(out=t2[:nrows], in0=t1[:nrows],
                                        scalar1=NEG, scalar2=None,
                                        op0=mybir.AluOpType.mult)
                nc.vector.tensor_tensor(out=dst16[:nrows], in0=t1[:nrows],
                                        in1=t2[:nrows],
                                        op=mybir.AluOpType.max)

            wc1_f = sb.tile([128, 512], F32, name="wc1_f")
            nc.sync.dma_start(out=wc1_f[:], in_=wc1[:])
            wc1_h = sb.tile([128, 512], F16, name="wc1_h")
            nc.vector.tensor_copy(out=wc1_h[:], in_=wc1_f[:])
            bc1_t = sb.tile([128, 4], F32, name="bc1_t")
            nc.sync.dma_start(out=bc1_t[:], in_=bc1[:])
            wc2_f = sb.tile([128, 4 * 256], F32, name="wc2_f")
            nc.sync.dma_start(out=wc2_f[:], in_=wc2[:])
            wc2_h = sb.tile([128, 4 * 256], F16, name="wc2_h")
            nc.vector.tensor_copy(out=wc2_h[:], in_=wc2_f[:])
            bc2_t = sb.tile([128, 2], F32, name="bc2_t")
            nc.sync.dma_start(out=bc2_t[:], in_=bc2[:])
            wc3_f = sb.tile([128, 2 * OUT], F32, name="wc3_f")
            nc.sync.dma_start(out=wc3_f[:], in_=wc3[:])
            wc3_h = sb.tile([128, 2 * OUT], F16, name="wc3_h")
            nc.vector.tensor_copy(out=wc3_h[:], in_=wc3_f[:])
            bc3_t = sb.tile([128, 1], F32, name="bc3_t")
            nc.sync.dma_start(out=bc3_t[:], in_=bc3[:])

            z1 = sb.tile([128, 4 * G], F16, name="z1")
            for c in range(4):
                ph = ps2.tile([128, G], F32, space="PSUM",
                              name=f"ph1_{c}", tag="b")
                nc.tensor.matmul(out=ph[:], lhsT=wc1_h[:, 128 * c:128 * (c + 1)],
                                 rhs=pooled[:], start=True, stop=True)
                lrelu_evict(ph, bc1_t, c, z1[:, G * c:G * (c + 1)])
            z2 = sb.tile([128, 2 * G], F16, name="z2")
            for jj in range(2):
                ph = ps2.tile([128, G], F32, space="PSUM",
                              name=f"ph2_{jj}", tag="b")
                for c in range(4):
                    nc.tensor.matmul(
                        out=ph[:],
                        lhsT=wc2_h[:, 256 * c + 128 * jj:256 * c + 128 * (jj + 1)],
                        rhs=z1[:, G * c:G * (c + 1)],
                        start=(c == 0), stop=(c == 3))
                lrelu_evict(ph, bc2_t, jj, z2[:, G * jj:G * (jj + 1)])
            ph3 = ps2.tile([128, G], F32, space="PSUM", name="ph3", tag="b")
            for c in range(2):
                nc.tensor.matmul(out=ph3[:OUT],
                                 lhsT=wc3_h[:, OUT * c:OUT * (c + 1)],
                                 rhs=z2[:, G * c:G * (c + 1)],
                                 start=(c == 0), stop=(c == 1))
            zout = sb.tile([128, G], F32, name="zout")
            lrelu_evict(ph3, bc3_t, 0, zout, nrows=OUT)
            nc.sync.dma_start(out=out_d[:], in_=zout[:OUT])

    split_multiwaits(nc)
    return nc


# ---------------------------------------------------------------- driver

def run_gcn(x, src, dst, gid, W1, b1, W2, b2, W3, b3,
            Wc1, bc1, Wc2, bc2, Wc3, bc3, n_nodes, shard,
            trace=False, tmpdir=None):
    x = np.asarray(x, np.float32)
    src = np.asarray(src).astype(np.int64)
    dst = np.asarray(dst).astype(np.int64)
    gid = np.asarray(gid).astype(np.int64)
    npad = NC * shard

    per_core, bw, offs, nbw, nb3 = _prep(x, src, dst, gid, n_nodes, shard)
    nc = build_program(n_nodes, shard, nbw, nb3, bw, offs, len(src))

    xpad = np.zeros((npad, D), np.float32)
    xpad[:n_nodes] = x
    iota = np.tile(np.arange(128, dtype=np.float32), (128, 1))
    ident = np.eye(128, dtype=np.float32)

    def colvec(v, n=128):
        a = np.zeros((n, 1), np.float32)
        a[:len(v), 0] = v
        return a

    bc1_a = np.asarray(bc1, np.float32).reshape(4, 128).T.copy()
    bc2_a = np.asarray(bc2, np.float32).reshape(2, 128).T.copy()
    wc2_a = np.concatenate(
        [np.asarray(Wc2, np.float32)[128 * c:128 * (c + 1), :] for c in range(4)],
        axis=1)
    wc3_a = np.concatenate(
        [np.asarray(Wc3, np.float32)[128 * c:128 * (c + 1), :] for c in range(2)],
        axis=1)

    in_maps = []
    for k in range(NC):
        pc = per_core[k]
        in_maps.append({
            "xT": np.ascontiguousarray(xpad[k * shard:(k + 1) * shard].T),
            "w1": np.asarray(W1, np.float32), "b1": colvec(np.asarray(b1)),
            "w2": np.asarray(W2, np.float32), "b2": colvec(np.asarray(b2)),
            "w3": np.asarray(W3, np.float32), "b3": colvec(np.asarray(b3)),
            "wc1": np.asarray(Wc1, np.float32), "bc1": bc1_a,
            "wc2": wc2_a, "bc2": bc2_a,
            "wc3": wc3_a, "bc3": colvec(np.asarray(bc3)),
            "gidx": pc["gidx"], "dstloc": pc["dstloc"], "wcol": pc["wcol"],
            "gidx3": pc["gidx3"], "oh3": pc["oh3"],
            "iota": iota, "ident": ident,
        })

    res = run_bass_kernel_spmd(nc, in_maps, core_ids=list(range(NC)),
                               trace=trace, tmpdir=tmpdir)
    out = res.results[0]["out"].T.astype(np.float32).copy()  # [G, OUT]
    return out, res


def kernel(**inputs):
    out, _ = run_gcn(
        inputs["inputs"], inputs["src"], inputs["dst"], inputs["graph_id"],
        inputs["W1"], inputs["b1"], inputs["W2"], inputs["b2"],
        inputs["W3"], inputs["b3"],
        inputs["Wc1"], inputs["bc1"], inputs["Wc2"], inputs["bc2"],
        inputs["Wc3"], inputs["bc3"],
        n_nodes=50000, shard=6272)
    return out


# revision 6
# speedup vs baseline: 1.0497x; 1.0497x over previous
"""GCN (3x GraphConv + mean-pool + MLP head) on 8 Trainium2 NeuronCores.

Strategy (SPMD, one program on all 8 cores):
  - Nodes are assigned to (core, 256-dst-window) buckets balanced by
    in-degree, so every core/window sees ~equal edge counts and the
    SPMD-uniform batch schedule wastes almost no padding.
  - Weight matrices replicated; degree norms folded into per-edge weights
    w_e = src_norm[src] * dst_norm[dst] carried by the scatter one-hots.
  - Per layer: project own shard (fp16 matmuls, PSUM f32) -> PE-transpose to
    node-major -> AllGather (2 pipelined chunks) into a DRAM table
    [50176, 128] fp16 -> gather each edge's source row via indirect DMA
    (128 rows/instruction, edges grouped per dst window) -> scatter-add via
    matmul with an on-chip weighted one-hot (iota==dst_local)*w_e
    accumulating in PSUM per window.
  - Layer 3's scatter is fused with dgl.mean_nodes: its one-hot is the
    8-wide graph-membership matrix scaled by w_e/cnt_g, so the whole last
    aggregation lands directly in a [128,8] pooled accumulator; partials
    are AllReduced and the tiny MLP head runs replicated on every core.
"""

import heapq
import sys

sys.path.insert(0, "/opt/trn_rl_repo")

import numpy as np

import concourse.bass as bass
import concourse.mybir as mybir
import concourse.tile as tile
import bass_rust
from concourse.bass_utils import run_bass_kernel_spmd

F32 = mybir.dt.float32
F16 = mybir.dt.float16
I32 = mybir.dt.int32

NC = 8          # cores
D = 128         # feature dim (== partition width)
G = 8           # graphs
OUT = 10
NEG = 0.01      # LeakyReLU slope
WIN = 256       # dst window per PSUM tile

_split_ctr = [0]


def split_multiwaits(nc):
    """This walrus encodes at most ONE sync-wait per instruction; hoist
    extra waits into preceding EventSemaphore ops on the same engine."""
    for f in nc.m.functions:
        for blk in f.blocks:
            insts = list(blk.instructions)
            new, changed = [], False
            for inst in insts:
                si = inst.sync_info
                if si is not None and len(si.on_wait) > 1:
                    waits = list(si.on_wait)
                    for w in waits[:-1]:
                        _split_ctr[0] += 1
                        es = mybir.InstEventSemaphore(
                            name=f"mwsplit_{_split_ctr[0]}", ins=[], outs=[])
                        es.engine = inst.engine
                        es.sync_info = bass_rust.SyncInfo(on_wait=[w], on_update=[])
                        new.append(es)
                    si.on_wait = waits[-1:]
                    changed = True
                new.append(inst)
            if changed:
                blk.instructions = new


def _windows(shard):
    """List of (start, width) dst windows covering [0, shard)."""
    out = []
    c = 0
    while c < shard:
        w = min(WIN, shard - c)
        out.append((c, w))
        c += w
    return out


# ---------------------------------------------------------------- host prep

def _prep(x, src, dst, gid, n_nodes, shard):
    wins = _windows(shard)
    nwin = len(wins)
    nchunk = shard // 128
    half_a = (nchunk // 2) * 128          # table chunk split (128-aligned)

    out_deg = np.bincount(src, minlength=n_nodes)
    in_deg = np.bincount(dst, minlength=n_nodes)
    snorm = np.clip(out_deg, 1, None).astype(np.float32) ** -0.5
    dnorm = np.clip(in_deg, 1, None).astype(np.float32) ** -0.5
    we = snorm[src] * dnorm[dst]
    cnt = np.bincount(gid, minlength=G).astype(np.float32)
    cinv = (1.0 / np.clip(cnt, 1, None)).astype(np.float32)

    # --- balanced node -> (core, window) assignment by in-degree
    order = np.argsort(-in_deg, kind="stable")
    heap = [(0.0, k * nwin + w) for k in range(NC) for w in range(nwin)]
    heapq.heapify(heap)
    fill = [[0] * nwin for _ in range(NC)]
    node_core = np.empty(n_nodes, np.int32)
    node_slot = np.empty(n_nodes, np.int32)
    for n in order:
        while True:
            load, b = heapq.heappop(heap)
            k, w = divmod(b, nwin)
            if fill[k][w] < wins[w][1]:
                break
        node_core[n] = k
        node_slot[n] = wins[w][0] + fill[k][w]
        fill[k][w] += 1
        heapq.heappush(heap, (load + float(in_deg[n]), b))

    # global gather-table id (matches the 2-chunk AllGather layout)
    s = node_slot.astype(np.int64)
    half_b = shard - half_a
    in_b = s >= half_a
    gid_tab = np.where(~in_b, node_core.astype(np.int64) * half_a + s,
                       NC * half_a + node_core.astype(np.int64) * half_b
                       + (s - half_a)).astype(np.int32)

    e_core = node_core[dst]
    e_slot_all = node_slot[dst]

    wcnt = np.zeros((NC, nwin), np.int64)
    for k in range(NC):
        m = e_core == k
        wcnt[k] = np.bincount(e_slot_all[m] // WIN, minlength=nwin)
    bw = np.maximum(1, -(-wcnt.max(axis=0) // 128)).astype(np.int64)
    offs = np.concatenate([[0], np.cumsum(bw)])
    nbw = int(offs[-1])
    nb3 = max(1, max(-(-int((e_core == k).sum()) // 128) for k in range(NC)))

    per_core = []
    for k in range(NC):
        m = e_core == k
        e_src = src[m].astype(np.int64)
        e_slot = e_slot_all[m].astype(np.int64)
        e_w = we[m].astype(np.float32)
        e_g = gid[dst[m]].astype(np.int64)
        o = np.argsort(e_slot, kind="stable")
        e_src, e_slot, e_w, e_g = e_src[o], e_slot[o], e_w[o], e_g[o]

        gidx = np.zeros((128, nbw), np.int32)
        dstloc = np.full((128, nbw), -1.0, np.float32)
        wcol = np.zeros((128, nbw), np.float32)
        win = e_slot // WIN
        pos = np.zeros(len(e_src), np.int64)
        for w in range(nwin):
            mm = win == w
            pos[mm] = np.arange(mm.sum())
        col = offs[win] + pos // 128
        row = pos % 128
        gidx[row, col] = gid_tab[e_src]
        dstloc[row, col] = (e_slot - win * WIN).astype(np.float32)
        wcol[row, col] = e_w

        gidx3 = np.zeros((128, nb3), np.int32)
        oh3 = np.zeros((128, nb3 * G), np.float32)
        p = np.arange(len(e_src))
        b3, r3 = p // 128, p % 128
        gidx3[r3, b3] = gid_tab[e_src]
        oh3[r3, b3 * G + e_g] = e_w * cinv[e_g]

        per_core.append(dict(gidx=gidx, dstloc=dstloc, wcol=wcol,
                             gidx3=gidx3, oh3=oh3))
    return per_core, node_core, node_slot, bw, offs, nbw, nb3


# ---------------------------------------------------------------- program

def build_program(shard, nbw, nb3, bw, offs):
    npad = NC * shard
    wins = _windows(shard)
    nwin = len(wins)
    nchunk = shard // 128
    half_a = (nchunk // 2) * 128
    chunks = [(0, half_a), (half_a, shard - half_a)]  # (start, size)

    nc = bass.Bass("TRN2", target_bir_lowering=False, debug=False,
                   num_devices=NC)

    xT = nc.dram_tensor("xT", [128, shard], F32, kind="ExternalInput")
    wg = [nc.dram_tensor(f"w{l}", [128, 128], F32, kind="ExternalInput")
          for l in (1, 2, 3)]
    bg = [nc.dram_tensor(f"b{l}", [128, 1], F32, kind="ExternalInput")
          for l in (1, 2, 3)]
    wc1 = nc.dram_tensor("wc1", [128, 512], F32, kind="ExternalInput")
    bc1 = nc.dram_tensor("bc1", [128, 4], F32, kind="ExternalInput")
    wc2 = nc.dram_tensor("wc2", [128, 4 * 256], F32, kind="ExternalInput")
    bc2 = nc.dram_tensor("bc2", [128, 2], F32, kind="ExternalInput")
    wc3 = nc.dram_tensor("wc3", [128, 2 * OUT], F32, kind="ExternalInput")
    bc3 = nc.dram_tensor("bc3", [128, 1], F32, kind="ExternalInput")
    gidx_in = nc.dram_tensor("gidx", [128, nbw], I32, kind="ExternalInput")
    dstloc_in = nc.dram_tensor("dstloc", [128, nbw], F32, kind="ExternalInput")
    wcol_in = nc.dram_tensor("wcol", [128, nbw], F32, kind="ExternalInput")
    gidx3_in = nc.dram_tensor("gidx3", [128, nb3], I32, kind="ExternalInput")
    oh3_in = nc.dram_tensor("oh3", [128, nb3 * G], F32, kind="ExternalInput")
    iota_in = nc.dram_tensor("iota", [128, WIN], F32, kind="ExternalInput")
    ident_in = nc.dram_tensor("ident", [128, 128], F32, kind="ExternalInput")
    out_d = nc.dram_tensor("out", [OUT, G], F32, kind="ExternalOutput")

    bounce = [nc.dram_tensor(f"bnc{l}", [shard, 128], F16) for l in range(3)]
    table = [nc.dram_tensor(f"tbl{l}", [npad, 128], F16) for l in range(3)]
    pool_in = nc.dram_tensor("pool_in", [128, G], F32)
    pool_out = nc.dram_tensor("pool_out", [128, G], F32)

    NGBUF = 80

    with tile.TileContext(nc) as tc:
        with tc.tile_pool(name="sb", bufs=1) as sb, \
             tc.tile_pool(name="ps", bufs=4, space="PSUM") as ps, \
             tc.tile_pool(name="ps2", bufs=2, space="PSUM") as ps2:

            iota_t = sb.tile([128, WIN], F32, name="iota_t")
            nc.sync.dma_start(out=iota_t[:], in_=iota_in[:])
            ident_t = sb.tile([128, 128], F32, name="ident_t")
            nc.sync.dma_start(out=ident_t[:], in_=ident_in[:])
            gidx_t = sb.tile([128, nbw], I32, name="gidx_t")
            nc.sync.dma_start(out=gidx_t[:], in_=gidx_in[:])
            dstloc_t = sb.tile([128, nbw], F32, name="dstloc_t")
            nc.sync.dma_start(out=dstloc_t[:], in_=dstloc_in[:])
            wcol_t = sb.tile([128, nbw], F32, name="wcol_t")
            nc.sync.dma_start(out=wcol_t[:], in_=wcol_in[:])
            gidx3_t = sb.tile([128, nb3], I32, name="gidx3_t")
            nc.sync.dma_start(out=gidx3_t[:], in_=gidx3_in[:])
            oh3_f32 = sb.tile([128, nb3 * G], F32, name="oh3_f32")
            nc.sync.dma_start(out=oh3_f32[:], in_=oh3_in[:])
            oh3_t = sb.tile([128, nb3 * G], F16, name="oh3_t")
            nc.vector.tensor_copy(out=oh3_t[:], in_=oh3_f32[:])

            w_t, b_t = [], []
            for l in range(3):
                wf = sb.tile([128, 128], F32, name=f"wf{l}")
                nc.sync.dma_start(out=wf[:], in_=wg[l][:])
                wh = sb.tile([128, 128], F16, name=f"wh{l}")
                nc.vector.tensor_copy(out=wh[:], in_=wf[:])
                w_t.append(wh)
                bt = sb.tile([128, 1], F32, name=f"bt{l}")
                nc.sync.dma_start(out=bt[:], in_=bg[l][:])
                b_t.append(bt)

            xT_t = sb.tile([128, shard], F32, name="xT_t")
            nc.sync.dma_start(out=xT_t[:], in_=xT[:])
            state = sb.tile([128, shard], F16, name="state0")
            nc.vector.tensor_copy(out=state[:], in_=xT_t[:])

            def project_and_allgather(l, state_t):
                """Per table chunk: project, transpose to node-major, DMA to
                bounce, AllGather -- chunk 0's collective overlaps chunk 1's
                transposes."""
                for (cst, csz) in chunks:
                    xp = sb.tile([128, csz], F32, name=f"xp{l}_{cst}",
                                 tag="xpf")
                    c = 0
                    while c < csz:
                        w = min(512, csz - c)
                        pt = ps.tile([128, w], F32, space="PSUM",
                                     name=f"pj{l}_{cst}_{c}", tag="a")
                        nc.tensor.matmul(out=pt[:], lhsT=w_t[l][:],
                                         rhs=state_t[:, cst + c:cst + c + w],
                                         start=True, stop=True)
                        nc.vector.tensor_copy(out=xp[:, c:c + w], in_=pt[:])
                        c += w
                    nck = csz // 128
                    xpnm = sb.tile([128, nck, 128], F16,
                                   name=f"xpnm{l}_{cst}", tag="xpnm")
                    for t in range(nck):
                        tp = ps.tile([128, 128], F32, space="PSUM",
                                     name=f"tp{l}_{cst}_{t}", tag="a")
                        nc.tensor.transpose(out=tp[:],
                                            in_=xp[:, 128 * t:128 * (t + 1)],
                                            identity=ident_t[:])
                        nc.vector.tensor_copy(out=xpnm[:, t, :], in_=tp[:])
                    nc.sync.dma_start(
                        out=bounce[l][cst:cst + csz, :]
                            .rearrange("(c p) f -> p c f", p=128),
                        in_=xpnm[:])
                    tb0 = NC * cst
                    nc.gpsimd.collective_compute(
                        "AllGather", mybir.AluOpType.bypass,
                        replica_groups=[list(range(NC))],
                        ins=[bounce[l][cst:cst + csz, :]],
                        outs=[table[l][tb0:tb0 + NC * csz, :]])

            def gather_batch(l, col, src_idx_t):
                m = sb.tile([128, 128], F16, name=f"m{l}_{col}",
                            tag=f"g{col % NGBUF}")
                nc.gpsimd.indirect_dma_start(
                    out=m[:], out_offset=None, in_=table[l][:],
                    in_offset=bass.IndirectOffsetOnAxis(
                        ap=src_idx_t[:, col:col + 1], axis=0))
                return m

            for l in range(2):
                project_and_allgather(l, state)
                nstate = sb.tile([128, shard], F16, name=f"state{l + 1}")
                for w in range(nwin):
                    wst, ww = wins[w]
                    pw = ps.tile([128, ww], F32, space="PSUM",
                                 name=f"pw{l}_{w}", tag="a")
                    nb = int(bw[w])
                    for j in range(nb):
                        col = int(offs[w]) + j
                        m = gather_batch(l, col, gidx_t)
                        oh = sb.tile([128, ww], F16, name=f"oh{l}_{col}",
                                     tag=f"oh{col % 16}")
                        nc.vector.tensor_scalar(
                            out=oh[:], in0=iota_t[:, :ww],
                            scalar1=dstloc_t[:, col:col + 1],
                            scalar2=wcol_t[:, col:col + 1],
                            op0=mybir.AluOpType.is_equal,
                            op1=mybir.AluOpType.mult)
                        nc.tensor.matmul(out=pw[:], lhsT=m[:], rhs=oh[:],
                                         start=(j == 0), stop=(j == nb - 1))
                    nc.vector.tensor_scalar(
                        out=nstate[:, wst:wst + ww], in0=pw[:],
                        scalar1=b_t[l][:, 0:1], scalar2=None,
                        op0=mybir.AluOpType.add)
                state = nstate

            # ---------- layer 3 fused with mean-pool
            project_and_allgather(2, state)
            pp = ps2.tile([128, G], F32, space="PSUM", name="pp", tag="b")
            for b3 in range(nb3):
                m = gather_batch(2, b3, gidx3_t)
                nc.tensor.matmul(out=pp[:], lhsT=m[:],
                                 rhs=oh3_t[:, G * b3:G * (b3 + 1)],
                                 start=(b3 == 0), stop=(b3 == nb3 - 1))
            ppool = sb.tile([128, G], F32, name="ppool")
            nc.vector.tensor_copy(out=ppool[:], in_=pp[:])
            nc.sync.dma_start(out=pool_in[:], in_=ppool[:])
            nc.gpsimd.collective_compute(
                "AllReduce", mybir.AluOpType.add,
                replica_groups=[list(range(NC))],
                ins=[pool_in.ap()], outs=[pool_out.ap()])
            pooled_f = sb.tile([128, G], F32, name="pooled_f")
            nc.sync.dma_start(out=pooled_f[:], in_=pool_out[:])
            nc.vector.tensor_scalar(out=pooled_f[:], in0=pooled_f[:],
                                    scalar1=b_t[2][:, 0:1], scalar2=None,
                                    op0=mybir.AluOpType.add)
            pooled = sb.tile([128, G], F16, name="pooled")
            nc.vector.tensor_copy(out=pooled[:], in_=pooled_f[:])

            # ---------- MLP head (replicated)
            _hctr = [0]

            def lrelu_evict(psrc, bias_tile, bias_c, dst16, nrows=128):
                _hctr[0] += 1
                t1 = sb.tile([128, G], F32, name=f"t1_{_hctr[0]}", tag="h1")
                nc.vector.tensor_scalar(out=t1[:nrows], in0=psrc[:nrows],
                                        scalar1=bias_tile[:nrows,
                                                          bias_c:bias_c + 1],
                                        scalar2=None,
                                        op0=mybir.AluOpType.add)
                t2 = sb.tile([128, G], F32, name=f"t2_{_hctr[0]}", tag="h2")
                nc.vector.tensor_scalar(out=t2[:nrows], in0=t1[:nrows],
                                        scalar1=NEG, scalar2=None,
                                        op0=mybir.AluOpType.mult)
                nc.vector.tensor_tensor(out=dst16[:nrows], in0=t1[:nrows],
                                        in1=t2[:nrows],
                                        op=mybir.AluOpType.max)

            wc1_f = sb.tile([128, 512], F32, name="wc1_f")
            nc.sync.dma_start(out=wc1_f[:], in_=wc1[:])
            wc1_h = sb.tile([128, 512], F16, name="wc1_h")
            nc.vector.tensor_copy(out=wc1_h[:], in_=wc1_f[:])
            bc1_t = sb.tile([128, 4], F32, name="bc1_t")
            nc.sync.dma_start(out=bc1_t[:], in_=bc1[:])
            wc2_f = sb.tile([128, 4 * 256], F32, name="wc2_f")
            nc.sync.dma_start(out=wc2_f[:], in_=wc2[:])
            wc2_h = sb.tile([128, 4 * 256], F16, name="wc2_h")
            nc.vector.tensor_copy(out=wc2_h[:], in_=wc2_f[:])
            bc2_t = sb.tile([128, 2], F32, name="bc2_t")
            nc.sync.dma_start(out=bc2_t[:], in_=bc2[:])
            wc3_f = sb.tile([128, 2 * OUT], F32, name="wc3_f")
            nc.sync.dma_start(out=wc3_f[:], in_=wc3[:])
            wc3_h = sb.tile([128, 2 * OUT], F16, name="wc3_h")
            nc.vector.tensor_copy(out=wc3_h[:], in_=wc3_f[:])
            bc3_t = sb.tile([128, 1], F32, name="bc3_t")
            nc.sync.dma_start(out=bc3_t[:], in_=bc3[:])

            z1 = sb.tile([128, 4 * G], F16, name="z1")
            for c in range(4):
                ph = ps2.tile([128, G], F32, space="PSUM",
                              name=f"ph1_{c}", tag="b")
                nc.tensor.matmul(out=ph[:], lhsT=wc1_h[:, 128 * c:128 * (c + 1)],
                                 rhs=pooled[:], start=True, stop=True)
                lrelu_evict(ph, bc1_t, c, z1[:, G * c:G * (c + 1)])
            z2 = sb.tile([128, 2 * G], F16, name="z2")
            for jj in range(2):
                ph = ps2.tile([128, G], F32, space="PSUM",
                              name=f"ph2_{jj}", tag="b")
                for c in range(4):
                    nc.tensor.matmul(
                        out=ph[:],
                        lhsT=wc2_h[:, 256 * c + 128 * jj:256 * c + 128 * (jj + 1)],
                        rhs=z1[:, G * c:G * (c + 1)],
                        start=(c == 0), stop=(c == 3))
                lrelu_evict(ph, bc2_t, jj, z2[:, G * jj:G * (jj + 1)])
            ph3 = ps2.tile([128, G], F32, space="PSUM", name="ph3", tag="b")
            for c in range(2):
                nc.tensor.matmul(out=ph3[:OUT],
                                 lhsT=wc3_h[:, OUT * c:OUT * (c + 1)],
                                 rhs=z2[:, G * c:G * (c + 1)],
                                 start=(c == 0), stop=(c == 1))
            zout = sb.tile([128, G], F32, name="zout")
            lrelu_evict(ph3, bc3_t, 0, zout, nrows=OUT)
            nc.sync.dma_start(out=out_d[:], in_=zout[:OUT])

    split_multiwaits(nc)
    return nc


# ---------------------------------------------------------------- driver

def run_gcn(x, src, dst, gid, W1, b1, W2, b2, W3, b3,
            Wc1, bc1, Wc2, bc2, Wc3, bc3, n_nodes, shard,
            trace=False, tmpdir=None):
    x = np.asarray(x, np.float32)
    src = np.asarray(src).astype(np.int64)
    dst = np.asarray(dst).astype(np.int64)
    gid = np.asarray(gid).astype(np.int64)

    per_core, node_core, node_slot, bw, offs, nbw, nb3 = _prep(
        x, src, dst, gid, n_nodes, shard)
    nc = build_program(shard, nbw, nb3, bw, offs)

    iota = np.tile(np.arange(WIN, dtype=np.float32), (128, 1))
    ident = np.eye(128, dtype=np.float32)

    def colvec(v, n=128):
        a = np.zeros((n, 1), np.float32)
        a[:len(v), 0] = v
        return a

    bc1_a = np.asarray(bc1, np.float32).reshape(4, 128).T.copy()
    bc2_a = np.asarray(bc2, np.float32).reshape(2, 128).T.copy()
    wc2_a = np.concatenate(
        [np.asarray(Wc2, np.float32)[128 * c:128 * (c + 1), :] for c in range(4)],
        axis=1)
    wc3_a = np.concatenate(
        [np.asarray(Wc3, np.float32)[128 * c:128 * (c + 1), :] for c in range(2)],
        axis=1)

    in_maps = []
    for k in range(NC):
        pc = per_core[k]
        xk = np.zeros((shard, D), np.float32)
        mk = node_core == k
        xk[node_slot[mk]] = x[mk]
        in_maps.append({
            "xT": np.ascontiguousarray(xk.T),
            "w1": np.asarray(W1, np.float32), "b1": colvec(np.asarray(b1)),
            "w2": np.asarray(W2, np.float32), "b2": colvec(np.asarray(b2)),
            "w3": np.asarray(W3, np.float32), "b3": colvec(np.asarray(b3)),
            "wc1": np.asarray(Wc1, np.float32), "bc1": bc1_a,
            "wc2": wc2_a, "bc2": bc2_a,
            "wc3": wc3_a, "bc3": colvec(np.asarray(bc3)),
            "gidx": pc["gidx"], "dstloc": pc["dstloc"], "wcol": pc["wcol"],
            "gidx3": pc["gidx3"], "oh3": pc["oh3"],
            "iota": iota, "ident": ident,
        })

    res = run_bass_kernel_spmd(nc, in_maps, core_ids=list(range(NC)),
                               trace=trace, tmpdir=tmpdir)
    out = res.results[0]["out"].T.astype(np.float32).copy()  # [G, OUT]
    return out, res


def kernel(**inputs):
    out, _ = run_gcn(
        inputs["inputs"], inputs["src"], inputs["dst"], inputs["graph_id"],
        inputs["W1"], inputs["b1"], inputs["W2"], inputs["b2"],
        inputs["W3"], inputs["b3"],
        inputs["Wc1"], inputs["bc1"], inputs["Wc2"], inputs["bc2"],
        inputs["Wc3"], inputs["bc3"],
        n_nodes=50000, shard=6272)
    return out


# revision 8
# speedup vs baseline: 1.0868x; 1.0353x over previous
"""GCN (3x GraphConv + mean-pool + MLP head) on 8 Trainium2 NeuronCores.

Strategy (SPMD, one program on all 8 cores):
  - Nodes are assigned to (core, 256-dst-window) buckets balanced by
    in-degree, so every core/window sees ~equal edge counts and the
    SPMD-uniform batch schedule wastes almost no padding.
  - Weight matrices replicated; degree norms folded into per-edge weights
    w_e = src_norm[src] * dst_norm[dst] carried by the scatter one-hots.
  - Per layer: project own shard (fp16 matmuls, PSUM f32) -> PE-transpose to
    node-major -> AllGather (2 pipelined chunks) into a DRAM table
    [50176, 128] fp16 -> gather each edge's source row via indirect DMA
    (128 rows/instruction, edges grouped per dst window) -> scatter-add via
    matmul with an on-chip weighted one-hot (iota==dst_local)*w_e
    accumulating in PSUM per window.
  - Layer 3's scatter is fused with dgl.mean_nodes: its one-hot is the
    8-wide graph-membership matrix scaled by w_e/cnt_g, so the whole last
    aggregation lands directly in a [128,8] pooled accumulator; partials
    are AllReduced and the tiny MLP head runs replicated on every core.
"""

import heapq
import sys

sys.path.insert(0, "/opt/trn_rl_repo")

import numpy as np

import concourse.bass as bass
import concourse.mybir as mybir
import concourse.tile as tile
import bass_rust
from concourse.bass_utils import run_bass_kernel_spmd

F32 = mybir.dt.float32
F16 = mybir.dt.float16
I32 = mybir.dt.int32

NC = 8          # cores
D = 128         # feature dim (== partition width)
G = 8           # graphs
OUT = 10
NEG = 0.01      # LeakyReLU slope
WIN = 256       # dst window per PSUM tile

_split_ctr = [0]


def split_multiwaits(nc):
    """This walrus encodes at most ONE sync-wait per instruction; hoist
    extra waits into preceding EventSemaphore ops on the same engine."""
    for f in nc.m.functions:
        for blk in f.blocks:
            insts = list(blk.instructions)
            new, changed = [], False
            for inst in insts:
                si = inst.sync_info
                if si is not None and len(si.on_wait) > 1:
                    waits = list(si.on_wait)
                    for w in waits[:-1]:
                        _split_ctr[0] += 1
                        es = mybir.InstEventSemaphore(
                            name=f"mwsplit_{_split_ctr[0]}", ins=[], outs=[])
                        es.engine = inst.engine
                        es.sync_info = bass_rust.SyncInfo(on_wait=[w], on_update=[])
                        new.append(es)
                    si.on_wait = waits[-1:]
                    changed = True
                new.append(inst)
            if changed:
                blk.instructions = new


def _chunks(shard):
    """Up to 4 roughly-equal 128-aligned table chunks covering [0, shard)."""
    ntile = shard // 128
    n = min(4, ntile)
    base = ntile // n
    sizes = [base] * n
    for i in range(ntile - base * n):
        sizes[n - 1 - i] += 1
    out, c = [], 0
    for z in sizes:
        out.append((c, z * 128))
        c += z * 128
    return out


def _windows(shard):
    """List of (start, width) dst windows covering [0, shard)."""
    out = []
    c = 0
    while c < shard:
        w = min(WIN, shard - c)
        out.append((c, w))
        c += w
    return out


# ---------------------------------------------------------------- host prep

def _prep(x, src, dst, gid, n_nodes, shard):
    wins = _windows(shard)
    nwin = len(wins)
    chunks = _chunks(shard)

    out_deg = np.bincount(src, minlength=n_nodes)
    in_deg = np.bincount(dst, minlength=n_nodes)
    snorm = np.clip(out_deg, 1, None).astype(np.float32) ** -0.5
    dnorm = np.clip(in_deg, 1, None).astype(np.float32) ** -0.5
    we = snorm[src] * dnorm[dst]
    cnt = np.bincount(gid, minlength=G).astype(np.float32)
    cinv = (1.0 / np.clip(cnt, 1, None)).astype(np.float32)

    # --- balanced node -> (core, window) assignment by in-degree
    order = np.argsort(-in_deg, kind="stable")
    heap = [(0.0, k * nwin + w) for k in range(NC) for w in range(nwin)]
    heapq.heapify(heap)
    fill = [[0] * nwin for _ in range(NC)]
    node_core = np.empty(n_nodes, np.int32)
    node_slot = np.empty(n_nodes, np.int32)
    for n in order:
        while True:
            load, b = heapq.heappop(heap)
            k, w = divmod(b, nwin)
            if fill[k][w] < wins[w][1]:
                break
        node_core[n] = k
        node_slot[n] = wins[w][0] + fill[k][w]
        fill[k][w] += 1
        heapq.heappush(heap, (load + float(in_deg[n]), b))

    # global gather-table id (matches the chunked AllGather layout):
    # chunk c of every rank is contiguous: row = NC*S_c + k*Z_c + (s - S_c)
    s = node_slot.astype(np.int64)
    kk = node_core.astype(np.int64)
    gid_tab = np.zeros(len(s), np.int64)
    for (cst, csz) in chunks:
        m = (s >= cst) & (s < cst + csz)
        gid_tab[m] = NC * cst + kk[m] * csz + (s[m] - cst)
    gid_tab = gid_tab.astype(np.int32)

    e_core = node_core[dst]
    e_slot_all = node_slot[dst]

    wcnt = np.zeros((NC, nwin), np.int64)
    for k in range(NC):
        m = e_core == k
        wcnt[k] = np.bincount(e_slot_all[m] // WIN, minlength=nwin)
    bw = np.maximum(1, -(-wcnt.max(axis=0) // 128)).astype(np.int64)
    offs = np.concatenate([[0], np.cumsum(bw)])
    nbw = int(offs[-1])
    nb3 = max(1, max(-(-int((e_core == k).sum()) // 128) for k in range(NC)))

    per_core = []
    for k in range(NC):
        m = e_core == k
        e_src = src[m].astype(np.int64)
        e_slot = e_slot_all[m].astype(np.int64)
        e_w = we[m].astype(np.float32)
        e_g = gid[dst[m]].astype(np.int64)
        o = np.argsort(e_slot, kind="stable")
        e_src, e_slot, e_w, e_g = e_src[o], e_slot[o], e_w[o], e_g[o]

        gidx = np.zeros((128, nbw), np.int32)
        dstloc = np.full((128, nbw), -1.0, np.float32)
        wcol = np.zeros((128, nbw), np.float32)
        win = e_slot // WIN
        pos = np.zeros(len(e_src), np.int64)
        for w in range(nwin):
            mm = win == w
            pos[mm] = np.arange(mm.sum())
        col = offs[win] + pos // 128
        row = pos % 128
        gidx[row, col] = gid_tab[e_src]
        dstloc[row, col] = (e_slot - win * WIN).astype(np.float32)
        wcol[row, col] = e_w

        gidx3 = np.zeros((128, nb3), np.int32)
        oh3 = np.zeros((128, nb3 * G), np.float32)
        p = np.arange(len(e_src))
        b3, r3 = p // 128, p % 128
        gidx3[r3, b3] = gid_tab[e_src]
        oh3[r3, b3 * G + e_g] = e_w * cinv[e_g]

        per_core.append(dict(gidx=gidx, dstloc=dstloc, wcol=wcol,
                             gidx3=gidx3, oh3=oh3))
    return per_core, node_core, node_slot, bw, offs, nbw, nb3


# ---------------------------------------------------------------- program

def build_program(shard, nbw, nb3, bw, offs):
    npad = NC * shard
    wins = _windows(shard)
    nwin = len(wins)
    chunks = _chunks(shard)
    # window index after which each chunk's columns are fully evicted
    trig = []
    for (cst, csz) in chunks:
        w = 0
        while wins[w][0] + wins[w][1] < cst + csz:
            w += 1
        trig.append(w)

    nc = bass.Bass("TRN2", target_bir_lowering=False, debug=False,
                   num_devices=NC)

    xT = nc.dram_tensor("xT", [128, shard], F32, kind="ExternalInput")
    wg = [nc.dram_tensor(f"w{l}", [128, 128], F32, kind="ExternalInput")
          for l in (1, 2, 3)]
    bg = [nc.dram_tensor(f"b{l}", [128, 1], F32, kind="ExternalInput")
          for l in (1, 2, 3)]
    wc1 = nc.dram_tensor("wc1", [128, 512], F32, kind="ExternalInput")
    bc1 = nc.dram_tensor("bc1", [128, 4], F32, kind="ExternalInput")
    wc2 = nc.dram_tensor("wc2", [128, 4 * 256], F32, kind="ExternalInput")
    bc2 = nc.dram_tensor("bc2", [128, 2], F32, kind="ExternalInput")
    wc3 = nc.dram_tensor("wc3", [128, 2 * OUT], F32, kind="ExternalInput")
    bc3 = nc.dram_tensor("bc3", [128, 1], F32, kind="ExternalInput")
    gidx_in = nc.dram_tensor("gidx", [128, nbw], I32, kind="ExternalInput")
    dstloc_in = nc.dram_tensor("dstloc", [128, nbw], F32, kind="ExternalInput")
    wcol_in = nc.dram_tensor("wcol", [128, nbw], F32, kind="ExternalInput")
    gidx3_in = nc.dram_tensor("gidx3", [128, nb3], I32, kind="ExternalInput")
    oh3_in = nc.dram_tensor("oh3", [128, nb3 * G], F32, kind="ExternalInput")
    iota_in = nc.dram_tensor("iota", [128, WIN], F32, kind="ExternalInput")
    ident_in = nc.dram_tensor("ident", [128, 128], F32, kind="ExternalInput")
    out_d = nc.dram_tensor("out", [OUT, G], F32, kind="ExternalOutput")

    bounce = [nc.dram_tensor(f"bnc{l}", [shard, 128], F16) for l in range(3)]
    table = [nc.dram_tensor(f"tbl{l}", [npad, 128], F16) for l in range(3)]
    pool_in = nc.dram_tensor("pool_in", [128, G], F32)
    pool_out = nc.dram_tensor("pool_out", [128, G], F32)

    with tile.TileContext(nc) as tc:
        with tc.tile_pool(name="sb", bufs=1) as sb, \
             tc.tile_pool(name="ps", bufs=4, space="PSUM") as ps, \
             tc.tile_pool(name="ps2", bufs=2, space="PSUM") as ps2:

            iota_t = sb.tile([128, WIN], F32, name="iota_t")
            nc.sync.dma_start(out=iota_t[:], in_=iota_in[:])
            ident_t = sb.tile([128, 128], F32, name="ident_t")
            nc.sync.dma_start(out=ident_t[:], in_=ident_in[:])
            gidx_t = sb.tile([128, nbw], I32, name="gidx_t")
            nc.sync.dma_start(out=gidx_t[:], in_=gidx_in[:])
            dstloc_t = sb.tile([128, nbw], F32, name="dstloc_t")
            nc.sync.dma_start(out=dstloc_t[:], in_=dstloc_in[:])
            wcol_t = sb.tile([128, nbw], F32, name="wcol_t")
            nc.sync.dma_start(out=wcol_t[:], in_=wcol_in[:])
            gidx3_t = sb.tile([128, nb3], I32, name="gidx3_t")
            nc.sync.dma_start(out=gidx3_t[:], in_=gidx3_in[:])
            oh3_f32 = sb.tile([128, nb3 * G], F32, name="oh3_f32")
            nc.sync.dma_start(out=oh3_f32[:], in_=oh3_in[:])
            oh3_t = sb.tile([128, nb3 * G], F16, name="oh3_t")
            nc.vector.tensor_copy(out=oh3_t[:], in_=oh3_f32[:])

            w_t, b_t = [], []
            for l in range(3):
                wf = sb.tile([128, 128], F32, name=f"wf{l}")
                nc.sync.dma_start(out=wf[:], in_=wg[l][:])
                wh = sb.tile([128, 128], F16, name=f"wh{l}")
                nc.vector.tensor_copy(out=wh[:], in_=wf[:])
                w_t.append(wh)
                bt = sb.tile([128, 1], F32, name=f"bt{l}")
                nc.sync.dma_start(out=bt[:], in_=bg[l][:])
                b_t.append(bt)

            xT_t = sb.tile([128, shard], F32, name="xT_t")
            nc.sync.dma_start(out=xT_t[:], in_=xT[:])
            state = sb.tile([128, shard], F16, name="state0")
            nc.vector.tensor_copy(out=state[:], in_=xT_t[:])

            def emit_chunk(l, state_t, cst, csz):
                """Project chunk [cst, cst+csz) of layer l's state, transpose
                to node-major, bounce to DRAM, AllGather into table[l]."""
                xp = sb.tile([128, csz], F32, name=f"xp{l}_{cst}", tag="xpf")
                c = 0
                while c < csz:
                    w = min(512, csz - c)
                    pt = ps2.tile([128, w], F32, space="PSUM",
                                  name=f"pj{l}_{cst}_{c}", tag="c")
                    nc.tensor.matmul(out=pt[:], lhsT=w_t[l][:],
                                     rhs=state_t[:, cst + c:cst + c + w],
                                     start=True, stop=True)
                    nc.vector.tensor_copy(out=xp[:, c:c + w], in_=pt[:])
                    c += w
                nck = csz // 128
                xpnm = sb.tile([128, nck, 128], F16,
                               name=f"xpnm{l}_{cst}", tag="xpnm")
                for t in range(nck):
                    tp = ps2.tile([128, 128], F32, space="PSUM",
                                  name=f"tp{l}_{cst}_{t}", tag="c")
                    nc.tensor.transpose(out=tp[:],
                                        in_=xp[:, 128 * t:128 * (t + 1)],
                                        identity=ident_t[:])
                    nc.vector.tensor_copy(out=xpnm[:, t, :], in_=tp[:])
                nc.sync.dma_start(
                    out=bounce[l][cst:cst + csz, :]
                        .rearrange("(c p) f -> p c f", p=128),
                    in_=xpnm[:])
                tb0 = NC * cst
                nc.gpsimd.collective_compute(
                    "AllGather", mybir.AluOpType.bypass,
                    replica_groups=[list(range(NC))],
                    ins=[bounce[l][cst:cst + csz, :]],
                    outs=[table[l][tb0:tb0 + NC * csz, :]])

            # gathers land in per-window grouped tiles so the WAR wait is
            # per-window (4-slot rotation), not per-gather
            def gather_group(l, cols, src_idx_t, gname, gtag):
                mt = sb.tile([128, len(cols) * 128], F16, name=gname,
                             tag=gtag)
                views = []
                for i, col in enumerate(cols):
                    v = mt[:, 128 * i:128 * (i + 1)]
                    nc.gpsimd.indirect_dma_start(
                        out=v, out_offset=None, in_=table[l][:],
                        in_offset=bass.IndirectOffsetOnAxis(
                            ap=src_idx_t[:, col:col + 1], axis=0))
                    views.append(v)
                return views

            # layer 0's table comes straight from the input state
            for (cst, csz) in chunks:
                emit_chunk(0, state, cst, csz)

            for l in range(2):
                nstate = sb.tile([128, shard], F16, name=f"state{l + 1}")
                for w in range(nwin):
                    wst, ww = wins[w]
                    pw = ps.tile([128, ww], F32, space="PSUM",
                                 name=f"pw{l}_{w}", tag="a")
                    nb = int(bw[w])
                    cols = [int(offs[w]) + j for j in range(nb)]
                    mviews = gather_group(l, cols, gidx_t,
                                          f"mg{l}_{w}", f"gw{w % 4}")
                    for j, col in enumerate(cols):
                        oh = sb.tile([128, ww], F16, name=f"oh{l}_{col}",
                                     tag=f"oh{col % 16}")
                        nc.vector.tensor_scalar(
                            out=oh[:], in0=iota_t[:, :ww],
                            scalar1=dstloc_t[:, col:col + 1],
                            scalar2=wcol_t[:, col:col + 1],
                            op0=mybir.AluOpType.is_equal,
                            op1=mybir.AluOpType.mult)
                        nc.tensor.matmul(out=pw[:], lhsT=mviews[j], rhs=oh[:],
                                         start=(j == 0), stop=(j == nb - 1))
                    nc.vector.tensor_scalar(
                        out=nstate[:, wst:wst + ww], in0=pw[:],
                        scalar1=b_t[l][:, 0:1], scalar2=None,
                        op0=mybir.AluOpType.add)
                    # emit next layer's table chunks as soon as their
                    # columns are final -- overlaps collectives with gathers
                    for ci, (cst, csz) in enumerate(chunks):
                        if trig[ci] == w:
                            emit_chunk(l + 1, nstate, cst, csz)
                state = nstate

            # ---------- layer 3 fused with mean-pool (table emitted above)
            pp = ps2.tile([128, G], F32, space="PSUM", name="pp", tag="b")
            GRP3 = 32
            b3 = 0
            while b3 < nb3:
                cols = list(range(b3, min(b3 + GRP3, nb3)))
                mviews = gather_group(2, cols, gidx3_t,
                                      f"mg3_{b3}", f"g3{(b3 // GRP3) % 3}")
                for i, col in enumerate(cols):
                    nc.tensor.matmul(out=pp[:], lhsT=mviews[i],
                                     rhs=oh3_t[:, G * col:G * (col + 1)],
                                     start=(col == 0), stop=(col == nb3 - 1))
                b3 += GRP3
            ppool = sb.tile([128, G], F32, name="ppool")
            nc.vector.tensor_copy(out=ppool[:], in_=pp[:])
            nc.sync.dma_start(out=pool_in[:], in_=ppool[:])
            nc.gpsimd.collective_compute(
                "AllReduce", mybir.AluOpType.add,
                replica_groups=[list(range(NC))],
                ins=[pool_in.ap()], outs=[pool_out.ap()])
            pooled_f = sb.tile([128, G], F32, name="pooled_f")
            nc.sync.dma_start(out=pooled_f[:], in_=pool_out[:])
            nc.vector.tensor_scalar(out=pooled_f[:], in0=pooled_f[:],
                                    scalar1=b_t[2][:, 0:1], scalar2=None,
                                    op0=mybir.AluOpType.add)
            pooled = sb.tile([128, G], F16, name="pooled")
            nc.vector.tensor_copy(out=pooled[:], in_=pooled_f[:])

            # ---------- MLP head (replicated)
            _hctr = [0]

            def lrelu_evict(psrc, bias_tile, bias_c, dst16, nrows=128):
                _hctr[0] += 1
                t1 = sb.tile([128, G], F32, name=f"t1_{_hctr[0]}", tag="h1")
                nc.vector.tensor_scalar(out=t1[:nrows], in0=psrc[:nrows],
                                        scalar1=bias_tile[:nrows,
                                                          bias_c:bias_c + 1],
                                        scalar2=None,
                                        op0=mybir.AluOpType.add)
                t2 = sb.tile([128, G], F32, name=f"t2_{_hctr[0]}", tag="h2")
                nc.vector.tensor_scalar(out=t2[:nrows], in0=t1[:nrows],
                                        scalar1=NEG, scalar2=None,
                                        op0=mybir.AluOpType.mult)
                nc.vector.tensor_tensor(out=dst16[:nrows], in0=t1[:nrows],
                                        in1=t2[:nrows],
                                        op=mybir.AluOpType.max)

            wc1_f = sb.tile([128, 512], F32, name="wc1_f")
            nc.sync.dma_start(out=wc1_f[:], in_=wc1[:])
            wc1_h = sb.tile([128, 512], F16, name="wc1_h")
            nc.vector.tensor_copy(out=wc1_h[:], in_=wc1_f[:])
            bc1_t = sb.tile([128, 4], F32, name="bc1_t")
            nc.sync.dma_start(out=bc1_t[:], in_=bc1[:])
            wc2_f = sb.tile([128, 4 * 256], F32, name="wc2_f")
            nc.sync.dma_start(out=wc2_f[:], in_=wc2[:])
            wc2_h = sb.tile([128, 4 * 256], F16, name="wc2_h")
            nc.vector.tensor_copy(out=wc2_h[:], in_=wc2_f[:])
            bc2_t = sb.tile([128, 2], F32, name="bc2_t")
            nc.sync.dma_start(out=bc2_t[:], in_=bc2[:])
            wc3_f = sb.tile([128, 2 * OUT], F32, name="wc3_f")
            nc.sync.dma_start(out=wc3_f[:], in_=wc3[:])
            wc3_h = sb.tile([128, 2 * OUT], F16, name="wc3_h")
            nc.vector.tensor_copy(out=wc3_h[:], in_=wc3_f[:])
            bc3_t = sb.tile([128, 1], F32, name="bc3_t")
            nc.sync.dma_start(out=bc3_t[:], in_=bc3[:])

            z1 = sb.tile([128, 4 * G], F16, name="z1")
            for c in range(4):
                ph = ps2.tile([128, G], F32, space="PSUM",
                              name=f"ph1_{c}", tag="b")
                nc.tensor.matmul(out=ph[:], lhsT=wc1_h[:, 128 * c:128 * (c + 1)],
                                 rhs=pooled[:], start=True, stop=True)
                lrelu_evict(ph, bc1_t, c, z1[:, G * c:G * (c + 1)])
            z2 = sb.tile([128, 2 * G], F16, name="z2")
            for jj in range(2):
                ph = ps2.tile([128, G], F32, space="PSUM",
                              name=f"ph2_{jj}", tag="b")
                for c in range(4):
                    nc.tensor.matmul(
                        out=ph[:],
                        lhsT=wc2_h[:, 256 * c + 128 * jj:256 * c + 128 * (jj + 1)],
                        rhs=z1[:, G * c:G * (c + 1)],
                        start=(c == 0), stop=(c == 3))
                lrelu_evict(ph, bc2_t, jj, z2[:, G * jj:G * (jj + 1)])
            ph3 = ps2.tile([128, G], F32, space="PSUM", name="ph3", tag="b")
            for c in range(2):
                nc.tensor.matmul(out=ph3[:OUT],
                                 lhsT=wc3_h[:, OUT * c:OUT * (c + 1)],
                                 rhs=z2[:, G * c:G * (c + 1)],
                                 start=(c == 0), stop=(c == 1))
            zout = sb.tile([128, G], F32, name="zout")
            lrelu_evict(ph3, bc3_t, 0, zout, nrows=OUT)
            nc.sync.dma_start(out=out_d[:], in_=zout[:OUT])

    split_multiwaits(nc)
    return nc


# ---------------------------------------------------------------- driver

def run_gcn(x, src, dst, gid, W1, b1, W2, b2, W3, b3,
            Wc1, bc1, Wc2, bc2, Wc3, bc3, n_nodes, shard,
            trace=False, tmpdir=None):
    x = np.asarray(x, np.float32)
    src = np.asarray(src).astype(np.int64)
    dst = np.asarray(dst).astype(np.int64)
    gid = np.asarray(gid).astype(np.int64)

    per_core, node_core, node_slot, bw, offs, nbw, nb3 = _prep(
        x, src, dst, gid, n_nodes, shard)
    nc = build_program(shard, nbw, nb3, bw, offs)

    iota = np.tile(np.arange(WIN, dtype=np.float32), (128, 1))
    ident = np.eye(128, dtype=np.float32)

    def colvec(v, n=128):
        a = np.zeros((n, 1), np.float32)
        a[:len(v), 0] = v
        return a

    bc1_a = np.asarray(bc1, np.float32).reshape(4, 128).T.copy()
    bc2_a = np.asarray(bc2, np.float32).reshape(2, 128).T.copy()
    wc2_a = np.concatenate(
        [np.asarray(Wc2, np.float32)[128 * c:128 * (c + 1), :] for c in range(4)],
        axis=1)
    wc3_a = np.concatenate(
        [np.asarray(Wc3, np.float32)[128 * c:128 * (c + 1), :] for c in range(2)],
        axis=1)

    in_maps = []
    for k in range(NC):
        pc = per_core[k]
        xk = np.zeros((shard, D), np.float32)
        mk = node_core == k
        xk[node_slot[mk]] = x[mk]
        in_maps.append({
            "xT": np.ascontiguousarray(xk.T),
            "w1": np.asarray(W1, np.float32), "b1": colvec(np.asarray(b1)),
            "w2": np.asarray(W2, np.float32), "b2": colvec(np.asarray(b2)),
            "w3": np.asarray(W3, np.float32), "b3": colvec(np.asarray(b3)),
            "wc1": np.asarray(Wc1, np.float32), "bc1": bc1_a,
            "wc2": wc2_a, "bc2": bc2_a,
            "wc3": wc3_a, "bc3": colvec(np.asarray(bc3)),
            "gidx": pc["gidx"], "dstloc": pc["dstloc"], "wcol": pc["wcol"],
            "gidx3": pc["gidx3"], "oh3": pc["oh3"],
            "iota": iota, "ident": ident,
        })

    res = run_bass_kernel_spmd(nc, in_maps, core_ids=list(range(NC)),
                               trace=trace, tmpdir=tmpdir)
    out = res.results[0]["out"].T.astype(np.float32).copy()  # [G, OUT]
    return out, res


def kernel(**inputs):
    out, _ = run_gcn(
        inputs["inputs"], inputs["src"], inputs["dst"], inputs["graph_id"],
        inputs["W1"], inputs["b1"], inputs["W2"], inputs["b2"],
        inputs["W3"], inputs["b3"],
        inputs["Wc1"], inputs["bc1"], inputs["Wc2"], inputs["bc2"],
        inputs["Wc3"], inputs["bc3"],
        n_nodes=50000, shard=6272)
    return out


# revision 9
# speedup vs baseline: 1.0916x; 1.0044x over previous
"""GCN (3x GraphConv + mean-pool + MLP head) on 8 Trainium2 NeuronCores.

Strategy (SPMD, one program on all 8 cores):
  - Nodes are assigned to (core, 256-dst-window) buckets balanced by
    in-degree, so every core/window sees ~equal edge counts and the
    SPMD-uniform batch schedule wastes almost no padding.
  - Weight matrices replicated; degree norms folded into per-edge weights
    w_e = src_norm[src] * dst_norm[dst] carried by the scatter one-hots.
  - Per layer: project own shard (fp16 matmuls, PSUM f32) -> PE-transpose to
    node-major -> AllGather (2 pipelined chunks) into a DRAM table
    [50176, 128] fp16 -> gather each edge's source row via indirect DMA
    (128 rows/instruction, edges grouped per dst window) -> scatter-add via
    matmul with an on-chip weighted one-hot (iota==dst_local)*w_e
    accumulating in PSUM per window.
  - Layer 3's scatter is fused with dgl.mean_nodes: its one-hot is the
    8-wide graph-membership matrix scaled by w_e/cnt_g, so the whole last
    aggregation lands directly in a [128,8] pooled accumulator; partials
    are AllReduced and the tiny MLP head runs replicated on every core.
"""

import heapq
import sys

sys.path.insert(0, "/opt/trn_rl_repo")

import numpy as np

import concourse.bass as bass
import concourse.mybir as mybir
import concourse.tile as tile
import bass_rust
from concourse.bass_utils import run_bass_kernel_spmd

F32 = mybir.dt.float32
F16 = mybir.dt.float16
I32 = mybir.dt.int32

NC = 8          # cores
D = 128         # feature dim (== partition width)
G = 8           # graphs
OUT = 10
NEG = 0.01      # LeakyReLU slope
WIN = 256       # dst window per PSUM tile

_split_ctr = [0]


def split_multiwaits(nc):
    """This walrus encodes at most ONE sync-wait per instruction; hoist
    extra waits into preceding EventSemaphore ops on the same engine."""
    for f in nc.m.functions:
        for blk in f.blocks:
            insts = list(blk.instructions)
            new, changed = [], False
            for inst in insts:
                si = inst.sync_info
                if si is not None and len(si.on_wait) > 1:
                    waits = list(si.on_wait)
                    for w in waits[:-1]:
                        _split_ctr[0] += 1
                        es = mybir.InstEventSemaphore(
                            name=f"mwsplit_{_split_ctr[0]}", ins=[], outs=[])
                        es.engine = inst.engine
                        es.sync_info = bass_rust.SyncInfo(on_wait=[w], on_update=[])
                        new.append(es)
                    si.on_wait = waits[-1:]
                    changed = True
                new.append(inst)
            if changed:
                blk.instructions = new


def _chunks(shard):
    """Up to 4 roughly-equal 128-aligned table chunks covering [0, shard)."""
    ntile = shard // 128
    n = min(4, ntile)
    base = ntile // n
    sizes = [base] * n
    for i in range(ntile - base * n):
        sizes[n - 1 - i] += 1
    out, c = [], 0
    for z in sizes:
        out.append((c, z * 128))
        c += z * 128
    return out


def _windows(shard):
    """List of (start, width) dst windows covering [0, shard)."""
    out = []
    c = 0
    while c < shard:
        w = min(WIN, shard - c)
        out.append((c, w))
        c += w
    return out


# ---------------------------------------------------------------- host prep

def _prep(x, src, dst, gid, n_nodes, shard):
    wins = _windows(shard)
    nwin = len(wins)
    chunks = _chunks(shard)

    out_deg = np.bincount(src, minlength=n_nodes)
    in_deg = np.bincount(dst, minlength=n_nodes)
    snorm = np.clip(out_deg, 1, None).astype(np.float32) ** -0.5
    dnorm = np.clip(in_deg, 1, None).astype(np.float32) ** -0.5
    we = snorm[src] * dnorm[dst]
    cnt = np.bincount(gid, minlength=G).astype(np.float32)
    cinv = (1.0 / np.clip(cnt, 1, None)).astype(np.float32)

    # --- balanced node -> (core, window) assignment by in-degree
    order = np.argsort(-in_deg, kind="stable")
    heap = [(0.0, k * nwin + w) for k in range(NC) for w in range(nwin)]
    heapq.heapify(heap)
    fill = [[0] * nwin for _ in range(NC)]
    node_core = np.empty(n_nodes, np.int32)
    node_slot = np.empty(n_nodes, np.int32)
    for n in order:
        while True:
            load, b = heapq.heappop(heap)
            k, w = divmod(b, nwin)
            if fill[k][w] < wins[w][1]:
                break
        node_core[n] = k
        node_slot[n] = wins[w][0] + fill[k][w]
        fill[k][w] += 1
        heapq.heappush(heap, (load + float(in_deg[n]), b))

    # global gather-table id (matches the chunked AllGather layout):
    # chunk c of every rank is contiguous: row = NC*S_c + k*Z_c + (s - S_c)
    s = node_slot.astype(np.int64)
    kk = node_core.astype(np.int64)
    gid_tab = np.zeros(len(s), np.int64)
    for (cst, csz) in chunks:
        m = (s >= cst) & (s < cst + csz)
        gid_tab[m] = NC * cst + kk[m] * csz + (s[m] - cst)
    gid_tab = gid_tab.astype(np.int32)

    e_core = node_core[dst]
    e_slot_all = node_slot[dst]

    wcnt = np.zeros((NC, nwin), np.int64)
    for k in range(NC):
        m = e_core == k
        wcnt[k] = np.bincount(e_slot_all[m] // WIN, minlength=nwin)
    bw = np.maximum(1, -(-wcnt.max(axis=0) // 128)).astype(np.int64)
    offs = np.concatenate([[0], np.cumsum(bw)])
    nbw = int(offs[-1])
    nb3 = max(1, max(-(-int((e_core == k).sum()) // 128) for k in range(NC)))

    per_core = []
    for k in range(NC):
        m = e_core == k
        e_src = src[m].astype(np.int64)
        e_slot = e_slot_all[m].astype(np.int64)
        e_w = we[m].astype(np.float32)
        e_g = gid[dst[m]].astype(np.int64)
        o = np.argsort(e_slot, kind="stable")
        e_src, e_slot, e_w, e_g = e_src[o], e_slot[o], e_w[o], e_g[o]

        gidx = np.zeros((128, nbw), np.int32)
        dstloc = np.full((128, nbw), -1.0, np.float32)
        wcol = np.zeros((128, nbw), np.float32)
        win = e_slot // WIN
        pos = np.zeros(len(e_src), np.int64)
        for w in range(nwin):
            mm = win == w
            pos[mm] = np.arange(mm.sum())
        col = offs[win] + pos // 128
        row = pos % 128
        gidx[row, col] = gid_tab[e_src]
        dstloc[row, col] = (e_slot - win * WIN).astype(np.float32)
        wcol[row, col] = e_w

        gidx3 = np.zeros((128, nb3), np.int32)
        oh3 = np.zeros((128, nb3 * G), np.float32)
        p = np.arange(len(e_src))
        b3, r3 = p // 128, p % 128
        gidx3[r3, b3] = gid_tab[e_src]
        oh3[r3, b3 * G + e_g] = e_w * cinv[e_g]

        per_core.append(dict(gidx=gidx, dstloc=dstloc, wcol=wcol,
                             gidx3=gidx3, oh3=oh3))
    return per_core, node_core, node_slot, bw, offs, nbw, nb3


# ---------------------------------------------------------------- program

def build_program(shard, nbw, nb3, bw, offs):
    npad = NC * shard
    wins = _windows(shard)
    nwin = len(wins)
    chunks = _chunks(shard)
    # window index after which each chunk's columns are fully evicted
    trig = []
    for (cst, csz) in chunks:
        w = 0
        while wins[w][0] + wins[w][1] < cst + csz:
            w += 1
        trig.append(w)

    nc = bass.Bass("TRN2", target_bir_lowering=False, debug=False,
                   num_devices=NC)

    xT = nc.dram_tensor("xT", [128, shard], F32, kind="ExternalInput")
    wg = [nc.dram_tensor(f"w{l}", [128, 128], F32, kind="ExternalInput")
          for l in (1, 2, 3)]
    bg = [nc.dram_tensor(f"b{l}", [128, 1], F32, kind="ExternalInput")
          for l in (1, 2, 3)]
    wc1 = nc.dram_tensor("wc1", [128, 512], F32, kind="ExternalInput")
    bc1 = nc.dram_tensor("bc1", [128, 4], F32, kind="ExternalInput")
    wc2 = nc.dram_tensor("wc2", [128, 4 * 256], F32, kind="ExternalInput")
    bc2 = nc.dram_tensor("bc2", [128, 2], F32, kind="ExternalInput")
    wc3 = nc.dram_tensor("wc3", [128, 2 * OUT], F32, kind="ExternalInput")
    bc3 = nc.dram_tensor("bc3", [128, 1], F32, kind="ExternalInput")
    gidx_in = nc.dram_tensor("gidx", [128, nbw], I32, kind="ExternalInput")
    dstloc_in = nc.dram_tensor("dstloc", [128, nbw], F32, kind="ExternalInput")
    wcol_in = nc.dram_tensor("wcol", [128, nbw], F32, kind="ExternalInput")
    gidx3_in = nc.dram_tensor("gidx3", [128, nb3], I32, kind="ExternalInput")
    oh3_in = nc.dram_tensor("oh3", [128, nb3 * G], F32, kind="ExternalInput")
    iota_in = nc.dram_tensor("iota", [128, WIN], F32, kind="ExternalInput")
    ident_in = nc.dram_tensor("ident", [128, 128], F32, kind="ExternalInput")
    out_d = nc.dram_tensor("out", [OUT, G], F32, kind="ExternalOutput")

    bounce = [nc.dram_tensor(f"bnc{l}", [shard, 128], F16) for l in range(3)]
    table = [nc.dram_tensor(f"tbl{l}", [npad, 128], F16) for l in range(3)]
    pool_in = nc.dram_tensor("pool_in", [128, G], F32)
    pool_out = nc.dram_tensor("pool_out", [128, G], F32)

    with tile.TileContext(nc) as tc:
        with tc.tile_pool(name="sb", bufs=1) as sb, \
             tc.tile_pool(name="ps", bufs=4, space="PSUM") as ps, \
             tc.tile_pool(name="ps2", bufs=2, space="PSUM") as ps2:

            # critical-path loads first: xT, W1, identity
            xT_t = sb.tile([128, shard], F32, name="xT_t")
            nc.sync.dma_start(out=xT_t[:], in_=xT[:])
            ident_t = sb.tile([128, 128], F32, name="ident_t")
            nc.sync.dma_start(out=ident_t[:], in_=ident_in[:])
            w_t, b_t = [], []
            for l in range(3):
                wf = sb.tile([128, 128], F32, name=f"wf{l}")
                nc.sync.dma_start(out=wf[:], in_=wg[l][:])
                wh = sb.tile([128, 128], F16, name=f"wh{l}")
                nc.vector.tensor_copy(out=wh[:], in_=wf[:])
                w_t.append(wh)
                bt = sb.tile([128, 1], F32, name=f"bt{l}")
                nc.sync.dma_start(out=bt[:], in_=bg[l][:])
                b_t.append(bt)
            state = sb.tile([128, shard], F16, name="state0")
            nc.vector.tensor_copy(out=state[:], in_=xT_t[:])

            iota_t = sb.tile([128, WIN], F32, name="iota_t")
            nc.sync.dma_start(out=iota_t[:], in_=iota_in[:])
            gidx_t = sb.tile([128, nbw], I32, name="gidx_t")
            nc.sync.dma_start(out=gidx_t[:], in_=gidx_in[:])
            dstloc_t = sb.tile([128, nbw], F32, name="dstloc_t")
            nc.sync.dma_start(out=dstloc_t[:], in_=dstloc_in[:])
            wcol_t = sb.tile([128, nbw], F32, name="wcol_t")
            nc.sync.dma_start(out=wcol_t[:], in_=wcol_in[:])
            gidx3_t = sb.tile([128, nb3], I32, name="gidx3_t")
            nc.sync.dma_start(out=gidx3_t[:], in_=gidx3_in[:])
            oh3_f32 = sb.tile([128, nb3 * G], F32, name="oh3_f32")
            nc.sync.dma_start(out=oh3_f32[:], in_=oh3_in[:])
            oh3_t = sb.tile([128, nb3 * G], F16, name="oh3_t")
            nc.vector.tensor_copy(out=oh3_t[:], in_=oh3_f32[:])

            def emit_chunk(l, state_t, cst, csz):
                """Project chunk [cst, cst+csz) of layer l's state, transpose
                to node-major, bounce to DRAM, AllGather into table[l]."""
                xp = sb.tile([128, csz], F32, name=f"xp{l}_{cst}", tag="xpf")
                c = 0
                while c < csz:
                    w = min(512, csz - c)
                    pt = ps2.tile([128, w], F32, space="PSUM",
                                  name=f"pj{l}_{cst}_{c}", tag="c")
                    nc.tensor.matmul(out=pt[:], lhsT=w_t[l][:],
                                     rhs=state_t[:, cst + c:cst + c + w],
                                     start=True, stop=True)
                    nc.vector.tensor_copy(out=xp[:, c:c + w], in_=pt[:])
                    c += w
                nck = csz // 128
                xpnm = sb.tile([128, nck, 128], F16,
                               name=f"xpnm{l}_{cst}", tag="xpnm")
                for t in range(nck):
                    tp = ps2.tile([128, 128], F32, space="PSUM",
                                  name=f"tp{l}_{cst}_{t}", tag="c")
                    nc.tensor.transpose(out=tp[:],
                                        in_=xp[:, 128 * t:128 * (t + 1)],
                                        identity=ident_t[:])
                    nc.vector.tensor_copy(out=xpnm[:, t, :], in_=tp[:])
                nc.sync.dma_start(
                    out=bounce[l][cst:cst + csz, :]
                        .rearrange("(c p) f -> p c f", p=128),
                    in_=xpnm[:])
                tb0 = NC * cst
                nc.gpsimd.collective_compute(
                    "AllGather", mybir.AluOpType.bypass,
                    replica_groups=[list(range(NC))],
                    ins=[bounce[l][cst:cst + csz, :]],
                    outs=[table[l][tb0:tb0 + NC * csz, :]])

            # gathers land in per-window grouped tiles so the WAR wait is
            # per-window (4-slot rotation), not per-gather
            def gather_group(l, cols, src_idx_t, gname, gtag):
                mt = sb.tile([128, len(cols) * 128], F16, name=gname,
                             tag=gtag)
                views = []
                for i, col in enumerate(cols):
                    v = mt[:, 128 * i:128 * (i + 1)]
                    nc.gpsimd.indirect_dma_start(
                        out=v, out_offset=None, in_=table[l][:],
                        in_offset=bass.IndirectOffsetOnAxis(
                            ap=src_idx_t[:, col:col + 1], axis=0))
                    views.append(v)
                return views

            # layer 0's table comes straight from the input state
            for (cst, csz) in chunks:
                emit_chunk(0, state, cst, csz)

            for l in range(2):
                nstate = sb.tile([128, shard], F16, name=f"state{l + 1}")
                for w in range(nwin):
                    wst, ww = wins[w]
                    pw = ps.tile([128, ww], F32, space="PSUM",
                                 name=f"pw{l}_{w}", tag="a")
                    nb = int(bw[w])
                    cols = [int(offs[w]) + j for j in range(nb)]
                    mviews = gather_group(l, cols, gidx_t,
                                          f"mg{l}_{w}", f"gw{w % 4}")
                    for j, col in enumerate(cols):
                        oh = sb.tile([128, ww], F16, name=f"oh{l}_{col}",
                                     tag=f"oh{col % 16}")
                        nc.vector.tensor_scalar(
                            out=oh[:], in0=iota_t[:, :ww],
                            scalar1=dstloc_t[:, col:col + 1],
                            scalar2=wcol_t[:, col:col + 1],
                            op0=mybir.AluOpType.is_equal,
                            op1=mybir.AluOpType.mult)
                        nc.tensor.matmul(out=pw[:], lhsT=mviews[j], rhs=oh[:],
                                         start=(j == 0), stop=(j == nb - 1))
                    nc.vector.tensor_scalar(
                        out=nstate[:, wst:wst + ww], in0=pw[:],
                        scalar1=b_t[l][:, 0:1], scalar2=None,
                        op0=mybir.AluOpType.add)
                    # emit next layer's table chunks as soon as their
                    # columns are final -- overlaps collectives with gathers
                    for ci, (cst, csz) in enumerate(chunks):
                        if trig[ci] == w:
                            emit_chunk(l + 1, nstate, cst, csz)
                state = nstate

            # ---------- layer 3 fused with mean-pool (table emitted above)
            pp = ps2.tile([128, G], F32, space="PSUM", name="pp", tag="b")
            GRP3 = 32
            b3 = 0
            while b3 < nb3:
                cols = list(range(b3, min(b3 + GRP3, nb3)))
                mviews = gather_group(2, cols, gidx3_t,
                                      f"mg3_{b3}", f"g3{(b3 // GRP3) % 3}")
                for i, col in enumerate(cols):
                    nc.tensor.matmul(out=pp[:], lhsT=mviews[i],
                                     rhs=oh3_t[:, G * col:G * (col + 1)],
                                     start=(col == 0), stop=(col == nb3 - 1))
                b3 += GRP3
            ppool = sb.tile([128, G], F32, name="ppool")
            nc.vector.tensor_copy(out=ppool[:], in_=pp[:])
            nc.sync.dma_start(out=pool_in[:], in_=ppool[:])
            nc.gpsimd.collective_compute(
                "AllReduce", mybir.AluOpType.add,
                replica_groups=[list(range(NC))],
                ins=[pool_in.ap()], outs=[pool_out.ap()])
            pooled_f = sb.tile([128, G], F32, name="pooled_f")
            nc.sync.dma_start(out=pooled_f[:], in_=pool_out[:])
            nc.vector.tensor_scalar(out=pooled_f[:], in0=pooled_f[:],
                                    scalar1=b_t[2][:, 0:1], scalar2=None,
                                    op0=mybir.AluOpType.add)
            pooled = sb.tile([128, G], F16, name="pooled")
            nc.vector.tensor_copy(out=pooled[:], in_=pooled_f[:])

            # ---------- MLP head (replicated)
            _hctr = [0]

            def lrelu_evict(psrc, bias_tile, bias_c, dst16, nrows=128):
                _hctr[0] += 1
                t1 = sb.tile([128, G], F32, name=f"t1_{_hctr[0]}", tag="h1")
                nc.vector.tensor_scalar(out=t1[:nrows], in0=psrc[:nrows],
                                        scalar1=bias_tile[:nrows,
                                                          bias_c:bias_c + 1],
                                        scalar2=None,
                                        op0=mybir.AluOpType.add)
                t2 = sb.tile([128, G], F32, name=f"t2_{_hctr[0]}", tag="h2")
                nc.vector.tensor_scalar(out=t2[:nrows], in0=t1[:nrows],
                                        scalar1=NEG, scalar2=None,
                                        op0=mybir.AluOpType.mult)
                nc.vector.tensor_tensor(out=dst16[:nrows], in0=t1[:nrows],
                                        in1=t2[:nrows],
                                        op=mybir.AluOpType.max)

            wc1_f = sb.tile([128, 512], F32, name="wc1_f")
            nc.sync.dma_start(out=wc1_f[:], in_=wc1[:])
            wc1_h = sb.tile([128, 512], F16, name="wc1_h")
            nc.vector.tensor_copy(out=wc1_h[:], in_=wc1_f[:])
            bc1_t = sb.tile([128, 4], F32, name="bc1_t")
            nc.sync.dma_start(out=bc1_t[:], in_=bc1[:])
            wc2_f = sb.tile([128, 4 * 256], F32, name="wc2_f")
            nc.sync.dma_start(out=wc2_f[:], in_=wc2[:])
            wc2_h = sb.tile([128, 4 * 256], F16, name="wc2_h")
            nc.vector.tensor_copy(out=wc2_h[:], in_=wc2_f[:])
            bc2_t = sb.tile([128, 2], F32, name="bc2_t")
            nc.sync.dma_start(out=bc2_t[:], in_=bc2[:])
            wc3_f = sb.tile([128, 2 * OUT], F32, name="wc3_f")
            nc.sync.dma_start(out=wc3_f[:], in_=wc3[:])
            wc3_h = sb.tile([128, 2 * OUT], F16, name="wc3_h")
            nc.vector.tensor_copy(out=wc3_h[:], in_=wc3_f[:])
            bc3_t = sb.tile([128, 1], F32, name="bc3_t")
            nc.sync.dma_start(out=bc3_t[:], in_=bc3[:])

            z1 = sb.tile([128, 4 * G], F16, name="z1")
            for c in range(4):
                ph = ps2.tile([128, G], F32, space="PSUM",
                              name=f"ph1_{c}", tag="b")
                nc.tensor.matmul(out=ph[:], lhsT=wc1_h[:, 128 * c:128 * (c + 1)],
                                 rhs=pooled[:], start=True, stop=True)
                lrelu_evict(ph, bc1_t, c, z1[:, G * c:G * (c + 1)])
            z2 = sb.tile([128, 2 * G], F16, name="z2")
            for jj in range(2):
                ph = ps2.tile([128, G], F32, space="PSUM",
                              name=f"ph2_{jj}", tag="b")
                for c in range(4):
                    nc.tensor.matmul(
                        out=ph[:],
                        lhsT=wc2_h[:, 256 * c + 128 * jj:256 * c + 128 * (jj + 1)],
                        rhs=z1[:, G * c:G * (c + 1)],
                        start=(c == 0), stop=(c == 3))
                lrelu_evict(ph, bc2_t, jj, z2[:, G * jj:G * (jj + 1)])
            ph3 = ps2.tile([128, G], F32, space="PSUM", name="ph3", tag="b")
            for c in range(2):
                nc.tensor.matmul(out=ph3[:OUT],
                                 lhsT=wc3_h[:, OUT * c:OUT * (c + 1)],
                                 rhs=z2[:, G * c:G * (c + 1)],
                                 start=(c == 0), stop=(c == 1))
            zout = sb.tile([128, G], F32, name="zout")
            lrelu_evict(ph3, bc3_t, 0, zout, nrows=OUT)
            nc.sync.dma_start(out=out_d[:], in_=zout[:OUT])

    split_multiwaits(nc)
    return nc


# ---------------------------------------------------------------- driver

def run_gcn(x, src, dst, gid, W1, b1, W2, b2, W3, b3,
            Wc1, bc1, Wc2, bc2, Wc3, bc3, n_nodes, shard,
            trace=False, tmpdir=None):
    x = np.asarray(x, np.float32)
    src = np.asarray(src).astype(np.int64)
    dst = np.asarray(dst).astype(np.int64)
    gid = np.asarray(gid).astype(np.int64)

    per_core, node_core, node_slot, bw, offs, nbw, nb3 = _prep(
        x, src, dst, gid, n_nodes, shard)
    nc = build_program(shard, nbw, nb3, bw, offs)

    iota = np.tile(np.arange(WIN, dtype=np.float32), (128, 1))
    ident = np.eye(128, dtype=np.float32)

    def colvec(v, n=128):
        a = np.zeros((n, 1), np.float32)
        a[:len(v), 0] = v
        return a

    bc1_a = np.asarray(bc1, np.float32).reshape(4, 128).T.copy()
    bc2_a = np.asarray(bc2, np.float32).reshape(2, 128).T.copy()
    wc2_a = np.concatenate(
        [np.asarray(Wc2, np.float32)[128 * c:128 * (c + 1), :] for c in range(4)],
        axis=1)
    wc3_a = np.concatenate(
        [np.asarray(Wc3, np.float32)[128 * c:128 * (c + 1), :] for c in range(2)],
        axis=1)

    in_maps = []
    for k in range(NC):
        pc = per_core[k]
        xk = np.zeros((shard, D), np.float32)
        mk = node_core == k
        xk[node_slot[mk]] = x[mk]
        in_maps.append({
            "xT": np.ascontiguousarray(xk.T),
            "w1": np.asarray(W1, np.float32), "b1": colvec(np.asarray(b1)),
            "w2": np.asarray(W2, np.float32), "b2": colvec(np.asarray(b2)),
            "w3": np.asarray(W3, np.float32), "b3": colvec(np.asarray(b3)),
            "wc1": np.asarray(Wc1, np.float32), "bc1": bc1_a,
            "wc2": wc2_a, "bc2": bc2_a,
            "wc3": wc3_a, "bc3": colvec(np.asarray(bc3)),
            "gidx": pc["gidx"], "dstloc": pc["dstloc"], "wcol": pc["wcol"],
            "gidx3": pc["gidx3"], "oh3": pc["oh3"],
            "iota": iota, "ident": ident,
        })

    res = run_bass_kernel_spmd(nc, in_maps, core_ids=list(range(NC)),
                               trace=trace, tmpdir=tmpdir)
    out = res.results[0]["out"].T.astype(np.float32).copy()  # [G, OUT]
    return out, res


def kernel(**inputs):
    out, _ = run_gcn(
        inputs["inputs"], inputs["src"], inputs["dst"], inputs["graph_id"],
        inputs["W1"], inputs["b1"], inputs["W2"], inputs["b2"],
        inputs["W3"], inputs["b3"],
        inputs["Wc1"], inputs["bc1"], inputs["Wc2"], inputs["bc2"],
        inputs["Wc3"], inputs["bc3"],
        n_nodes=50000, shard=6272)
    return out


# revision 10
# speedup vs baseline: 1.5596x; 1.4288x over previous
"""GCN (3x GraphConv + mean-pool + MLP head) on 8 Trainium2 NeuronCores.

Strategy (SPMD, one program on all 8 cores):
  - Nodes are assigned to (core, 256-dst-window) buckets balanced by
    in-degree, so every core/window sees ~equal edge counts and the
    SPMD-uniform batch schedule wastes almost no padding.
  - Weight matrices replicated; degree norms folded into per-edge weights
    w_e = src_norm[src] * dst_norm[dst] carried by the scatter one-hots.
  - Per layer: project own shard (fp16 matmuls, PSUM f32) -> PE-transpose to
    node-major -> AllGather (2 pipelined chunks) into a DRAM table
    [50176, 128] fp16 -> gather each edge's source row via indirect DMA
    (128 rows/instruction, edges grouped per dst window) -> scatter-add via
    matmul with an on-chip weighted one-hot (iota==dst_local)*w_e
    accumulating in PSUM per window.
  - Layer 3's scatter is fused with dgl.mean_nodes: its one-hot is the
    8-wide graph-membership matrix scaled by w_e/cnt_g, so the whole last
    aggregation lands directly in a [128,8] pooled accumulator; partials
    are AllReduced and the tiny MLP head runs replicated on every core.
"""

import heapq
import sys

sys.path.insert(0, "/opt/trn_rl_repo")

import numpy as np

import concourse.bass as bass
import concourse.mybir as mybir
import concourse.tile as tile
import bass_rust
from concourse.bass_utils import run_bass_kernel_spmd

F32 = mybir.dt.float32
F16 = mybir.dt.float16
I32 = mybir.dt.int32

NC = 8          # cores
D = 128         # feature dim (== partition width)
G = 8           # graphs
OUT = 10
NEG = 0.01      # LeakyReLU slope
WIN = 256       # dst window per PSUM tile

_split_ctr = [0]


def split_multiwaits(nc):
    """This walrus encodes at most ONE sync-wait per instruction; hoist
    extra waits into preceding EventSemaphore ops on the same engine."""
    for f in nc.m.functions:
        for blk in f.blocks:
            insts = list(blk.instructions)
            new, changed = [], False
            for inst in insts:
                si = inst.sync_info
                if si is not None and len(si.on_wait) > 1:
                    waits = list(si.on_wait)
                    for w in waits[:-1]:
                        _split_ctr[0] += 1
                        es = mybir.InstEventSemaphore(
                            name=f"mwsplit_{_split_ctr[0]}", ins=[], outs=[])
                        es.engine = inst.engine
                        es.sync_info = bass_rust.SyncInfo(on_wait=[w], on_update=[])
                        new.append(es)
                    si.on_wait = waits[-1:]
                    changed = True
                new.append(inst)
            if changed:
                blk.instructions = new


def _chunks(shard):
    """Up to 4 roughly-equal 128-aligned table chunks covering [0, shard)."""
    ntile = shard // 128
    n = min(4, ntile)
    base = ntile // n
    sizes = [base] * n
    for i in range(ntile - base * n):
        sizes[n - 1 - i] += 1
    out, c = [], 0
    for z in sizes:
        out.append((c, z * 128))
        c += z * 128
    return out


def _windows(shard):
    """List of (start, width) dst windows covering [0, shard)."""
    out = []
    c = 0
    while c < shard:
        w = min(WIN, shard - c)
        out.append((c, w))
        c += w
    return out


# ---------------------------------------------------------------- host prep

def _prep(x, src, dst, gid, n_nodes, shard):
    wins = _windows(shard)
    nwin = len(wins)
    chunks = _chunks(shard)

    out_deg = np.bincount(src, minlength=n_nodes)
    in_deg = np.bincount(dst, minlength=n_nodes)
    snorm = np.clip(out_deg, 1, None).astype(np.float32) ** -0.5
    dnorm = np.clip(in_deg, 1, None).astype(np.float32) ** -0.5
    we = snorm[src] * dnorm[dst]
    cnt = np.bincount(gid, minlength=G).astype(np.float32)
    cinv = (1.0 / np.clip(cnt, 1, None)).astype(np.float32)

    # --- balanced node -> (core, window) assignment by in-degree
    order = np.argsort(-in_deg, kind="stable")
    heap = [(0.0, k * nwin + w) for k in range(NC) for w in range(nwin)]
    heapq.heapify(heap)
    fill = [[0] * nwin for _ in range(NC)]
    node_core = np.empty(n_nodes, np.int32)
    node_slot = np.empty(n_nodes, np.int32)
    for n in order:
        while True:
            load, b = heapq.heappop(heap)
            k, w = divmod(b, nwin)
            if fill[k][w] < wins[w][1]:
                break
        node_core[n] = k
        node_slot[n] = wins[w][0] + fill[k][w]
        fill[k][w] += 1
        heapq.heappush(heap, (load + float(in_deg[n]), b))

    # global gather-table id (matches the chunked AllGather layout):
    # chunk c of every rank is contiguous: row = NC*S_c + k*Z_c + (s - S_c)
    s = node_slot.astype(np.int64)
    kk = node_core.astype(np.int64)
    gid_tab = np.zeros(len(s), np.int64)
    for (cst, csz) in chunks:
        m = (s >= cst) & (s < cst + csz)
        gid_tab[m] = NC * cst + kk[m] * csz + (s[m] - cst)
    gid_tab = gid_tab.astype(np.int32)

    e_core = node_core[dst]
    e_slot_all = node_slot[dst]

    wcnt = np.zeros((NC, nwin), np.int64)
    for k in range(NC):
        m = e_core == k
        wcnt[k] = np.bincount(e_slot_all[m] // WIN, minlength=nwin)
    bw = np.maximum(1, -(-wcnt.max(axis=0) // 128)).astype(np.int64)
    offs = np.concatenate([[0], np.cumsum(bw)])
    nbw = int(offs[-1])
    nb3 = max(1, max(-(-int((e_core == k).sum()) // 128) for k in range(NC)))

    per_core = []
    for k in range(NC):
        m = e_core == k
        e_src = src[m].astype(np.int64)
        e_slot = e_slot_all[m].astype(np.int64)
        e_w = we[m].astype(np.float32)
        e_g = gid[dst[m]].astype(np.int64)
        o = np.argsort(e_slot, kind="stable")
        e_src, e_slot, e_w, e_g = e_src[o], e_slot[o], e_w[o], e_g[o]

        gidx = np.zeros((128, nbw), np.int32)
        dstloc = np.full((128, nbw), -1.0, np.float32)
        wcol = np.zeros((128, nbw), np.float32)
        win = e_slot // WIN
        pos = np.zeros(len(e_src), np.int64)
        for w in range(nwin):
            mm = win == w
            pos[mm] = np.arange(mm.sum())
        col = offs[win] + pos // 128
        row = pos % 128
        gidx[row, col] = gid_tab[e_src]
        dstloc[row, col] = (e_slot - win * WIN).astype(np.float32)
        wcol[row, col] = e_w

        # layer-3 SpMM folds into mean-pool: per-node pooling weights
        # q[n, g] = sum_{e: src=n} w_e * cinv[g] * [graph(dst_e) == g]
        qk = np.zeros((shard, G), np.float32)
        loc = node_slot[e_src]          # unused; q is per SRC node below
        per_core.append(dict(gidx=gidx, dstloc=dstloc, wcol=wcol))
    # q over all nodes, then scatter into each core's slot order
    q_full = np.zeros((n_nodes, G), np.float32)
    np.add.at(q_full, (src, gid[dst]), we * cinv[gid[dst]])
    ntile = shard // 128
    for k in range(NC):
        mk = node_core == k
        qk = np.zeros((shard, G), np.float32)
        qk[node_slot[mk]] = q_full[mk]
        per_core[k]["q3"] = np.ascontiguousarray(
            qk.reshape(ntile, 128, G).transpose(1, 0, 2).reshape(128, ntile * G))
    return per_core, node_core, node_slot, bw, offs, nbw, nb3


# ---------------------------------------------------------------- program

def build_program(shard, nbw, nb3, bw, offs):
    npad = NC * shard
    wins = _windows(shard)
    nwin = len(wins)
    chunks = _chunks(shard)
    # window index after which each chunk's columns are fully evicted
    trig = []
    for (cst, csz) in chunks:
        w = 0
        while wins[w][0] + wins[w][1] < cst + csz:
            w += 1
        trig.append(w)

    nc = bass.Bass("TRN2", target_bir_lowering=False, debug=False,
                   num_devices=NC)

    xT = nc.dram_tensor("xT", [128, shard], F32, kind="ExternalInput")
    wg = [nc.dram_tensor(f"w{l}", [128, 128], F32, kind="ExternalInput")
          for l in (1, 2, 3)]
    bg = [nc.dram_tensor(f"b{l}", [128, 1], F32, kind="ExternalInput")
          for l in (1, 2, 3)]
    wc1 = nc.dram_tensor("wc1", [128, 512], F32, kind="ExternalInput")
    bc1 = nc.dram_tensor("bc1", [128, 4], F32, kind="ExternalInput")
    wc2 = nc.dram_tensor("wc2", [128, 4 * 256], F32, kind="ExternalInput")
    bc2 = nc.dram_tensor("bc2", [128, 2], F32, kind="ExternalInput")
    wc3 = nc.dram_tensor("wc3", [128, 2 * OUT], F32, kind="ExternalInput")
    bc3 = nc.dram_tensor("bc3", [128, 1], F32, kind="ExternalInput")
    gidx_in = nc.dram_tensor("gidx", [128, nbw], I32, kind="ExternalInput")
    dstloc_in = nc.dram_tensor("dstloc", [128, nbw], F32, kind="ExternalInput")
    wcol_in = nc.dram_tensor("wcol", [128, nbw], F32, kind="ExternalInput")
    ntile = shard // 128
    q3_in = nc.dram_tensor("q3", [128, ntile * G], F32, kind="ExternalInput")
    iota_in = nc.dram_tensor("iota", [128, WIN], F32, kind="ExternalInput")
    ident_in = nc.dram_tensor("ident", [128, 128], F32, kind="ExternalInput")
    out_d = nc.dram_tensor("out", [OUT, G], F32, kind="ExternalOutput")

    bounce = [nc.dram_tensor(f"bnc{l}", [shard, 128], F16) for l in range(2)]
    table = [nc.dram_tensor(f"tbl{l}", [npad, 128], F16) for l in range(2)]
    pool_in = nc.dram_tensor("pool_in", [128, G], F32)
    pool_out = nc.dram_tensor("pool_out", [128, G], F32)

    with tile.TileContext(nc) as tc:
        with tc.tile_pool(name="sb", bufs=1) as sb, \
             tc.tile_pool(name="ps", bufs=4, space="PSUM") as ps, \
             tc.tile_pool(name="ps2", bufs=2, space="PSUM") as ps2:

            # critical-path loads first: xT, W1, identity
            xT_t = sb.tile([128, shard], F32, name="xT_t")
            nc.sync.dma_start(out=xT_t[:], in_=xT[:])
            ident_t = sb.tile([128, 128], F32, name="ident_t")
            nc.sync.dma_start(out=ident_t[:], in_=ident_in[:])
            w_t, b_t = [], []
            for l in range(3):
                wf = sb.tile([128, 128], F32, name=f"wf{l}")
                nc.sync.dma_start(out=wf[:], in_=wg[l][:])
                wh = sb.tile([128, 128], F16, name=f"wh{l}")
                nc.vector.tensor_copy(out=wh[:], in_=wf[:])
                w_t.append(wh)
                bt = sb.tile([128, 1], F32, name=f"bt{l}")
                nc.sync.dma_start(out=bt[:], in_=bg[l][:])
                b_t.append(bt)
            state = sb.tile([128, shard], F16, name="state0")
            nc.vector.tensor_copy(out=state[:], in_=xT_t[:])

            iota_t = sb.tile([128, WIN], F32, name="iota_t")
            nc.sync.dma_start(out=iota_t[:], in_=iota_in[:])
            gidx_t = sb.tile([128, nbw], I32, name="gidx_t")
            nc.sync.dma_start(out=gidx_t[:], in_=gidx_in[:])
            dstloc_t = sb.tile([128, nbw], F32, name="dstloc_t")
            nc.sync.dma_start(out=dstloc_t[:], in_=dstloc_in[:])
            wcol_t = sb.tile([128, nbw], F32, name="wcol_t")
            nc.sync.dma_start(out=wcol_t[:], in_=wcol_in[:])
            q3_f32 = sb.tile([128, ntile * G], F32, name="q3_f32")
            nc.sync.dma_start(out=q3_f32[:], in_=q3_in[:])
            q3_t = sb.tile([128, ntile * G], F16, name="q3_t")
            nc.vector.tensor_copy(out=q3_t[:], in_=q3_f32[:])

            def emit_chunk(l, state_t, cst, csz):
                """Project chunk [cst, cst+csz) of layer l's state, transpose
                to node-major, bounce to DRAM, AllGather into table[l]."""
                xp = sb.tile([128, csz], F32, name=f"xp{l}_{cst}", tag="xpf")
                c = 0
                while c < csz:
                    w = min(512, csz - c)
                    pt = ps2.tile([128, w], F32, space="PSUM",
                                  name=f"pj{l}_{cst}_{c}", tag="c")
                    nc.tensor.matmul(out=pt[:], lhsT=w_t[l][:],
                                     rhs=state_t[:, cst + c:cst + c + w],
                                     start=True, stop=True)
                    nc.vector.tensor_copy(out=xp[:, c:c + w], in_=pt[:])
                    c += w
                nck = csz // 128
                xpnm = sb.tile([128, nck, 128], F16,
                               name=f"xpnm{l}_{cst}", tag="xpnm")
                for t in range(nck):
                    tp = ps2.tile([128, 128], F32, space="PSUM",
                                  name=f"tp{l}_{cst}_{t}", tag="c")
                    nc.tensor.transpose(out=tp[:],
                                        in_=xp[:, 128 * t:128 * (t + 1)],
                                        identity=ident_t[:])
                    nc.vector.tensor_copy(out=xpnm[:, t, :], in_=tp[:])
                nc.sync.dma_start(
                    out=bounce[l][cst:cst + csz, :]
                        .rearrange("(c p) f -> p c f", p=128),
                    in_=xpnm[:])
                tb0 = NC * cst
                nc.gpsimd.collective_compute(
                    "AllGather", mybir.AluOpType.bypass,
                    replica_groups=[list(range(NC))],
                    ins=[bounce[l][cst:cst + csz, :]],
                    outs=[table[l][tb0:tb0 + NC * csz, :]])

            # gathers land in per-window grouped tiles so the WAR wait is
            # per-window (4-slot rotation), not per-gather
            def gather_group(l, cols, src_idx_t, gname, gtag):
                mt = sb.tile([128, len(cols) * 128], F16, name=gname,
                             tag=gtag)
                views = []
                for i, col in enumerate(cols):
                    v = mt[:, 128 * i:128 * (i + 1)]
                    nc.gpsimd.indirect_dma_start(
                        out=v, out_offset=None, in_=table[l][:],
                        in_offset=bass.IndirectOffsetOnAxis(
                            ap=src_idx_t[:, col:col + 1], axis=0))
                    views.append(v)
                return views

            # layer-3 x W3 projection folds into pooling: per chunk,
            # project+transpose state2 and contract with q3 into pp
            pp = ps2.tile([128, G], F32, space="PSUM", name="pp", tag="b")
            _pcnt = [0]

            def emit_pool_chunk(state_t, cst, csz):
                xp = sb.tile([128, csz], F32, name=f"xpq_{cst}", tag="xpf")
                c = 0
                while c < csz:
                    w = min(512, csz - c)
                    pt = ps2.tile([128, w], F32, space="PSUM",
                                  name=f"pjq_{cst}_{c}", tag="c")
                    nc.tensor.matmul(out=pt[:], lhsT=w_t[2][:],
                                     rhs=state_t[:, cst + c:cst + c + w],
                                     start=True, stop=True)
                    nc.vector.tensor_copy(out=xp[:, c:c + w], in_=pt[:])
                    c += w
                nck = csz // 128
                xpnm = sb.tile([128, nck, 128], F16,
                               name=f"xpnmq_{cst}", tag="xpnm")
                for t in range(nck):
                    tp = ps2.tile([128, 128], F32, space="PSUM",
                                  name=f"tpq_{cst}_{t}", tag="c")
                    nc.tensor.transpose(out=tp[:],
                                        in_=xp[:, 128 * t:128 * (t + 1)],
                                        identity=ident_t[:])
                    nc.vector.tensor_copy(out=xpnm[:, t, :], in_=tp[:])
                    gt = cst // 128 + t
                    nc.tensor.matmul(out=pp[:], lhsT=xpnm[:, t, :],
                                     rhs=q3_t[:, G * gt:G * (gt + 1)],
                                     start=(_pcnt[0] == 0),
                                     stop=(_pcnt[0] == ntile - 1))
                    _pcnt[0] += 1

            # layer 0's table comes straight from the input state
            for (cst, csz) in chunks:
                emit_chunk(0, state, cst, csz)

            for l in range(2):
                nstate = sb.tile([128, shard], F16, name=f"state{l + 1}")
                for w in range(nwin):
                    wst, ww = wins[w]
                    pw = ps.tile([128, ww], F32, space="PSUM",
                                 name=f"pw{l}_{w}", tag="a")
                    nb = int(bw[w])
                    cols = [int(offs[w]) + j for j in range(nb)]
                    mviews = gather_group(l, cols, gidx_t,
                                          f"mg{l}_{w}", f"gw{w % 4}")
                    for j, col in enumerate(cols):
                        oh = sb.tile([128, ww], F16, name=f"oh{l}_{col}",
                                     tag=f"oh{col % 16}")
                        nc.vector.tensor_scalar(
                            out=oh[:], in0=iota_t[:, :ww],
                            scalar1=dstloc_t[:, col:col + 1],
                            scalar2=wcol_t[:, col:col + 1],
                            op0=mybir.AluOpType.is_equal,
                            op1=mybir.AluOpType.mult)
                        nc.tensor.matmul(out=pw[:], lhsT=mviews[j], rhs=oh[:],
                                         start=(j == 0), stop=(j == nb - 1))
                    nc.vector.tensor_scalar(
                        out=nstate[:, wst:wst + ww], in0=pw[:],
                        scalar1=b_t[l][:, 0:1], scalar2=None,
                        op0=mybir.AluOpType.add)
                    # emit the next stage's work as soon as the columns
                    # are final -- overlaps collectives/PE with gathers
                    for ci, (cst, csz) in enumerate(chunks):
                        if trig[ci] == w:
                            if l == 0:
                                emit_chunk(1, nstate, cst, csz)
                            else:
                                emit_pool_chunk(nstate, cst, csz)
                state = nstate

            # ---------- pooled accumulator already filled by emit_pool_chunk
            ppool = sb.tile([128, G], F32, name="ppool")
            nc.vector.tensor_copy(out=ppool[:], in_=pp[:])
            nc.sync.dma_start(out=pool_in[:], in_=ppool[:])
            nc.gpsimd.collective_compute(
                "AllReduce", mybir.AluOpType.add,
                replica_groups=[list(range(NC))],
                ins=[pool_in.ap()], outs=[pool_out.ap()])
            pooled_f = sb.tile([128, G], F32, name="pooled_f")
            nc.sync.dma_start(out=pooled_f[:], in_=pool_out[:])
            nc.vector.tensor_scalar(out=pooled_f[:], in0=pooled_f[:],
                                    scalar1=b_t[2][:, 0:1], scalar2=None,
                                    op0=mybir.AluOpType.add)
            pooled = sb.tile([128, G], F16, name="pooled")
            nc.vector.tensor_copy(out=pooled[:], in_=pooled_f[:])

            # ---------- MLP head (replicated)
            _hctr = [0]

            def lrelu_evict(psrc, bias_tile, bias_c, dst16, nrows=128):
                _hctr[0] += 1
                t1 = sb.tile([128, G], F32, name=f"t1_{_hctr[0]}", tag="h1")
                nc.vector.tensor_scalar(out=t1[:nrows], in0=psrc[:nrows],
                                        scalar1=bias_tile[:nrows,
                                                          bias_c:bias_c + 1],
                                        scalar2=None,
                                        op0=mybir.AluOpType.add)
                t2 = sb.tile([128, G], F32, name=f"t2_{_hctr[0]}", tag="h2")
                nc.vector.tensor_scalar(out=t2[:nrows], in0=t1[:nrows],
                                        scalar1=NEG, scalar2=None,
                                        op0=mybir.AluOpType.mult)
                nc.vector.tensor_tensor(out=dst16[:nrows], in0=t1[:nrows],
                                        in1=t2[:nrows],
                                        op=mybir.AluOpType.max)

            wc1_f = sb.tile([128, 512], F32, name="wc1_f")
            nc.sync.dma_start(out=wc1_f[:], in_=wc1[:])
            wc1_h = sb.tile([128, 512], F16, name="wc1_h")
            nc.vector.tensor_copy(out=wc1_h[:], in_=wc1_f[:])
            bc1_t = sb.tile([128, 4], F32, name="bc1_t")
            nc.sync.dma_start(out=bc1_t[:], in_=bc1[:])
            wc2_f = sb.tile([128, 4 * 256], F32, name="wc2_f")
            nc.sync.dma_start(out=wc2_f[:], in_=wc2[:])
            wc2_h = sb.tile([128, 4 * 256], F16, name="wc2_h")
            nc.vector.tensor_copy(out=wc2_h[:], in_=wc2_f[:])
            bc2_t = sb.tile([128, 2], F32, name="bc2_t")
            nc.sync.dma_start(out=bc2_t[:], in_=bc2[:])
            wc3_f = sb.tile([128, 2 * OUT], F32, name="wc3_f")
            nc.sync.dma_start(out=wc3_f[:], in_=wc3[:])
            wc3_h = sb.tile([128, 2 * OUT], F16, name="wc3_h")
            nc.vector.tensor_copy(out=wc3_h[:], in_=wc3_f[:])
            bc3_t = sb.tile([128, 1], F32, name="bc3_t")
            nc.sync.dma_start(out=bc3_t[:], in_=bc3[:])

            z1 = sb.tile([128, 4 * G], F16, name="z1")
            for c in range(4):
                ph = ps2.tile([128, G], F32, space="PSUM",
                              name=f"ph1_{c}", tag="b")
                nc.tensor.matmul(out=ph[:], lhsT=wc1_h[:, 128 * c:128 * (c + 1)],
                                 rhs=pooled[:], start=True, stop=True)
                lrelu_evict(ph, bc1_t, c, z1[:, G * c:G * (c + 1)])
            z2 = sb.tile([128, 2 * G], F16, name="z2")
            for jj in range(2):
                ph = ps2.tile([128, G], F32, space="PSUM",
                              name=f"ph2_{jj}", tag="b")
                for c in range(4):
                    nc.tensor.matmul(
                        out=ph[:],
                        lhsT=wc2_h[:, 256 * c + 128 * jj:256 * c + 128 * (jj + 1)],
                        rhs=z1[:, G * c:G * (c + 1)],
                        start=(c == 0), stop=(c == 3))
                lrelu_evict(ph, bc2_t, jj, z2[:, G * jj:G * (jj + 1)])
            ph3 = ps2.tile([128, G], F32, space="PSUM", name="ph3", tag="b")
            for c in range(2):
                nc.tensor.matmul(out=ph3[:OUT],
                                 lhsT=wc3_h[:, OUT * c:OUT * (c + 1)],
                                 rhs=z2[:, G * c:G * (c + 1)],
                                 start=(c == 0), stop=(c == 1))
            zout = sb.tile([128, G], F32, name="zout")
            lrelu_evict(ph3, bc3_t, 0, zout, nrows=OUT)
            nc.sync.dma_start(out=out_d[:], in_=zout[:OUT])

    split_multiwaits(nc)
    return nc


# ---------------------------------------------------------------- driver

def run_gcn(x, src, dst, gid, W1, b1, W2, b2, W3, b3,
            Wc1, bc1, Wc2, bc2, Wc3, bc3, n_nodes, shard,
            trace=False, tmpdir=None):
    x = np.asarray(x, np.float32)
    src = np.asarray(src).astype(np.int64)
    dst = np.asarray(dst).astype(np.int64)
    gid = np.asarray(gid).astype(np.int64)

    per_core, node_core, node_slot, bw, offs, nbw, nb3 = _prep(
        x, src, dst, gid, n_nodes, shard)
    nc = build_program(shard, nbw, nb3, bw, offs)

    iota = np.tile(np.arange(WIN, dtype=np.float32), (128, 1))
    ident = np.eye(128, dtype=np.float32)

    def colvec(v, n=128):
        a = np.zeros((n, 1), np.float32)
        a[:len(v), 0] = v
        return a

    bc1_a = np.asarray(bc1, np.float32).reshape(4, 128).T.copy()
    bc2_a = np.asarray(bc2, np.float32).reshape(2, 128).T.copy()
    wc2_a = np.concatenate(
        [np.asarray(Wc2, np.float32)[128 * c:128 * (c + 1), :] for c in range(4)],
        axis=1)
    wc3_a = np.concatenate(
        [np.asarray(Wc3, np.float32)[128 * c:128 * (c + 1), :] for c in range(2)],
        axis=1)

    in_maps = []
    for k in range(NC):
        pc = per_core[k]
        xk = np.zeros((shard, D), np.float32)
        mk = node_core == k
        xk[node_slot[mk]] = x[mk]
        in_maps.append({
            "xT": np.ascontiguousarray(xk.T),
            "w1": np.asarray(W1, np.float32), "b1": colvec(np.asarray(b1)),
            "w2": np.asarray(W2, np.float32), "b2": colvec(np.asarray(b2)),
            "w3": np.asarray(W3, np.float32), "b3": colvec(np.asarray(b3)),
            "wc1": np.asarray(Wc1, np.float32), "bc1": bc1_a,
            "wc2": wc2_a, "bc2": bc2_a,
            "wc3": wc3_a, "bc3": colvec(np.asarray(bc3)),
            "gidx": pc["gidx"], "dstloc": pc["dstloc"], "wcol": pc["wcol"],
            "q3": pc["q3"],
            "iota": iota, "ident": ident,
        })

    res = run_bass_kernel_spmd(nc, in_maps, core_ids=list(range(NC)),
                               trace=trace, tmpdir=tmpdir)
    out = res.results[0]["out"].T.astype(np.float32).copy()  # [G, OUT]
    return out, res


def kernel(**inputs):
    out, _ = run_gcn(
        inputs["inputs"], inputs["src"], inputs["dst"], inputs["graph_id"],
        inputs["W1"], inputs["b1"], inputs["W2"], inputs["b2"],
        inputs["W3"], inputs["b3"],
        inputs["Wc1"], inputs["bc1"], inputs["Wc2"], inputs["bc2"],
        inputs["Wc3"], inputs["bc3"],
        n_nodes=50000, shard=6272)
    return out


# revision 11
# speedup vs baseline: 1.5741x; 1.0093x over previous
"""GCN (3x GraphConv + mean-pool + MLP head) on 8 Trainium2 NeuronCores.

Strategy (SPMD, one program on all 8 cores):
  - Nodes are assigned to (core, 256-dst-window) buckets balanced by
    in-degree, so every core/window sees ~equal edge counts and the
    SPMD-uniform batch schedule wastes almost no padding.
  - Weight matrices replicated; degree norms folded into per-edge weights
    w_e = src_norm[src] * dst_norm[dst] carried by the scatter one-hots.
  - Per layer: project own shard (fp16 matmuls, PSUM f32) -> PE-transpose to
    node-major -> AllGather (2 pipelined chunks) into a DRAM table
    [50176, 128] fp16 -> gather each edge's source row via indirect DMA
    (128 rows/instruction, edges grouped per dst window) -> scatter-add via
    matmul with an on-chip weighted one-hot (iota==dst_local)*w_e
    accumulating in PSUM per window.
  - Layer 3's scatter is fused with dgl.mean_nodes: its one-hot is the
    8-wide graph-membership matrix scaled by w_e/cnt_g, so the whole last
    aggregation lands directly in a [128,8] pooled accumulator; partials
    are AllReduced and the tiny MLP head runs replicated on every core.
"""

import heapq
import sys

sys.path.insert(0, "/opt/trn_rl_repo")

import numpy as np

import concourse.bass as bass
import concourse.mybir as mybir
import concourse.tile as tile
import bass_rust
from concourse.bass_utils import run_bass_kernel_spmd

F32 = mybir.dt.float32
F16 = mybir.dt.float16
I32 = mybir.dt.int32

NC = 8          # cores
D = 128         # feature dim (== partition width)
G = 8           # graphs
OUT = 10
NEG = 0.01      # LeakyReLU slope
WIN = 256       # dst window per PSUM tile

_split_ctr = [0]


def split_multiwaits(nc):
    """This walrus encodes at most ONE sync-wait per instruction; hoist
    extra waits into preceding EventSemaphore ops on the same engine."""
    for f in nc.m.functions:
        for blk in f.blocks:
            insts = list(blk.instructions)
            new, changed = [], False
            for inst in insts:
                si = inst.sync_info
                if si is not None and len(si.on_wait) > 1:
                    waits = list(si.on_wait)
                    for w in waits[:-1]:
                        _split_ctr[0] += 1
                        es = mybir.InstEventSemaphore(
                            name=f"mwsplit_{_split_ctr[0]}", ins=[], outs=[])
                        es.engine = inst.engine
                        es.sync_info = bass_rust.SyncInfo(on_wait=[w], on_update=[])
                        new.append(es)
                    si.on_wait = waits[-1:]
                    changed = True
                new.append(inst)
            if changed:
                blk.instructions = new


def _chunks(shard):
    """Up to 4 roughly-equal 128-aligned table chunks covering [0, shard)."""
    ntile = shard // 128
    n = min(4, ntile)
    base = ntile // n
    sizes = [base] * n
    for i in range(ntile - base * n):
        sizes[n - 1 - i] += 1
    out, c = [], 0
    for z in sizes:
        out.append((c, z * 128))
        c += z * 128
    return out


def _windows(shard):
    """List of (start, width) dst windows covering [0, shard)."""
    out = []
    c = 0
    while c < shard:
        w = min(WIN, shard - c)
        out.append((c, w))
        c += w
    return out


# ---------------------------------------------------------------- host prep

def _prep(x, src, dst, gid, n_nodes, shard):
    wins = _windows(shard)
    nwin = len(wins)
    chunks = _chunks(shard)

    out_deg = np.bincount(src, minlength=n_nodes)
    in_deg = np.bincount(dst, minlength=n_nodes)
    snorm = np.clip(out_deg, 1, None).astype(np.float32) ** -0.5
    dnorm = np.clip(in_deg, 1, None).astype(np.float32) ** -0.5
    we = snorm[src] * dnorm[dst]
    cnt = np.bincount(gid, minlength=G).astype(np.float32)
    cinv = (1.0 / np.clip(cnt, 1, None)).astype(np.float32)

    # --- balanced node -> (core, window) assignment by in-degree
    order = np.argsort(-in_deg, kind="stable")
    heap = [(0.0, k * nwin + w) for k in range(NC) for w in range(nwin)]
    heapq.heapify(heap)
    fill = [[0] * nwin for _ in range(NC)]
    node_core = np.empty(n_nodes, np.int32)
    node_slot = np.empty(n_nodes, np.int32)
    for n in order:
        while True:
            load, b = heapq.heappop(heap)
            k, w = divmod(b, nwin)
            if fill[k][w] < wins[w][1]:
                break
        node_core[n] = k
        node_slot[n] = wins[w][0] + fill[k][w]
        fill[k][w] += 1
        heapq.heappush(heap, (load + float(in_deg[n]), b))

    # global gather-table id (matches the chunked AllGather layout):
    # chunk c of every rank is contiguous: row = NC*S_c + k*Z_c + (s - S_c)
    s = node_slot.astype(np.int64)
    kk = node_core.astype(np.int64)
    gid_tab = np.zeros(len(s), np.int64)
    for (cst, csz) in chunks:
        m = (s >= cst) & (s < cst + csz)
        gid_tab[m] = NC * cst + kk[m] * csz + (s[m] - cst)
    gid_tab = gid_tab.astype(np.int32)

    e_core = node_core[dst]
    e_slot_all = node_slot[dst]

    wcnt = np.zeros((NC, nwin), np.int64)
    for k in range(NC):
        m = e_core == k
        wcnt[k] = np.bincount(e_slot_all[m] // WIN, minlength=nwin)
    bw = np.maximum(1, -(-wcnt.max(axis=0) // 128)).astype(np.int64)
    offs = np.concatenate([[0], np.cumsum(bw)])
    nbw = int(offs[-1])
    nb3 = max(1, max(-(-int((e_core == k).sum()) // 128) for k in range(NC)))

    per_core = []
    for k in range(NC):
        m = e_core == k
        e_src = src[m].astype(np.int64)
        e_slot = e_slot_all[m].astype(np.int64)
        e_w = we[m].astype(np.float32)
        e_g = gid[dst[m]].astype(np.int64)
        o = np.argsort(e_slot, kind="stable")
        e_src, e_slot, e_w, e_g = e_src[o], e_slot[o], e_w[o], e_g[o]

        gidx = np.zeros((128, nbw), np.int32)
        dstloc = np.full((128, nbw), -1.0, np.float32)
        wcol = np.zeros((128, nbw), np.float32)
        win = e_slot // WIN
        pos = np.zeros(len(e_src), np.int64)
        for w in range(nwin):
            mm = win == w
            pos[mm] = np.arange(mm.sum())
        col = offs[win] + pos // 128
        row = pos % 128
        gidx[row, col] = gid_tab[e_src]
        dstloc[row, col] = (e_slot - win * WIN).astype(np.float32)
        wcol[row, col] = e_w

        # layer-3 SpMM folds into mean-pool: per-node pooling weights
        # q[n, g] = sum_{e: src=n} w_e * cinv[g] * [graph(dst_e) == g]
        qk = np.zeros((shard, G), np.float32)
        loc = node_slot[e_src]          # unused; q is per SRC node below
        per_core.append(dict(gidx=gidx, dstloc=dstloc, wcol=wcol))
    # q over all nodes, then scatter into each core's slot order
    q_full = np.zeros((n_nodes, G), np.float32)
    np.add.at(q_full, (src, gid[dst]), we * cinv[gid[dst]])
    ntile = shard // 128
    for k in range(NC):
        mk = node_core == k
        qk = np.zeros((shard, G), np.float32)
        qk[node_slot[mk]] = q_full[mk]
        per_core[k]["q3"] = np.ascontiguousarray(
            qk.reshape(ntile, 128, G).transpose(1, 0, 2).reshape(128, ntile * G))
    return per_core, node_core, node_slot, bw, offs, nbw, nb3


# ---------------------------------------------------------------- program

def build_program(shard, nbw, nb3, bw, offs):
    npad = NC * shard
    wins = _windows(shard)
    nwin = len(wins)
    chunks = _chunks(shard)
    # window index after which each chunk's columns are fully evicted
    trig = []
    for (cst, csz) in chunks:
        w = 0
        while wins[w][0] + wins[w][1] < cst + csz:
            w += 1
        trig.append(w)

    nc = bass.Bass("TRN2", target_bir_lowering=False, debug=False,
                   num_devices=NC)

    xT = nc.dram_tensor("xT", [128, shard], F32, kind="ExternalInput")
    wg = [nc.dram_tensor(f"w{l}", [128, 128], F32, kind="ExternalInput")
          for l in (1, 2, 3)]
    bg = [nc.dram_tensor(f"b{l}", [128, 1], F32, kind="ExternalInput")
          for l in (1, 2, 3)]
    wc1 = nc.dram_tensor("wc1", [128, 512], F32, kind="ExternalInput")
    bc1 = nc.dram_tensor("bc1", [128, 4], F32, kind="ExternalInput")
    wc2 = nc.dram_tensor("wc2", [128, 4 * 256], F32, kind="ExternalInput")
    bc2 = nc.dram_tensor("bc2", [128, 2], F32, kind="ExternalInput")
    wc3 = nc.dram_tensor("wc3", [128, 2 * OUT], F32, kind="ExternalInput")
    bc3 = nc.dram_tensor("bc3", [128, 1], F32, kind="ExternalInput")
    gidx_in = nc.dram_tensor("gidx", [128, nbw], I32, kind="ExternalInput")
    dstloc_in = nc.dram_tensor("dstloc", [128, nbw], F32, kind="ExternalInput")
    wcol_in = nc.dram_tensor("wcol", [128, nbw], F32, kind="ExternalInput")
    ntile = shard // 128
    q3_in = nc.dram_tensor("q3", [128, ntile * G], F32, kind="ExternalInput")
    iota_in = nc.dram_tensor("iota", [128, WIN], F32, kind="ExternalInput")
    ident_in = nc.dram_tensor("ident", [128, 128], F32, kind="ExternalInput")
    out_d = nc.dram_tensor("out", [OUT, G], F32, kind="ExternalOutput")

    bounce = [nc.dram_tensor(f"bnc{l}", [shard, 128], F16) for l in range(2)]
    table = [nc.dram_tensor(f"tbl{l}", [npad, 128], F16) for l in range(2)]
    pool_in = nc.dram_tensor("pool_in", [128, G], F32)
    pool_out = nc.dram_tensor("pool_out", [128, G], F32)

    with tile.TileContext(nc) as tc:
        with tc.tile_pool(name="sb", bufs=1) as sb, \
             tc.tile_pool(name="ps", bufs=4, space="PSUM") as ps, \
             tc.tile_pool(name="ps2", bufs=2, space="PSUM") as ps2:

            # critical-path loads first: xT, W1, identity
            xT_t = sb.tile([128, shard], F32, name="xT_t")
            nc.sync.dma_start(out=xT_t[:], in_=xT[:])
            ident_t = sb.tile([128, 128], F32, name="ident_t")
            nc.sync.dma_start(out=ident_t[:], in_=ident_in[:])
            w_t, b_t = [], []
            for l in range(3):
                wf = sb.tile([128, 128], F32, name=f"wf{l}")
                nc.sync.dma_start(out=wf[:], in_=wg[l][:])
                wh = sb.tile([128, 128], F16, name=f"wh{l}")
                nc.vector.tensor_copy(out=wh[:], in_=wf[:])
                w_t.append(wh)
                bt = sb.tile([128, 1], F32, name=f"bt{l}")
                nc.sync.dma_start(out=bt[:], in_=bg[l][:])
                b_t.append(bt)
            state = sb.tile([128, shard], F16, name="state0")
            nc.vector.tensor_copy(out=state[:], in_=xT_t[:])

            iota_t = sb.tile([128, WIN], F32, name="iota_t")
            nc.sync.dma_start(out=iota_t[:], in_=iota_in[:])
            gidx_t = sb.tile([128, nbw], I32, name="gidx_t")
            nc.sync.dma_start(out=gidx_t[:], in_=gidx_in[:])
            dstloc_t = sb.tile([128, nbw], F32, name="dstloc_t")
            nc.sync.dma_start(out=dstloc_t[:], in_=dstloc_in[:])
            wcol_t = sb.tile([128, nbw], F32, name="wcol_t")
            nc.sync.dma_start(out=wcol_t[:], in_=wcol_in[:])
            q3_f32 = sb.tile([128, ntile * G], F32, name="q3_f32")
            nc.sync.dma_start(out=q3_f32[:], in_=q3_in[:])
            q3_t = sb.tile([128, ntile * G], F16, name="q3_t")
            nc.vector.tensor_copy(out=q3_t[:], in_=q3_f32[:])

            def emit_chunk(l, state_t, cst, csz):
                """Project chunk [cst, cst+csz) of layer l's state, transpose
                to node-major, bounce to DRAM, AllGather into table[l]."""
                xp = sb.tile([128, csz], F32, name=f"xp{l}_{cst}", tag="xpf")
                c = 0
                while c < csz:
                    w = min(512, csz - c)
                    pt = ps2.tile([128, w], F32, space="PSUM",
                                  name=f"pj{l}_{cst}_{c}", tag="c")
                    nc.tensor.matmul(out=pt[:], lhsT=w_t[l][:],
                                     rhs=state_t[:, cst + c:cst + c + w],
                                     start=True, stop=True)
                    nc.vector.tensor_copy(out=xp[:, c:c + w], in_=pt[:])
                    c += w
                nck = csz // 128
                xpnm = sb.tile([128, nck, 128], F16,
                               name=f"xpnm{l}_{cst}", tag="xpnm")
                for t in range(nck):
                    tp = ps2.tile([128, 128], F32, space="PSUM",
                                  name=f"tp{l}_{cst}_{t}", tag="c")
                    nc.tensor.transpose(out=tp[:],
                                        in_=xp[:, 128 * t:128 * (t + 1)],
                                        identity=ident_t[:])
                    nc.vector.tensor_copy(out=xpnm[:, t, :], in_=tp[:])
                nc.sync.dma_start(
                    out=bounce[l][cst:cst + csz, :]
                        .rearrange("(c p) f -> p c f", p=128),
                    in_=xpnm[:])
                tb0 = NC * cst

                def fire(l=l, cst=cst, csz=csz, tb0=tb0):
                    nc.gpsimd.collective_compute(
                        "AllGather", mybir.AluOpType.bypass,
                        replica_groups=[list(range(NC))],
                        ins=[bounce[l][cst:cst + csz, :]],
                        outs=[table[l][tb0:tb0 + NC * csz, :]])
                return fire

            # gathers land in per-window grouped tiles so the WAR wait is
            # per-window (4-slot rotation), not per-gather
            def gather_group(l, cols, src_idx_t, gname, gtag):
                mt = sb.tile([128, len(cols) * 128], F16, name=gname,
                             tag=gtag)
                views = []
                for i, col in enumerate(cols):
                    v = mt[:, 128 * i:128 * (i + 1)]
                    nc.gpsimd.indirect_dma_start(
                        out=v, out_offset=None, in_=table[l][:],
                        in_offset=bass.IndirectOffsetOnAxis(
                            ap=src_idx_t[:, col:col + 1], axis=0))
                    views.append(v)
                return views

            # layer-3 x W3 projection folds into pooling: per chunk,
            # project+transpose state2 and contract with q3 into pp
            pp = ps2.tile([128, G], F32, space="PSUM", name="pp", tag="b")
            _pcnt = [0]

            def emit_pool_chunk(state_t, cst, csz):
                xp = sb.tile([128, csz], F32, name=f"xpq_{cst}", tag="xpf")
                c = 0
                while c < csz:
                    w = min(512, csz - c)
                    pt = ps2.tile([128, w], F32, space="PSUM",
                                  name=f"pjq_{cst}_{c}", tag="c")
                    nc.tensor.matmul(out=pt[:], lhsT=w_t[2][:],
                                     rhs=state_t[:, cst + c:cst + c + w],
                                     start=True, stop=True)
                    nc.vector.tensor_copy(out=xp[:, c:c + w], in_=pt[:])
                    c += w
                nck = csz // 128
                xpnm = sb.tile([128, nck, 128], F16,
                               name=f"xpnmq_{cst}", tag="xpnm")
                for t in range(nck):
                    tp = ps2.tile([128, 128], F32, space="PSUM",
                                  name=f"tpq_{cst}_{t}", tag="c")
                    nc.tensor.transpose(out=tp[:],
                                        in_=xp[:, 128 * t:128 * (t + 1)],
                                        identity=ident_t[:])
                    nc.vector.tensor_copy(out=xpnm[:, t, :], in_=tp[:])
                    gt = cst // 128 + t
                    nc.tensor.matmul(out=pp[:], lhsT=xpnm[:, t, :],
                                     rhs=q3_t[:, G * gt:G * (gt + 1)],
                                     start=(_pcnt[0] == 0),
                                     stop=(_pcnt[0] == ntile - 1))
                    _pcnt[0] += 1

            # layer 0's table comes straight from the input state
            for (cst, csz) in chunks:
                emit_chunk(0, state, cst, csz)()

            for l in range(2):
                nstate = sb.tile([128, shard], F16, name=f"state{l + 1}")
                pending = []
                for w in range(nwin):
                    wst, ww = wins[w]
                    pw = ps.tile([128, ww], F32, space="PSUM",
                                 name=f"pw{l}_{w}", tag="a")
                    nb = int(bw[w])
                    cols = [int(offs[w]) + j for j in range(nb)]
                    mviews = gather_group(l, cols, gidx_t,
                                          f"mg{l}_{w}", f"gw{w % 4}")
                    for j, col in enumerate(cols):
                        oh = sb.tile([128, ww], F16, name=f"oh{l}_{col}",
                                     tag=f"oh{col % 16}")
                        nc.vector.tensor_scalar(
                            out=oh[:], in0=iota_t[:, :ww],
                            scalar1=dstloc_t[:, col:col + 1],
                            scalar2=wcol_t[:, col:col + 1],
                            op0=mybir.AluOpType.is_equal,
                            op1=mybir.AluOpType.mult)
                        nc.tensor.matmul(out=pw[:], lhsT=mviews[j], rhs=oh[:],
                                         start=(j == 0), stop=(j == nb - 1))
                    nc.vector.tensor_scalar(
                        out=nstate[:, wst:wst + ww], in0=pw[:],
                        scalar1=b_t[l][:, 0:1], scalar2=None,
                        op0=mybir.AluOpType.add)
                    # fire the previous window's deferred collective now
                    # that this window's gathers are queued ahead of it
                    for f in pending:
                        f()
                    pending = []
                    # emit the next stage's work as soon as the columns
                    # are final -- overlaps collectives/PE with gathers
                    for ci, (cst, csz) in enumerate(chunks):
                        if trig[ci] == w:
                            if l == 0:
                                pending.append(
                                    emit_chunk(1, nstate, cst, csz))
                            else:
                                emit_pool_chunk(nstate, cst, csz)
                for f in pending:
                    f()
                state = nstate

            # ---------- pooled accumulator already filled by emit_pool_chunk
            ppool = sb.tile([128, G], F32, name="ppool")
            nc.vector.tensor_copy(out=ppool[:], in_=pp[:])
            nc.sync.dma_start(out=pool_in[:], in_=ppool[:])
            nc.gpsimd.collective_compute(
                "AllReduce", mybir.AluOpType.add,
                replica_groups=[list(range(NC))],
                ins=[pool_in.ap()], outs=[pool_out.ap()])
            pooled_f = sb.tile([128, G], F32, name="pooled_f")
            nc.sync.dma_start(out=pooled_f[:], in_=pool_out[:])
            nc.vector.tensor_scalar(out=pooled_f[:], in0=pooled_f[:],
                                    scalar1=b_t[2][:, 0:1], scalar2=None,
                                    op0=mybir.AluOpType.add)
            pooled = sb.tile([128, G], F16, name="pooled")
            nc.vector.tensor_copy(out=pooled[:], in_=pooled_f[:])

            # ---------- MLP head (replicated)
            _hctr = [0]

            def lrelu_evict(psrc, bias_tile, bias_c, dst16, nrows=128):
                _hctr[0] += 1
                t1 = sb.tile([128, G], F32, name=f"t1_{_hctr[0]}", tag="h1")
                nc.vector.tensor_scalar(out=t1[:nrows], in0=psrc[:nrows],
                                        scalar1=bias_tile[:nrows,
                                                          bias_c:bias_c + 1],
                                        scalar2=None,
                                        op0=mybir.AluOpType.add)
                t2 = sb.tile([128, G], F32, name=f"t2_{_hctr[0]}", tag="h2")
                nc.vector.tensor_scalar(out=t2[:nrows], in0=t1[:nrows],
                                        scalar1=NEG, scalar2=None,
                                        op0=mybir.AluOpType.mult)
                nc.vector.tensor_tensor(out=dst16[:nrows], in0=t1[:nrows],
                                        in1=t2[:nrows],
                                        op=mybir.AluOpType.max)

            wc1_f = sb.tile([128, 512], F32, name="wc1_f")
            nc.sync.dma_start(out=wc1_f[:], in_=wc1[:])
            wc1_h = sb.tile([128, 512], F16, name="wc1_h")
            nc.vector.tensor_copy(out=wc1_h[:], in_=wc1_f[:])
            bc1_t = sb.tile([128, 4], F32, name="bc1_t")
            nc.sync.dma_start(out=bc1_t[:], in_=bc1[:])
            wc2_f = sb.tile([128, 4 * 256], F32, name="wc2_f")
            nc.sync.dma_start(out=wc2_f[:], in_=wc2[:])
            wc2_h = sb.tile([128, 4 * 256], F16, name="wc2_h")
            nc.vector.tensor_copy(out=wc2_h[:], in_=wc2_f[:])
            bc2_t = sb.tile([128, 2], F32, name="bc2_t")
            nc.sync.dma_start(out=bc2_t[:], in_=bc2[:])
            wc3_f = sb.tile([128, 2 * OUT], F32, name="wc3_f")
            nc.sync.dma_start(out=wc3_f[:], in_=wc3[:])
            wc3_h = sb.tile([128, 2 * OUT], F16, name="wc3_h")
            nc.vector.tensor_copy(out=wc3_h[:], in_=wc3_f[:])
            bc3_t = sb.tile([128, 1], F32, name="bc3_t")
            nc.sync.dma_start(out=bc3_t[:], in_=bc3[:])

            z1 = sb.tile([128, 4 * G], F16, name="z1")
            for c in range(4):
                ph = ps2.tile([128, G], F32, space="PSUM",
                              name=f"ph1_{c}", tag="b")
                nc.tensor.matmul(out=ph[:], lhsT=wc1_h[:, 128 * c:128 * (c + 1)],
                                 rhs=pooled[:], start=True, stop=True)
                lrelu_evict(ph, bc1_t, c, z1[:, G * c:G * (c + 1)])
            z2 = sb.tile([128, 2 * G], F16, name="z2")
            for jj in range(2):
                ph = ps2.tile([128, G], F32, space="PSUM",
                              name=f"ph2_{jj}", tag="b")
                for c in range(4):
                    nc.tensor.matmul(
                        out=ph[:],
                        lhsT=wc2_h[:, 256 * c + 128 * jj:256 * c + 128 * (jj + 1)],
                        rhs=z1[:, G * c:G * (c + 1)],
                        start=(c == 0), stop=(c == 3))
                lrelu_evict(ph, bc2_t, jj, z2[:, G * jj:G * (jj + 1)])
            ph3 = ps2.tile([128, G], F32, space="PSUM", name="ph3", tag="b")
            for c in range(2):
                nc.tensor.matmul(out=ph3[:OUT],
                                 lhsT=wc3_h[:, OUT * c:OUT * (c + 1)],
                                 rhs=z2[:, G * c:G * (c + 1)],
                                 start=(c == 0), stop=(c == 1))
            zout = sb.tile([128, G], F32, name="zout")
            lrelu_evict(ph3, bc3_t, 0, zout, nrows=OUT)
            nc.sync.dma_start(out=out_d[:], in_=zout[:OUT])

    split_multiwaits(nc)
    return nc


# ---------------------------------------------------------------- driver

def run_gcn(x, src, dst, gid, W1, b1, W2, b2, W3, b3,
            Wc1, bc1, Wc2, bc2, Wc3, bc3, n_nodes, shard,
            trace=False, tmpdir=None):
    x = np.asarray(x, np.float32)
    src = np.asarray(src).astype(np.int64)
    dst = np.asarray(dst).astype(np.int64)
    gid = np.asarray(gid).astype(np.int64)

    per_core, node_core, node_slot, bw, offs, nbw, nb3 = _prep(
        x, src, dst, gid, n_nodes, shard)
    nc = build_program(shard, nbw, nb3, bw, offs)

    iota = np.tile(np.arange(WIN, dtype=np.float32), (128, 1))
    ident = np.eye(128, dtype=np.float32)

    def colvec(v, n=128):
        a = np.zeros((n, 1), np.float32)
        a[:len(v), 0] = v
        return a

    bc1_a = np.asarray(bc1, np.float32).reshape(4, 128).T.copy()
    bc2_a = np.asarray(bc2, np.float32).reshape(2, 128).T.copy()
    wc2_a = np.concatenate(
        [np.asarray(Wc2, np.float32)[128 * c:128 * (c + 1), :] for c in range(4)],
        axis=1)
    wc3_a = np.concatenate(
        [np.asarray(Wc3, np.float32)[128 * c:128 * (c + 1), :] for c in range(2)],
        axis=1)

    in_maps = []
    for k in range(NC):
        pc = per_core[k]
        xk = np.zeros((shard, D), np.float32)
        mk = node_core == k
        xk[node_slot[mk]] = x[mk]
        in_maps.append({
            "xT": np.ascontiguousarray(xk.T),
            "w1": np.asarray(W1, np.float32), "b1": colvec(np.asarray(b1)),
            "w2": np.asarray(W2, np.float32), "b2": colvec(np.asarray(b2)),
            "w3": np.asarray(W3, np.float32), "b3": colvec(np.asarray(b3)),
            "wc1": np.asarray(Wc1, np.float32), "bc1": bc1_a,
            "wc2": wc2_a, "bc2": bc2_a,
            "wc3": wc3_a, "bc3": colvec(np.asarray(bc3)),
            "gidx": pc["gidx"], "dstloc": pc["dstloc"], "wcol": pc["wcol"],
            "q3": pc["q3"],
            "iota": iota, "ident": ident,
        })

    res = run_bass_kernel_spmd(nc, in_maps, core_ids=list(range(NC)),
                               trace=trace, tmpdir=tmpdir)
    out = res.results[0]["out"].T.astype(np.float32).copy()  # [G, OUT]
    return out, res


def kernel(**inputs):
    out, _ = run_gcn(
        inputs["inputs"], inputs["src"], inputs["dst"], inputs["graph_id"],
        inputs["W1"], inputs["b1"], inputs["W2"], inputs["b2"],
        inputs["W3"], inputs["b3"],
        inputs["Wc1"], inputs["bc1"], inputs["Wc2"], inputs["bc2"],
        inputs["Wc3"], inputs["bc3"],
        n_nodes=50000, shard=6272)
    return out


# revision 13
# speedup vs baseline: 1.5970x; 1.0145x over previous
"""GCN (3x GraphConv + mean-pool + MLP head) on 8 Trainium2 NeuronCores.

Strategy (SPMD, one program on all 8 cores):
  - Nodes are assigned to (core, 256-dst-window) buckets balanced by
    in-degree, so every core/window sees ~equal edge counts and the
    SPMD-uniform batch schedule wastes almost no padding.
  - Weight matrices replicated; degree norms folded into per-edge weights
    w_e = src_norm[src] * dst_norm[dst] carried by the scatter one-hots.
  - Per layer: project own shard (fp16 matmuls, PSUM f32) -> PE-transpose to
    node-major -> AllGather (2 pipelined chunks) into a DRAM table
    [50176, 128] fp16 -> gather each edge's source row via indirect DMA
    (128 rows/instruction, edges grouped per dst window) -> scatter-add via
    matmul with an on-chip weighted one-hot (iota==dst_local)*w_e
    accumulating in PSUM per window.
  - Layer 3's scatter is fused with dgl.mean_nodes: its one-hot is the
    8-wide graph-membership matrix scaled by w_e/cnt_g, so the whole last
    aggregation lands directly in a [128,8] pooled accumulator; partials
    are AllReduced and the tiny MLP head runs replicated on every core.
"""

import heapq
import sys

sys.path.insert(0, "/opt/trn_rl_repo")

import numpy as np

import concourse.bass as bass
import concourse.mybir as mybir
import concourse.tile as tile
import bass_rust
from concourse.bass_utils import run_bass_kernel_spmd

F32 = mybir.dt.float32
F16 = mybir.dt.float16
I32 = mybir.dt.int32

NC = 8          # cores
D = 128         # feature dim (== partition width)
G = 8           # graphs
OUT = 10
NEG = 0.01      # LeakyReLU slope
WIN = 256       # dst window per PSUM tile

_split_ctr = [0]


def split_multiwaits(nc):
    """This walrus encodes at most ONE sync-wait per instruction; hoist
    extra waits into preceding EventSemaphore ops on the same engine."""
    for f in nc.m.functions:
        for blk in f.blocks:
            insts = list(blk.instructions)
            new, changed = [], False
            for inst in insts:
                si = inst.sync_info
                if si is not None and len(si.on_wait) > 1:
                    waits = list(si.on_wait)
                    for w in waits[:-1]:
                        _split_ctr[0] += 1
                        es = mybir.InstEventSemaphore(
                            name=f"mwsplit_{_split_ctr[0]}", ins=[], outs=[])
                        es.engine = inst.engine
                        es.sync_info = bass_rust.SyncInfo(on_wait=[w], on_update=[])
                        new.append(es)
                    si.on_wait = waits[-1:]
                    changed = True
                new.append(inst)
            if changed:
                blk.instructions = new


def _chunks(shard):
    """Up to 4 roughly-equal 128-aligned table chunks covering [0, shard)."""
    ntile = shard // 128
    n = min(4, ntile)
    base = ntile // n
    sizes = [base] * n
    for i in range(ntile - base * n):
        sizes[n - 1 - i] += 1
    out, c = [], 0
    for z in sizes:
        out.append((c, z * 128))
        c += z * 128
    return out


def _windows(shard):
    """List of (start, width) dst windows covering [0, shard)."""
    out = []
    c = 0
    while c < shard:
        w = min(WIN, shard - c)
        out.append((c, w))
        c += w
    return out


# ---------------------------------------------------------------- host prep

def _prep(x, src, dst, gid, n_nodes, shard):
    wins = _windows(shard)
    nwin = len(wins)
    chunks = _chunks(shard)

    out_deg = np.bincount(src, minlength=n_nodes)
    in_deg = np.bincount(dst, minlength=n_nodes)
    snorm = np.clip(out_deg, 1, None).astype(np.float32) ** -0.5
    dnorm = np.clip(in_deg, 1, None).astype(np.float32) ** -0.5
    we = snorm[src] * dnorm[dst]
    cnt = np.bincount(gid, minlength=G).astype(np.float32)
    cinv = (1.0 / np.clip(cnt, 1, None)).astype(np.float32)

    # --- balanced node -> (core, window) assignment by in-degree
    order = np.argsort(-in_deg, kind="stable")
    heap = [(0.0, k * nwin + w) for k in range(NC) for w in range(nwin)]
    heapq.heapify(heap)
    fill = [[0] * nwin for _ in range(NC)]
    node_core = np.empty(n_nodes, np.int32)
    node_slot = np.empty(n_nodes, np.int32)
    for n in order:
        while True:
            load, b = heapq.heappop(heap)
            k, w = divmod(b, nwin)
            if fill[k][w] < wins[w][1]:
                break
        node_core[n] = k
        node_slot[n] = wins[w][0] + fill[k][w]
        fill[k][w] += 1
        heapq.heappush(heap, (load + float(in_deg[n]), b))

    # global gather-table id (matches the chunked AllGather layout):
    # chunk c of every rank is contiguous: row = NC*S_c + k*Z_c + (s - S_c)
    s = node_slot.astype(np.int64)
    kk = node_core.astype(np.int64)
    gid_tab = np.zeros(len(s), np.int64)
    for (cst, csz) in chunks:
        m = (s >= cst) & (s < cst + csz)
        gid_tab[m] = NC * cst + kk[m] * csz + (s[m] - cst)
    gid_tab = gid_tab.astype(np.int32)

    e_core = node_core[dst]
    e_slot_all = node_slot[dst]

    wcnt = np.zeros((NC, nwin), np.int64)
    for k in range(NC):
        m = e_core == k
        wcnt[k] = np.bincount(e_slot_all[m] // WIN, minlength=nwin)
    bw = np.maximum(1, -(-wcnt.max(axis=0) // 128)).astype(np.int64)
    offs = np.concatenate([[0], np.cumsum(bw)])
    nbw = int(offs[-1])
    nb3 = max(1, max(-(-int((e_core == k).sum()) // 128) for k in range(NC)))

    per_core = []
    for k in range(NC):
        m = e_core == k
        e_src = src[m].astype(np.int64)
        e_slot = e_slot_all[m].astype(np.int64)
        e_w = we[m].astype(np.float32)
        e_g = gid[dst[m]].astype(np.int64)
        o = np.argsort(e_slot, kind="stable")
        e_src, e_slot, e_w, e_g = e_src[o], e_slot[o], e_w[o], e_g[o]

        gidx = np.zeros((128, nbw), np.int32)
        dstloc = np.full((128, nbw), -1.0, np.float32)
        wcol = np.zeros((128, nbw), np.float32)
        win = e_slot // WIN
        pos = np.zeros(len(e_src), np.int64)
        for w in range(nwin):
            mm = win == w
            pos[mm] = np.arange(mm.sum())
        col = offs[win] + pos // 128
        row = pos % 128
        gidx[row, col] = gid_tab[e_src]
        dstloc[row, col] = (e_slot - win * WIN).astype(np.float32)
        wcol[row, col] = e_w

        # layer-3 SpMM folds into mean-pool: per-node pooling weights
        # q[n, g] = sum_{e: src=n} w_e * cinv[g] * [graph(dst_e) == g]
        qk = np.zeros((shard, G), np.float32)
        loc = node_slot[e_src]          # unused; q is per SRC node below
        per_core.append(dict(gidx=gidx, dstloc=dstloc, wcol=wcol))
    # q over all nodes, then scatter into each core's slot order
    q_full = np.zeros((n_nodes, G), np.float32)
    np.add.at(q_full, (src, gid[dst]), we * cinv[gid[dst]])
    ntile = shard // 128
    x_nm = np.zeros((NC * shard, D), np.float32)
    x_nm[gid_tab] = x
    for k in range(NC):
        mk = node_core == k
        qk = np.zeros((shard, G), np.float32)
        qk[node_slot[mk]] = q_full[mk]
        per_core[k]["q3"] = np.ascontiguousarray(
            qk.reshape(ntile, 128, G).transpose(1, 0, 2).reshape(128, ntile * G))
    return per_core, node_core, node_slot, bw, offs, nbw, nb3, x_nm


# ---------------------------------------------------------------- program

def build_program(shard, nbw, nb3, bw, offs):
    npad = NC * shard
    wins = _windows(shard)
    nwin = len(wins)
    chunks = _chunks(shard)
    # window index after which each chunk's columns are fully evicted
    trig = []
    for (cst, csz) in chunks:
        w = 0
        while wins[w][0] + wins[w][1] < cst + csz:
            w += 1
        trig.append(w)

    nc = bass.Bass("TRN2", target_bir_lowering=False, debug=False,
                   num_devices=NC)

    x_nm = nc.dram_tensor("x_nm", [npad, D], F32, kind="ExternalInput")
    wg = [nc.dram_tensor(f"w{l}", [128, 128], F32, kind="ExternalInput")
          for l in (1, 2, 3)]
    bg = [nc.dram_tensor(f"b{l}", [128, 1], F32, kind="ExternalInput")
          for l in (1, 2, 3)]
    wc1 = nc.dram_tensor("wc1", [128, 512], F32, kind="ExternalInput")
    bc1 = nc.dram_tensor("bc1", [128, 4], F32, kind="ExternalInput")
    wc2 = nc.dram_tensor("wc2", [128, 4 * 256], F32, kind="ExternalInput")
    bc2 = nc.dram_tensor("bc2", [128, 2], F32, kind="ExternalInput")
    wc3 = nc.dram_tensor("wc3", [128, 2 * OUT], F32, kind="ExternalInput")
    bc3 = nc.dram_tensor("bc3", [128, 1], F32, kind="ExternalInput")
    gidx_in = nc.dram_tensor("gidx", [128, nbw], I32, kind="ExternalInput")
    dstloc_in = nc.dram_tensor("dstloc", [128, nbw], F32, kind="ExternalInput")
    wcol_in = nc.dram_tensor("wcol", [128, nbw], F32, kind="ExternalInput")
    ntile = shard // 128
    q3_in = nc.dram_tensor("q3", [128, ntile * G], F32, kind="ExternalInput")
    iota_in = nc.dram_tensor("iota", [128, WIN], F32, kind="ExternalInput")
    ident_in = nc.dram_tensor("ident", [128, 128], F32, kind="ExternalInput")
    out_d = nc.dram_tensor("out", [OUT, G], F32, kind="ExternalOutput")

    bounce = {1: nc.dram_tensor("bnc1", [shard, 128], F16)}
    table = {1: nc.dram_tensor("tbl1", [npad, 128], F16)}
    pool_in = nc.dram_tensor("pool_in", [128, G], F32)
    pool_out = nc.dram_tensor("pool_out", [128, G], F32)

    with tile.TileContext(nc) as tc:
        with tc.tile_pool(name="sb", bufs=1) as sb, \
             tc.tile_pool(name="ps", bufs=4, space="PSUM") as ps, \
             tc.tile_pool(name="ps2", bufs=2, space="PSUM") as ps2:

            # critical-path loads first: gather schedule + weights
            ident_t = sb.tile([128, 128], F32, name="ident_t")
            nc.sync.dma_start(out=ident_t[:], in_=ident_in[:])
            w_t, b_t = [], []
            for l in range(3):
                wf = sb.tile([128, 128], F32, name=f"wf{l}")
                nc.sync.dma_start(out=wf[:], in_=wg[l][:])
                wh = sb.tile([128, 128], F16, name=f"wh{l}")
                nc.vector.tensor_copy(out=wh[:], in_=wf[:])
                w_t.append(wh)
                bt = sb.tile([128, 1], F32, name=f"bt{l}")
                nc.sync.dma_start(out=bt[:], in_=bg[l][:])
                b_t.append(bt)
            iota_t = sb.tile([128, WIN], F32, name="iota_t")
            nc.sync.dma_start(out=iota_t[:], in_=iota_in[:])
            gidx_t = sb.tile([128, nbw], I32, name="gidx_t")
            nc.sync.dma_start(out=gidx_t[:], in_=gidx_in[:])
            dstloc_t = sb.tile([128, nbw], F32, name="dstloc_t")
            nc.sync.dma_start(out=dstloc_t[:], in_=dstloc_in[:])
            wcol_t = sb.tile([128, nbw], F32, name="wcol_t")
            nc.sync.dma_start(out=wcol_t[:], in_=wcol_in[:])
            q3_f32 = sb.tile([128, ntile * G], F32, name="q3_f32")
            nc.sync.dma_start(out=q3_f32[:], in_=q3_in[:])
            q3_t = sb.tile([128, ntile * G], F16, name="q3_t")
            nc.vector.tensor_copy(out=q3_t[:], in_=q3_f32[:])

            def emit_chunk(l, state_t, cst, csz):
                """Project chunk [cst, cst+csz) of layer l's state, transpose
                to node-major, bounce to DRAM, AllGather into table[l]."""
                xp = sb.tile([128, csz], F32, name=f"xp{l}_{cst}", tag="xpf")
                c = 0
                while c < csz:
                    w = min(512, csz - c)
                    pt = ps2.tile([128, w], F32, space="PSUM",
                                  name=f"pj{l}_{cst}_{c}", tag="c")
                    nc.tensor.matmul(out=pt[:], lhsT=w_t[l][:],
                                     rhs=state_t[:, cst + c:cst + c + w],
                                     start=True, stop=True)
                    nc.vector.tensor_copy(out=xp[:, c:c + w], in_=pt[:])
                    c += w
                nck = csz // 128
                xpnm = sb.tile([128, nck, 128], F16,
                               name=f"xpnm{l}_{cst}", tag="xpnm")
                for t in range(nck):
                    tp = ps2.tile([128, 128], F32, space="PSUM",
                                  name=f"tp{l}_{cst}_{t}", tag="c")
                    nc.tensor.transpose(out=tp[:],
                                        in_=xp[:, 128 * t:128 * (t + 1)],
                                        identity=ident_t[:])
                    nc.vector.tensor_copy(out=xpnm[:, t, :], in_=tp[:])
                nc.sync.dma_start(
                    out=bounce[l][cst:cst + csz, :]
                        .rearrange("(c p) f -> p c f", p=128),
                    in_=xpnm[:])
                tb0 = NC * cst

                def fire(l=l, cst=cst, csz=csz, tb0=tb0):
                    nc.gpsimd.collective_compute(
                        "AllGather", mybir.AluOpType.bypass,
                        replica_groups=[list(range(NC))],
                        ins=[bounce[l][cst:cst + csz, :]],
                        outs=[table[l][tb0:tb0 + NC * csz, :]])
                return fire

            # gathers land in per-window grouped tiles so the WAR wait is
            # per-window (4-slot rotation), not per-gather
            def gather_group(l, cols, src_idx_t, gname, gtag):
                srct = x_nm if l == 0 else table[1]
                dt = F32 if l == 0 else F16
                mt = sb.tile([128, len(cols) * 128], dt, name=gname,
                             tag=gtag)
                views = []
                for i, col in enumerate(cols):
                    v = mt[:, 128 * i:128 * (i + 1)]
                    nc.gpsimd.indirect_dma_start(
                        out=v, out_offset=None, in_=srct[:],
                        in_offset=bass.IndirectOffsetOnAxis(
                            ap=src_idx_t[:, col:col + 1], axis=0))
                    views.append(v)
                return views

            # layer-3 x W3 projection folds into pooling: per chunk,
            # project+transpose state2 and contract with q3 into pp
            pp = ps2.tile([128, G], F32, space="PSUM", name="pp", tag="b")
            _pcnt = [0]

            def emit_pool_chunk(state_t, cst, csz):
                xp = sb.tile([128, csz], F32, name=f"xpq_{cst}", tag="xpf")
                c = 0
                while c < csz:
                    w = min(512, csz - c)
                    pt = ps2.tile([128, w], F32, space="PSUM",
                                  name=f"pjq_{cst}_{c}", tag="c")
                    nc.tensor.matmul(out=pt[:], lhsT=w_t[2][:],
                                     rhs=state_t[:, cst + c:cst + c + w],
                                     start=True, stop=True)
                    nc.vector.tensor_copy(out=xp[:, c:c + w], in_=pt[:])
                    c += w
                nck = csz // 128
                xpnm = sb.tile([128, nck, 128], F16,
                               name=f"xpnmq_{cst}", tag="xpnm")
                for t in range(nck):
                    tp = ps2.tile([128, 128], F32, space="PSUM",
                                  name=f"tpq_{cst}_{t}", tag="c")
                    nc.tensor.transpose(out=tp[:],
                                        in_=xp[:, 128 * t:128 * (t + 1)],
                                        identity=ident_t[:])
                    nc.vector.tensor_copy(out=xpnm[:, t, :], in_=tp[:])
                    gt = cst // 128 + t
                    nc.tensor.matmul(out=pp[:], lhsT=xpnm[:, t, :],
                                     rhs=q3_t[:, G * gt:G * (gt + 1)],
                                     start=(_pcnt[0] == 0),
                                     stop=(_pcnt[0] == ntile - 1))
                    _pcnt[0] += 1

            # layer 0's table comes straight from the input state

            for l in range(2):
                nstate = sb.tile([128, shard], F16, name=f"state{l + 1}")
                pending = []
                for w in range(nwin):
                    wst, ww = wins[w]
                    pw = ps.tile([128, ww], F32, space="PSUM",
                                 name=f"pw{l}_{w}", tag="a")
                    nb = int(bw[w])
                    cols = [int(offs[w]) + j for j in range(nb)]
                    mviews = gather_group(l, cols, gidx_t,
                                          f"mg{l}_{w}", f"gw{w % 4}")
                    for j, col in enumerate(cols):
                        if l == 0:
                            mh = sb.tile([128, 128], F16, name=f"mh_{col}",
                                         tag=f"mh{col % 16}")
                            nc.vector.tensor_copy(out=mh[:], in_=mviews[j])
                            lhs = mh[:]
                        else:
                            lhs = mviews[j]
                        oh = sb.tile([128, ww], F16, name=f"oh{l}_{col}",
                                     tag=f"oh{col % 16}")
                        nc.vector.tensor_scalar(
                            out=oh[:], in0=iota_t[:, :ww],
                            scalar1=dstloc_t[:, col:col + 1],
                            scalar2=wcol_t[:, col:col + 1],
                            op0=mybir.AluOpType.is_equal,
                            op1=mybir.AluOpType.mult)
                        nc.tensor.matmul(out=pw[:], lhsT=lhs, rhs=oh[:],
                                         start=(j == 0), stop=(j == nb - 1))
                    if l == 0:
                        # agg is over raw x; apply the commuted W1 here
                        aggs = sb.tile([128, ww], F16, name=f"aggs_{w}",
                                       tag=f"ag{w % 4}")
                        nc.vector.tensor_copy(out=aggs[:], in_=pw[:])
                        ph = ps2.tile([128, ww], F32, space="PSUM",
                                      name=f"phw_{w}", tag="c")
                        nc.tensor.matmul(out=ph[:], lhsT=w_t[0][:],
                                         rhs=aggs[:], start=True, stop=True)
                        nc.vector.tensor_scalar(
                            out=nstate[:, wst:wst + ww], in0=ph[:],
                            scalar1=b_t[l][:, 0:1], scalar2=None,
                            op0=mybir.AluOpType.add)
                    else:
                        nc.vector.tensor_scalar(
                            out=nstate[:, wst:wst + ww], in0=pw[:],
                            scalar1=b_t[l][:, 0:1], scalar2=None,
                            op0=mybir.AluOpType.add)
                    # fire the previous window's deferred collective now
                    # that this window's gathers are queued ahead of it
                    for f in pending:
                        f()
                    pending = []
                    # emit the next stage's work as soon as the columns
                    # are final -- overlaps collectives/PE with gathers
                    for ci, (cst, csz) in enumerate(chunks):
                        if trig[ci] == w:
                            if l == 0:
                                pending.append(
                                    emit_chunk(1, nstate, cst, csz))
                            else:
                                emit_pool_chunk(nstate, cst, csz)
                for f in pending:
                    f()
                state = nstate

            # ---------- pooled accumulator already filled by emit_pool_chunk
            ppool = sb.tile([128, G], F32, name="ppool")
            nc.vector.tensor_copy(out=ppool[:], in_=pp[:])
            nc.sync.dma_start(out=pool_in[:], in_=ppool[:])
            nc.gpsimd.collective_compute(
                "AllReduce", mybir.AluOpType.add,
                replica_groups=[list(range(NC))],
                ins=[pool_in.ap()], outs=[pool_out.ap()])
            pooled_f = sb.tile([128, G], F32, name="pooled_f")
            nc.sync.dma_start(out=pooled_f[:], in_=pool_out[:])
            nc.vector.tensor_scalar(out=pooled_f[:], in0=pooled_f[:],
                                    scalar1=b_t[2][:, 0:1], scalar2=None,
                                    op0=mybir.AluOpType.add)
            pooled = sb.tile([128, G], F16, name="pooled")
            nc.vector.tensor_copy(out=pooled[:], in_=pooled_f[:])

            # ---------- MLP head (replicated)
            _hctr = [0]

            def lrelu_evict(psrc, bias_tile, bias_c, dst16, nrows=128):
                _hctr[0] += 1
                t1 = sb.tile([128, G], F32, name=f"t1_{_hctr[0]}", tag="h1")
                nc.vector.tensor_scalar(out=t1[:nrows], in0=psrc[:nrows],
                                        scalar1=bias_tile[:nrows,
                                                          bias_c:bias_c + 1],
                                        scalar2=None,
                                        op0=mybir.AluOpType.add)
                t2 = sb.tile([128, G], F32, name=f"t2_{_hctr[0]}", tag="h2")
                nc.vector.tensor_scalar(out=t2[:nrows], in0=t1[:nrows],
                                        scalar1=NEG, scalar2=None,
                                        op0=mybir.AluOpType.mult)
                nc.vector.tensor_tensor(out=dst16[:nrows], in0=t1[:nrows],
                                        in1=t2[:nrows],
                                        op=mybir.AluOpType.max)

            wc1_f = sb.tile([128, 512], F32, name="wc1_f")
            nc.sync.dma_start(out=wc1_f[:], in_=wc1[:])
            wc1_h = sb.tile([128, 512], F16, name="wc1_h")
            nc.vector.tensor_copy(out=wc1_h[:], in_=wc1_f[:])
            bc1_t = sb.tile([128, 4], F32, name="bc1_t")
            nc.sync.dma_start(out=bc1_t[:], in_=bc1[:])
            wc2_f = sb.tile([128, 4 * 256], F32, name="wc2_f")
            nc.sync.dma_start(out=wc2_f[:], in_=wc2[:])
            wc2_h = sb.tile([128, 4 * 256], F16, name="wc2_h")
            nc.vector.tensor_copy(out=wc2_h[:], in_=wc2_f[:])
            bc2_t = sb.tile([128, 2], F32, name="bc2_t")
            nc.sync.dma_start(out=bc2_t[:], in_=bc2[:])
            wc3_f = sb.tile([128, 2 * OUT], F32, name="wc3_f")
            nc.sync.dma_start(out=wc3_f[:], in_=wc3[:])
            wc3_h = sb.tile([128, 2 * OUT], F16, name="wc3_h")
            nc.vector.tensor_copy(out=wc3_h[:], in_=wc3_f[:])
            bc3_t = sb.tile([128, 1], F32, name="bc3_t")
            nc.sync.dma_start(out=bc3_t[:], in_=bc3[:])

            z1 = sb.tile([128, 4 * G], F16, name="z1")
            for c in range(4):
                ph = ps2.tile([128, G], F32, space="PSUM",
                              name=f"ph1_{c}", tag="b")
                nc.tensor.matmul(out=ph[:], lhsT=wc1_h[:, 128 * c:128 * (c + 1)],
                                 rhs=pooled[:], start=True, stop=True)
                lrelu_evict(ph, bc1_t, c, z1[:, G * c:G * (c + 1)])
            z2 = sb.tile([128, 2 * G], F16, name="z2")
            for jj in range(2):
                ph = ps2.tile([128, G], F32, space="PSUM",
                              name=f"ph2_{jj}", tag="b")
                for c in range(4):
                    nc.tensor.matmul(
                        out=ph[:],
                        lhsT=wc2_h[:, 256 * c + 128 * jj:256 * c + 128 * (jj + 1)],
                        rhs=z1[:, G * c:G * (c + 1)],
                        start=(c == 0), stop=(c == 3))
                lrelu_evict(ph, bc2_t, jj, z2[:, G * jj:G * (jj + 1)])
            ph3 = ps2.tile([128, G], F32, space="PSUM", name="ph3", tag="b")
            for c in range(2):
                nc.tensor.matmul(out=ph3[:OUT],
                                 lhsT=wc3_h[:, OUT * c:OUT * (c + 1)],
                                 rhs=z2[:, G * c:G * (c + 1)],
                                 start=(c == 0), stop=(c == 1))
            zout = sb.tile([128, G], F32, name="zout")
            lrelu_evict(ph3, bc3_t, 0, zout, nrows=OUT)
            nc.sync.dma_start(out=out_d[:], in_=zout[:OUT])

    split_multiwaits(nc)
    return nc


# ---------------------------------------------------------------- driver

def run_gcn(x, src, dst, gid, W1, b1, W2, b2, W3, b3,
            Wc1, bc1, Wc2, bc2, Wc3, bc3, n_nodes, shard,
            trace=False, tmpdir=None):
    x = np.asarray(x, np.float32)
    src = np.asarray(src).astype(np.int64)
    dst = np.asarray(dst).astype(np.int64)
    gid = np.asarray(gid).astype(np.int64)

    per_core, node_core, node_slot, bw, offs, nbw, nb3, x_nm = _prep(
        x, src, dst, gid, n_nodes, shard)
    nc = build_program(shard, nbw, nb3, bw, offs)

    iota = np.tile(np.arange(WIN, dtype=np.float32), (128, 1))
    ident = np.eye(128, dtype=np.float32)

    def colvec(v, n=128):
        a = np.zeros((n, 1), np.float32)
        a[:len(v), 0] = v
        return a

    bc1_a = np.asarray(bc1, np.float32).reshape(4, 128).T.copy()
    bc2_a = np.asarray(bc2, np.float32).reshape(2, 128).T.copy()
    wc2_a = np.concatenate(
        [np.asarray(Wc2, np.float32)[128 * c:128 * (c + 1), :] for c in range(4)],
        axis=1)
    wc3_a = np.concatenate(
        [np.asarray(Wc3, np.float32)[128 * c:128 * (c + 1), :] for c in range(2)],
        axis=1)

    in_maps = []
    for k in range(NC):
        pc = per_core[k]
        in_maps.append({
            "x_nm": x_nm,
            "w1": np.asarray(W1, np.float32), "b1": colvec(np.asarray(b1)),
            "w2": np.asarray(W2, np.float32), "b2": colvec(np.asarray(b2)),
            "w3": np.asarray(W3, np.float32), "b3": colvec(np.asarray(b3)),
            "wc1": np.asarray(Wc1, np.float32), "bc1": bc1_a,
            "wc2": wc2_a, "bc2": bc2_a,
            "wc3": wc3_a, "bc3": colvec(np.asarray(bc3)),
            "gidx": pc["gidx"], "dstloc": pc["dstloc"], "wcol": pc["wcol"],
            "q3": pc["q3"],
            "iota": iota, "ident": ident,
        })

    res = run_bass_kernel_spmd(nc, in_maps, core_ids=list(range(NC)),
                               trace=trace, tmpdir=tmpdir)
    out = res.results[0]["out"].T.astype(np.float32).copy()  # [G, OUT]
    return out, res


def kernel(**inputs):
    out, _ = run_gcn(
        inputs["inputs"], inputs["src"], inputs["dst"], inputs["graph_id"],
        inputs["W1"], inputs["b1"], inputs["W2"], inputs["b2"],
        inputs["W3"], inputs["b3"],
        inputs["Wc1"], inputs["bc1"], inputs["Wc2"], inputs["bc2"],
        inputs["Wc3"], inputs["bc3"],
        n_nodes=50000, shard=6272)
    return out


# revision 14
# speedup vs baseline: 1.7312x; 1.0840x over previous
"""GCN (3x GraphConv + mean-pool + MLP head) on 8 Trainium2 NeuronCores.

Strategy (SPMD, one program on all 8 cores):
  - Nodes are assigned to (core, 256-dst-window) buckets balanced by
    in-degree, so every core/window sees ~equal edge counts and the
    SPMD-uniform batch schedule wastes almost no padding.
  - Weight matrices replicated; degree norms folded into per-edge weights
    w_e = src_norm[src] * dst_norm[dst] carried by the scatter one-hots.
  - Per layer: project own shard (fp16 matmuls, PSUM f32) -> PE-transpose to
    node-major -> AllGather (2 pipelined chunks) into a DRAM table
    [50176, 128] fp16 -> gather each edge's source row via indirect DMA
    (128 rows/instruction, edges grouped per dst window) -> scatter-add via
    matmul with an on-chip weighted one-hot (iota==dst_local)*w_e
    accumulating in PSUM per window.
  - Layer 3's scatter is fused with dgl.mean_nodes: its one-hot is the
    8-wide graph-membership matrix scaled by w_e/cnt_g, so the whole last
    aggregation lands directly in a [128,8] pooled accumulator; partials
    are AllReduced and the tiny MLP head runs replicated on every core.
"""

import heapq
import sys

sys.path.insert(0, "/opt/trn_rl_repo")

import numpy as np

import concourse.bass as bass
import concourse.mybir as mybir
import concourse.tile as tile
import bass_rust
from concourse.bass_utils import run_bass_kernel_spmd

F32 = mybir.dt.float32
F16 = mybir.dt.float16
I32 = mybir.dt.int32

NC = 8          # cores
D = 128         # feature dim (== partition width)
G = 8           # graphs
OUT = 10
NEG = 0.01      # LeakyReLU slope
WIN = 256       # dst window per PSUM tile

_split_ctr = [0]


def split_multiwaits(nc):
    """This walrus encodes at most ONE sync-wait per instruction; hoist
    extra waits into preceding EventSemaphore ops on the same engine."""
    for f in nc.m.functions:
        for blk in f.blocks:
            insts = list(blk.instructions)
            new, changed = [], False
            for inst in insts:
                si = inst.sync_info
                if si is not None and len(si.on_wait) > 1:
                    waits = list(si.on_wait)
                    for w in waits[:-1]:
                        _split_ctr[0] += 1
                        es = mybir.InstEventSemaphore(
                            name=f"mwsplit_{_split_ctr[0]}", ins=[], outs=[])
                        es.engine = inst.engine
                        es.sync_info = bass_rust.SyncInfo(on_wait=[w], on_update=[])
                        new.append(es)
                    si.on_wait = waits[-1:]
                    changed = True
                new.append(inst)
            if changed:
                blk.instructions = new


def _chunks(shard):
    """Up to 4 roughly-equal 128-aligned table chunks covering [0, shard)."""
    ntile = shard // 128
    n = min(6, ntile)
    base = ntile // n
    sizes = [base] * n
    for i in range(ntile - base * n):
        sizes[n - 1 - i] += 1
    out, c = [], 0
    for z in sizes:
        out.append((c, z * 128))
        c += z * 128
    return out


def _windows(shard):
    """List of (start, width) dst windows covering [0, shard)."""
    out = []
    c = 0
    while c < shard:
        w = min(WIN, shard - c)
        out.append((c, w))
        c += w
    return out


# ---------------------------------------------------------------- host prep

def _prep(x, src, dst, gid, n_nodes, shard):
    wins = _windows(shard)
    nwin = len(wins)
    chunks = _chunks(shard)

    out_deg = np.bincount(src, minlength=n_nodes)
    in_deg = np.bincount(dst, minlength=n_nodes)
    snorm = np.clip(out_deg, 1, None).astype(np.float32) ** -0.5
    dnorm = np.clip(in_deg, 1, None).astype(np.float32) ** -0.5
    we = snorm[src] * dnorm[dst]
    cnt = np.bincount(gid, minlength=G).astype(np.float32)
    cinv = (1.0 / np.clip(cnt, 1, None)).astype(np.float32)

    # --- balanced node -> (core, window) assignment by in-degree
    order = np.argsort(-in_deg, kind="stable")
    heap = [(0.0, k * nwin + w) for k in range(NC) for w in range(nwin)]
    heapq.heapify(heap)
    fill = [[0] * nwin for _ in range(NC)]
    node_core = np.empty(n_nodes, np.int32)
    node_slot = np.empty(n_nodes, np.int32)
    for n in order:
        while True:
            load, b = heapq.heappop(heap)
            k, w = divmod(b, nwin)
            if fill[k][w] < wins[w][1]:
                break
        node_core[n] = k
        node_slot[n] = wins[w][0] + fill[k][w]
        fill[k][w] += 1
        heapq.heappush(heap, (load + float(in_deg[n]), b))

    # global gather-table id (matches the chunked AllGather layout):
    # chunk c of every rank is contiguous: row = NC*S_c + k*Z_c + (s - S_c)
    s = node_slot.astype(np.int64)
    kk = node_core.astype(np.int64)
    gid_tab = np.zeros(len(s), np.int64)
    for (cst, csz) in chunks:
        m = (s >= cst) & (s < cst + csz)
        gid_tab[m] = NC * cst + kk[m] * csz + (s[m] - cst)
    gid_tab = gid_tab.astype(np.int32)

    e_core = node_core[dst]
    e_slot_all = node_slot[dst]

    wcnt = np.zeros((NC, nwin), np.int64)
    for k in range(NC):
        m = e_core == k
        wcnt[k] = np.bincount(e_slot_all[m] // WIN, minlength=nwin)
    bw = np.maximum(1, -(-wcnt.max(axis=0) // 128)).astype(np.int64)
    offs = np.concatenate([[0], np.cumsum(bw)])
    nbw = int(offs[-1])
    nb3 = max(1, max(-(-int((e_core == k).sum()) // 128) for k in range(NC)))

    per_core = []
    for k in range(NC):
        m = e_core == k
        e_src = src[m].astype(np.int64)
        e_slot = e_slot_all[m].astype(np.int64)
        e_w = we[m].astype(np.float32)
        e_g = gid[dst[m]].astype(np.int64)
        o = np.argsort(e_slot, kind="stable")
        e_src, e_slot, e_w, e_g = e_src[o], e_slot[o], e_w[o], e_g[o]

        gidx = np.zeros((128, nbw), np.int32)
        dstloc = np.full((128, nbw), -1.0, np.float32)
        wcol = np.zeros((128, nbw), np.float32)
        win = e_slot // WIN
        pos = np.zeros(len(e_src), np.int64)
        for w in range(nwin):
            mm = win == w
            pos[mm] = np.arange(mm.sum())
        col = offs[win] + pos // 128
        row = pos % 128
        gidx[row, col] = gid_tab[e_src]
        dstloc[row, col] = (e_slot - win * WIN).astype(np.float32)
        wcol[row, col] = e_w

        # layer-3 SpMM folds into mean-pool: per-node pooling weights
        # q[n, g] = sum_{e: src=n} w_e * cinv[g] * [graph(dst_e) == g]
        qk = np.zeros((shard, G), np.float32)
        loc = node_slot[e_src]          # unused; q is per SRC node below
        per_core.append(dict(gidx=gidx, dstloc=dstloc, wcol=wcol))
    # q over all nodes, then scatter into each core's slot order
    q_full = np.zeros((n_nodes, G), np.float32)
    np.add.at(q_full, (src, gid[dst]), we * cinv[gid[dst]])
    ntile = shard // 128
    x_nm = np.zeros((NC * shard, D), np.float32)
    x_nm[gid_tab] = x
    for k in range(NC):
        mk = node_core == k
        qk = np.zeros((shard, G), np.float32)
        qk[node_slot[mk]] = q_full[mk]
        per_core[k]["q3"] = np.ascontiguousarray(
            qk.reshape(ntile, 128, G).transpose(1, 0, 2).reshape(128, ntile * G))
    return per_core, node_core, node_slot, bw, offs, nbw, nb3, x_nm


# ---------------------------------------------------------------- program

def build_program(shard, nbw, nb3, bw, offs):
    npad = NC * shard
    wins = _windows(shard)
    nwin = len(wins)
    chunks = _chunks(shard)
    # window index after which each chunk's columns are fully evicted
    trig = []
    for (cst, csz) in chunks:
        w = 0
        while wins[w][0] + wins[w][1] < cst + csz:
            w += 1
        trig.append(w)

    nc = bass.Bass("TRN2", target_bir_lowering=False, debug=False,
                   num_devices=NC)

    x_nm = nc.dram_tensor("x_nm", [npad, D], F32, kind="ExternalInput")
    wg = [nc.dram_tensor(f"w{l}", [128, 128], F32, kind="ExternalInput")
          for l in (1, 2, 3)]
    bg = [nc.dram_tensor(f"b{l}", [128, 1], F32, kind="ExternalInput")
          for l in (1, 2, 3)]
    wc1 = nc.dram_tensor("wc1", [128, 512], F32, kind="ExternalInput")
    bc1 = nc.dram_tensor("bc1", [128, 4], F32, kind="ExternalInput")
    wc2 = nc.dram_tensor("wc2", [128, 4 * 256], F32, kind="ExternalInput")
    bc2 = nc.dram_tensor("bc2", [128, 2], F32, kind="ExternalInput")
    wc3 = nc.dram_tensor("wc3", [128, 2 * OUT], F32, kind="ExternalInput")
    bc3 = nc.dram_tensor("bc3", [128, 1], F32, kind="ExternalInput")
    gidx_in = nc.dram_tensor("gidx", [128, nbw], I32, kind="ExternalInput")
    dstloc_in = nc.dram_tensor("dstloc", [128, nbw], F32, kind="ExternalInput")
    wcol_in = nc.dram_tensor("wcol", [128, nbw], F32, kind="ExternalInput")
    ntile = shard // 128
    q3_in = nc.dram_tensor("q3", [128, ntile * G], F32, kind="ExternalInput")
    iota_in = nc.dram_tensor("iota", [128, WIN], F32, kind="ExternalInput")
    ident_in = nc.dram_tensor("ident", [128, 128], F32, kind="ExternalInput")
    out_d = nc.dram_tensor("out", [OUT, G], F32, kind="ExternalOutput")

    bounce = {1: nc.dram_tensor("bnc1", [shard, 128], F16)}
    table = {1: nc.dram_tensor("tbl1", [npad, 128], F16)}
    pool_in = nc.dram_tensor("pool_in", [128, G], F32)
    pool_out = nc.dram_tensor("pool_out", [128, G], F32)

    with tile.TileContext(nc) as tc:
        with tc.tile_pool(name="sb", bufs=1) as sb, \
             tc.tile_pool(name="ps", bufs=4, space="PSUM") as ps, \
             tc.tile_pool(name="ps2", bufs=2, space="PSUM") as ps2:

            # critical-path loads first: gather schedule + weights
            ident_t = sb.tile([128, 128], F32, name="ident_t")
            nc.sync.dma_start(out=ident_t[:], in_=ident_in[:])
            w_t, b_t = [], []
            for l in range(3):
                wf = sb.tile([128, 128], F32, name=f"wf{l}")
                nc.sync.dma_start(out=wf[:], in_=wg[l][:])
                wh = sb.tile([128, 128], F16, name=f"wh{l}")
                nc.vector.tensor_copy(out=wh[:], in_=wf[:])
                w_t.append(wh)
                bt = sb.tile([128, 1], F32, name=f"bt{l}")
                nc.sync.dma_start(out=bt[:], in_=bg[l][:])
                b_t.append(bt)
            iota_t = sb.tile([128, WIN], F32, name="iota_t")
            nc.sync.dma_start(out=iota_t[:], in_=iota_in[:])
            gidx_t = sb.tile([128, nbw], I32, name="gidx_t")
            nc.sync.dma_start(out=gidx_t[:], in_=gidx_in[:])
            dstloc_t = sb.tile([128, nbw], F32, name="dstloc_t")
            nc.sync.dma_start(out=dstloc_t[:], in_=dstloc_in[:])
            wcol_t = sb.tile([128, nbw], F32, name="wcol_t")
            nc.sync.dma_start(out=wcol_t[:], in_=wcol_in[:])
            q3_f32 = sb.tile([128, ntile * G], F32, name="q3_f32")
            nc.sync.dma_start(out=q3_f32[:], in_=q3_in[:])
            q3_t = sb.tile([128, ntile * G], F16, name="q3_t")
            nc.vector.tensor_copy(out=q3_t[:], in_=q3_f32[:])

            def emit_chunk(l, state_t, cst, csz):
                """Project chunk [cst, cst+csz) of layer l's state, transpose
                to node-major, bounce to DRAM, AllGather into table[l]."""
                xp = sb.tile([128, csz], F32, name=f"xp{l}_{cst}", tag="xpf")
                c = 0
                while c < csz:
                    w = min(512, csz - c)
                    pt = ps2.tile([128, w], F32, space="PSUM",
                                  name=f"pj{l}_{cst}_{c}", tag="c")
                    nc.tensor.matmul(out=pt[:], lhsT=w_t[l][:],
                                     rhs=state_t[:, cst + c:cst + c + w],
                                     start=True, stop=True)
                    nc.vector.tensor_copy(out=xp[:, c:c + w], in_=pt[:])
                    c += w
                nck = csz // 128
                xpnm = sb.tile([128, nck, 128], F16,
                               name=f"xpnm{l}_{cst}", tag="xpnm")
                for t in range(nck):
                    tp = ps2.tile([128, 128], F32, space="PSUM",
                                  name=f"tp{l}_{cst}_{t}", tag="c")
                    nc.tensor.transpose(out=tp[:],
                                        in_=xp[:, 128 * t:128 * (t + 1)],
                                        identity=ident_t[:])
                    nc.vector.tensor_copy(out=xpnm[:, t, :], in_=tp[:])
                nc.sync.dma_start(
                    out=bounce[l][cst:cst + csz, :]
                        .rearrange("(c p) f -> p c f", p=128),
                    in_=xpnm[:])
                tb0 = NC * cst

                def fire(l=l, cst=cst, csz=csz, tb0=tb0):
                    nc.gpsimd.collective_compute(
                        "AllGather", mybir.AluOpType.bypass,
                        replica_groups=[list(range(NC))],
                        ins=[bounce[l][cst:cst + csz, :]],
                        outs=[table[l][tb0:tb0 + NC * csz, :]])
                return fire

            # gathers land in per-window grouped tiles so the WAR wait is
            # per-window (4-slot rotation), not per-gather
            def gather_group(l, cols, src_idx_t, gname, gtag):
                srct = x_nm if l == 0 else table[1]
                dt = F32 if l == 0 else F16
                mt = sb.tile([128, len(cols) * 128], dt, name=gname,
                             tag=gtag)
                views = []
                for i, col in enumerate(cols):
                    v = mt[:, 128 * i:128 * (i + 1)]
                    nc.gpsimd.indirect_dma_start(
                        out=v, out_offset=None, in_=srct[:],
                        in_offset=bass.IndirectOffsetOnAxis(
                            ap=src_idx_t[:, col:col + 1], axis=0))
                    views.append(v)
                return views

            # layer-3 x W3 projection folds into pooling: per chunk,
            # project+transpose state2 and contract with q3 into pp
            pp = ps2.tile([128, G], F32, space="PSUM", name="pp", tag="b")
            _pcnt = [0]

            def emit_pool_chunk(state_t, cst, csz):
                xp = sb.tile([128, csz], F32, name=f"xpq_{cst}", tag="xpf")
                c = 0
                while c < csz:
                    w = min(512, csz - c)
                    pt = ps2.tile([128, w], F32, space="PSUM",
                                  name=f"pjq_{cst}_{c}", tag="c")
                    nc.tensor.matmul(out=pt[:], lhsT=w_t[2][:],
                                     rhs=state_t[:, cst + c:cst + c + w],
                                     start=True, stop=True)
                    nc.vector.tensor_copy(out=xp[:, c:c + w], in_=pt[:])
                    c += w
                nck = csz // 128
                xpnm = sb.tile([128, nck, 128], F16,
                               name=f"xpnmq_{cst}", tag="xpnm")
                for t in range(nck):
                    tp = ps2.tile([128, 128], F32, space="PSUM",
                                  name=f"tpq_{cst}_{t}", tag="c")
                    nc.tensor.transpose(out=tp[:],
                                        in_=xp[:, 128 * t:128 * (t + 1)],
                                        identity=ident_t[:])
                    nc.vector.tensor_copy(out=xpnm[:, t, :], in_=tp[:])
                    gt = cst // 128 + t
                    nc.tensor.matmul(out=pp[:], lhsT=xpnm[:, t, :],
                                     rhs=q3_t[:, G * gt:G * (gt + 1)],
                                     start=(_pcnt[0] == 0),
                                     stop=(_pcnt[0] == ntile - 1))
                    _pcnt[0] += 1

            # layer 0's table comes straight from the input state

            for l in range(2):
                nstate = sb.tile([128, shard], F16, name=f"state{l + 1}")
                pending = []
                for w in range(nwin):
                    wst, ww = wins[w]
                    pw = ps.tile([128, ww], F32, space="PSUM",
                                 name=f"pw{l}_{w}", tag="a")
                    nb = int(bw[w])
                    cols = [int(offs[w]) + j for j in range(nb)]
                    mviews = gather_group(l, cols, gidx_t,
                                          f"mg{l}_{w}", f"gw{w % 6}")
                    for j, col in enumerate(cols):
                        if l == 0:
                            mh = sb.tile([128, 128], F16, name=f"mh_{col}",
                                         tag=f"mh{col % 16}")
                            nc.vector.tensor_copy(out=mh[:], in_=mviews[j])
                            lhs = mh[:]
                        else:
                            lhs = mviews[j]
                        oh = sb.tile([128, ww], F16, name=f"oh{l}_{col}",
                                     tag=f"oh{col % 16}")
                        nc.vector.tensor_scalar(
                            out=oh[:], in0=iota_t[:, :ww],
                            scalar1=dstloc_t[:, col:col + 1],
                            scalar2=wcol_t[:, col:col + 1],
                            op0=mybir.AluOpType.is_equal,
                            op1=mybir.AluOpType.mult)
                        nc.tensor.matmul(out=pw[:], lhsT=lhs, rhs=oh[:],
                                         start=(j == 0), stop=(j == nb - 1))
                    if l == 0:
                        # agg is over raw x; apply the commuted W1 here
                        aggs = sb.tile([128, ww], F16, name=f"aggs_{w}",
                                       tag=f"ag{w % 4}")
                        nc.vector.tensor_copy(out=aggs[:], in_=pw[:])
                        ph = ps2.tile([128, ww], F32, space="PSUM",
                                      name=f"phw_{w}", tag="c")
                        nc.tensor.matmul(out=ph[:], lhsT=w_t[0][:],
                                         rhs=aggs[:], start=True, stop=True)
                        nc.vector.tensor_scalar(
                            out=nstate[:, wst:wst + ww], in0=ph[:],
                            scalar1=b_t[l][:, 0:1], scalar2=None,
                            op0=mybir.AluOpType.add)
                    else:
                        nc.vector.tensor_scalar(
                            out=nstate[:, wst:wst + ww], in0=pw[:],
                            scalar1=b_t[l][:, 0:1], scalar2=None,
                            op0=mybir.AluOpType.add)
                    # fire the previous window's deferred collective now
                    # that this window's gathers are queued ahead of it
                    for f in pending:
                        f()
                    pending = []
                    # emit the next stage's work as soon as the columns
                    # are final -- overlaps collectives/PE with gathers
                    for ci, (cst, csz) in enumerate(chunks):
                        if trig[ci] == w:
                            if l == 0:
                                pending.append(
                                    emit_chunk(1, nstate, cst, csz))
                            else:
                                emit_pool_chunk(nstate, cst, csz)
                for f in pending:
                    f()
                state = nstate

            # ---------- pooled accumulator already filled by emit_pool_chunk
            ppool = sb.tile([128, G], F32, name="ppool")
            nc.vector.tensor_copy(out=ppool[:], in_=pp[:])
            nc.sync.dma_start(out=pool_in[:], in_=ppool[:])
            nc.gpsimd.collective_compute(
                "AllReduce", mybir.AluOpType.add,
                replica_groups=[list(range(NC))],
                ins=[pool_in.ap()], outs=[pool_out.ap()])
            pooled_f = sb.tile([128, G], F32, name="pooled_f")
            nc.sync.dma_start(out=pooled_f[:], in_=pool_out[:])
            nc.vector.tensor_scalar(out=pooled_f[:], in0=pooled_f[:],
                                    scalar1=b_t[2][:, 0:1], scalar2=None,
                                    op0=mybir.AluOpType.add)
            pooled = sb.tile([128, G], F16, name="pooled")
            nc.vector.tensor_copy(out=pooled[:], in_=pooled_f[:])

            # ---------- MLP head (replicated)
            _hctr = [0]

            def lrelu_evict(psrc, bias_tile, bias_c, dst16, nrows=128):
                _hctr[0] += 1
                t1 = sb.tile([128, G], F32, name=f"t1_{_hctr[0]}", tag="h1")
                nc.vector.tensor_scalar(out=t1[:nrows], in0=psrc[:nrows],
                                        scalar1=bias_tile[:nrows,
                                                          bias_c:bias_c + 1],
                                        scalar2=None,
                                        op0=mybir.AluOpType.add)
                t2 = sb.tile([128, G], F32, name=f"t2_{_hctr[0]}", tag="h2")
                nc.vector.tensor_scalar(out=t2[:nrows], in0=t1[:nrows],
                                        scalar1=NEG, scalar2=None,
                                        op0=mybir.AluOpType.mult)
                nc.vector.tensor_tensor(out=dst16[:nrows], in0=t1[:nrows],
                                        in1=t2[:nrows],
                                        op=mybir.AluOpType.max)

            wc1_f = sb.tile([128, 512], F32, name="wc1_f")
            nc.sync.dma_start(out=wc1_f[:], in_=wc1[:])
            wc1_h = sb.tile([128, 512], F16, name="wc1_h")
            nc.vector.tensor_copy(out=wc1_h[:], in_=wc1_f[:])
            bc1_t = sb.tile([128, 4], F32, name="bc1_t")
            nc.sync.dma_start(out=bc1_t[:], in_=bc1[:])
            wc2_f = sb.tile([128, 4 * 256], F32, name="wc2_f")
            nc.sync.dma_start(out=wc2_f[:], in_=wc2[:])
            wc2_h = sb.tile([128, 4 * 256], F16, name="wc2_h")
            nc.vector.tensor_copy(out=wc2_h[:], in_=wc2_f[:])
            bc2_t = sb.tile([128, 2], F32, name="bc2_t")
            nc.sync.dma_start(out=bc2_t[:], in_=bc2[:])
            wc3_f = sb.tile([128, 2 * OUT], F32, name="wc3_f")
            nc.sync.dma_start(out=wc3_f[:], in_=wc3[:])
            wc3_h = sb.tile([128, 2 * OUT], F16, name="wc3_h")
            nc.vector.tensor_copy(out=wc3_h[:], in_=wc3_f[:])
            bc3_t = sb.tile([128, 1], F32, name="bc3_t")
            nc.sync.dma_start(out=bc3_t[:], in_=bc3[:])

            z1 = sb.tile([128, 4 * G], F16, name="z1")
            for c in range(4):
                ph = ps2.tile([128, G], F32, space="PSUM",
                              name=f"ph1_{c}", tag="b")
                nc.tensor.matmul(out=ph[:], lhsT=wc1_h[:, 128 * c:128 * (c + 1)],
                                 rhs=pooled[:], start=True, stop=True)
                lrelu_evict(ph, bc1_t, c, z1[:, G * c:G * (c + 1)])
            z2 = sb.tile([128, 2 * G], F16, name="z2")
            for jj in range(2):
                ph = ps2.tile([128, G], F32, space="PSUM",
                              name=f"ph2_{jj}", tag="b")
                for c in range(4):
                    nc.tensor.matmul(
                        out=ph[:],
                        lhsT=wc2_h[:, 256 * c + 128 * jj:256 * c + 128 * (jj + 1)],
                        rhs=z1[:, G * c:G * (c + 1)],
                        start=(c == 0), stop=(c == 3))
                lrelu_evict(ph, bc2_t, jj, z2[:, G * jj:G * (jj + 1)])
            ph3 = ps2.tile([128, G], F32, space="PSUM", name="ph3", tag="b")
            for c in range(2):
                nc.tensor.matmul(out=ph3[:OUT],
                                 lhsT=wc3_h[:, OUT * c:OUT * (c + 1)],
                                 rhs=z2[:, G * c:G * (c + 1)],
                                 start=(c == 0), stop=(c == 1))
            zout = sb.tile([128, G], F32, name="zout")
            lrelu_evict(ph3, bc3_t, 0, zout, nrows=OUT)
            nc.sync.dma_start(out=out_d[:], in_=zout[:OUT])

    split_multiwaits(nc)
    return nc


# ---------------------------------------------------------------- driver

def run_gcn(x, src, dst, gid, W1, b1, W2, b2, W3, b3,
            Wc1, bc1, Wc2, bc2, Wc3, bc3, n_nodes, shard,
            trace=False, tmpdir=None):
    x = np.asarray(x, np.float32)
    src = np.asarray(src).astype(np.int64)
    dst = np.asarray(dst).astype(np.int64)
    gid = np.asarray(gid).astype(np.int64)

    per_core, node_core, node_slot, bw, offs, nbw, nb3, x_nm = _prep(
        x, src, dst, gid, n_nodes, shard)
    nc = build_program(shard, nbw, nb3, bw, offs)

    iota = np.tile(np.arange(WIN, dtype=np.float32), (128, 1))
    ident = np.eye(128, dtype=np.float32)

    def colvec(v, n=128):
        a = np.zeros((n, 1), np.float32)
        a[:len(v), 0] = v
        return a

    bc1_a = np.asarray(bc1, np.float32).reshape(4, 128).T.copy()
    bc2_a = np.asarray(bc2, np.float32).reshape(2, 128).T.copy()
    wc2_a = np.concatenate(
        [np.asarray(Wc2, np.float32)[128 * c:128 * (c + 1), :] for c in range(4)],
        axis=1)
    wc3_a = np.concatenate(
        [np.asarray(Wc3, np.float32)[128 * c:128 * (c + 1), :] for c in range(2)],
        axis=1)

    in_maps = []
    for k in range(NC):
        pc = per_core[k]
        in_maps.append({
            "x_nm": x_nm,
            "w1": np.asarray(W1, np.float32), "b1": colvec(np.asarray(b1)),
            "w2": np.asarray(W2, np.float32), "b2": colvec(np.asarray(b2)),
            "w3": np.asarray(W3, np.float32), "b3": colvec(np.asarray(b3)),
            "wc1": np.asarray(Wc1, np.float32), "bc1": bc1_a,
            "wc2": wc2_a, "bc2": bc2_a,
            "wc3": wc3_a, "bc3": colvec(np.asarray(bc3)),
            "gidx": pc["gidx"], "dstloc": pc["dstloc"], "wcol": pc["wcol"],
            "q3": pc["q3"],
            "iota": iota, "ident": ident,
        })

    res = run_bass_kernel_spmd(nc, in_maps, core_ids=list(range(NC)),
                               trace=trace, tmpdir=tmpdir)
    out = res.results[0]["out"].T.astype(np.float32).copy()  # [G, OUT]
    return out, res


def kernel(**inputs):
    out, _ = run_gcn(
        inputs["inputs"], inputs["src"], inputs["dst"], inputs["graph_id"],
        inputs["W1"], inputs["b1"], inputs["W2"], inputs["b2"],
        inputs["W3"], inputs["b3"],
        inputs["Wc1"], inputs["bc1"], inputs["Wc2"], inputs["bc2"],
        inputs["Wc3"], inputs["bc3"],
        n_nodes=50000, shard=6272)
    return out
